# revision 1
# baseline (speedup 1.0000x reference)
"""CameraHead Trainium2 kernel — data-parallel over b*v across 8 NeuronCores.

Per-core layout: activations live feature-major in SBUF (X^T: [feat(4x128 part
chunks), tokens]), so every Linear is out = W^T_chunk.T @ X^T_chunk accumulated
over 4 K-chunks on the PE, and layer outputs come out feature-major again.
The host pre-transposes each core's token shard once (cheap numpy) so the DMA
loads are fully contiguous.

Pipeline per core (32 samples x 256 tokens = 8192 token rows):
  - 16 token-tiles of 512: 6 fused Linear+ReLU layers (PE matmul fp32r at
    1 cyc/row + ACT relu+bias), residual adds on DVE, per-sample pooling
    reduces on DVE.
  - tail: 2 small MLP layers, fused heads (t + rot in one [32,12] matmul),
    branchless 3x3 SVD -> SO(3) (Jacobi eigensolver with quaternion
    accumulation, McAdams-style) entirely on DVE/ACT, pose assembly.
Returns the full (16,16,4,4) pose tensor.
"""
import sys
import numpy as np

sys.path.insert(0, '/opt/trn_rl_repo')

import concourse.bacc as bacc  # noqa: E402
import concourse.mybir as mybir  # noqa: E402
from concourse import tile  # noqa: E402
from concourse import dve_ops as _dvo  # noqa: E402
from concourse.bass_utils import run_bass_kernel_spmd  # noqa: E402
from concourse.dve_spec import (  # noqa: E402
    C0, C1, C2, One, Spec, Src0, Src1, select as dve_select, sq as dve_sq,
)


def _reg_op(name, body, ref):
    """Register a custom DVE op (per-NEFF uop table; no firmware change).

    The uops sha pin is bootstrapped by parsing compile()'s drift error."""
    for op in _dvo.OPS:
        if op.name == name:
            return op
    import re as _re

    from concourse.dve_table_gen import dve_ver_for

    row = _dvo._CUSTOM_DVE_ROW_BASE + len(_dvo.OPS)
    assert row < 0x20, "custom DVE opcode rows exhausted"
    spec = Spec(body=body, reference=ref)
    op = _dvo.DveOp(name, spec, subdim=False, uops_sha={})
    _dvo.OPS.append(op)
    _dvo._SUB_OPCODE_FOR_NAME[name] = row
    _dvo.CUSTOM_DVE_SPECS[name] = spec
    ver = dve_ver_for("TRN2")
    try:
        op.compile(ver)
    except ValueError as e:
        m = _re.search(r'uops_sha\["' + ver + r'"\]="([0-9a-f]+)"', str(e))
        if not m:
            raise
        op.uops_sha[ver] = m.group(1)
        op.compile(ver)
    return op


_f32 = np.float32
OP_AXPBY = _reg_op(
    "ANT_AXPBY", Src0 * C0 + Src1 * C1,
    lambda in0, in1, s0, s1, imm2: (in0 * s0 + in1 * s1).astype(_f32))
OP_AXMBY = _reg_op(
    "ANT_AXMBY", Src0 * C0 - Src1 * C1,
    lambda in0, in1, s0, s1, imm2: (in0 * s0 - in1 * s1).astype(_f32))
OP_AXPBY2 = _reg_op(
    "ANT_AXPBY2", (Src0 * C0 + Src1 * C1) * C2,
    lambda in0, in1, s0, s1, imm2: ((in0 * s0 + in1 * s1) * imm2).astype(_f32))
OP_AXMBY2 = _reg_op(
    "ANT_AXMBY2", (Src0 * C0 - Src1 * C1) * C2,
    lambda in0, in1, s0, s1, imm2: ((in0 * s0 - in1 * s1) * imm2).astype(_f32))
OP_SELNA = _reg_op(
    "ANT_SELNA", dve_select(dve_sq(Src1) * C1 < dve_sq(Src0), Src0 * C0, C2),
    lambda in0, in1, s0, s1, imm2: np.where(
        in1 * in1 * s1 < in0 * in0, in0 * s0, imm2).astype(_f32))
OP_SELNB = _reg_op(
    "ANT_SELNB", dve_select(dve_sq(Src1) * C1 < dve_sq(Src0), Src1 * C0, C2),
    lambda in0, in1, s0, s1, imm2: np.where(
        in1 * in1 * s1 < in0 * in0, in1 * s0, imm2).astype(_f32))
OP_SQDIFF = _reg_op(
    "ANT_SQDIFF", dve_sq(Src0) - dve_sq(Src1),
    lambda in0, in1, s0, s1, imm2: (in0 * in0 - in1 * in1).astype(_f32))


def _xy2_body():
    t = Src0 * Src1
    return t + t


OP_XY2 = _reg_op(
    "ANT_XY2", _xy2_body(),
    lambda in0, in1, s0, s1, imm2: (2.0 * in0 * in1).astype(_f32))
OP_WHERE = _reg_op(
    "ANT_WHERE", dve_select(C0, Src0, Src1),
    lambda in0, in1, s0, s1, imm2: np.where(
        s0 != 0, in0, in1).astype(_f32))
OP_WHERENEG = _reg_op(
    "ANT_WHERENEG", dve_select(C0, -Src0, Src1),
    lambda in0, in1, s0, s1, imm2: np.where(
        s0 != 0, -in0, in1).astype(_f32))
OP_VDIAG = _reg_op(
    "ANT_VDIAG", One - (dve_sq(Src0) + dve_sq(Src1)) * C2,
    lambda in0, in1, s0, s1, imm2: (
        1.0 - (in0 * in0 + in1 * in1) * imm2).astype(_f32))
# c from (ch2, sh2): select(gamma*sh2 < ch2, (ch2-sh2)*w2, cos(pi/4))
OP_SELC2 = _reg_op(
    "ANT_SELC2",
    dve_select(Src1 * C1 < Src0, (Src0 - Src1) * C0, C2),
    lambda in0, in1, s0, s1, imm2: np.where(
        in1 * s1 < in0, (in0 - in1) * s0, imm2).astype(_f32))


def _xyw_body():
    t = Src0 * Src1
    return (t + t) * C0


OP_XYW2 = _reg_op(
    "ANT_XYW2", _xyw_body(),
    lambda in0, in1, s0, s1, imm2: (2.0 * in0 * in1 * s0).astype(_f32))

F32 = mybir.dt.float32
F32R = mybir.dt.float32r
AF = mybir.ActivationFunctionType
ALU = mybir.AluOpType
AX = mybir.AxisListType

N_CORES = 8
D = 512
SAMPLES = 256          # b*v
TOK = 256              # tokens per sample
S_CORE = SAMPLES // N_CORES       # 32 samples per core
T_CORE = S_CORE * TOK             # 8192 token rows per core
T_TILE = 512
N_TILES = T_CORE // T_TILE        # 16
S_TILE = T_TILE // TOK            # 2 samples per token tile

USE_F32R = True        # fp32r matmuls for the 6 big layers (4x PE throughput)
N_SWEEPS = 4           # Jacobi sweeps
N_ROTATIONS = 11       # 11 == 12 in accuracy on this data (1.8e-6)

GAMMA = float(3.0 + 2.0 * np.sqrt(2.0))
CS8 = float(np.cos(np.pi / 8))
SS8 = float(np.sin(np.pi / 8))
CQ45 = float(np.cos(np.pi / 4))


# ---------------------------------------------------------------------------
# small-op emitter for the SVD tail: SSA-style column allocation on a scratch
# tile; every value is an AP (or list of APs).
# ---------------------------------------------------------------------------
class Emit:
    def __init__(self, nc, pool):
        self.nc = nc
        self.scr = pool.tile([32, 2048], F32, tag="svd_scratch", name="svd_scratch")
        self.ptr = 0

    def new(self, n=1):
        c = self.ptr
        self.ptr += n
        assert self.ptr <= 2048, "svd scratch overflow"
        return self.scr[:, c:c + n]

    # --- primitive wrappers (each returns the output AP) ---
    def tt(self, op, a, b, n=1):
        o = self.new(n)
        self.nc.vector.tensor_tensor(o, a, b, op)
        return o

    def tt3(self, op, a, b, n=9):
        """3D-free-AP tensor_tensor writing n contiguous cols."""
        o = self.new(n)
        self.nc.vector.tensor_tensor(
            o.rearrange("p (i j) -> p i j", i=3, j=n // 3), a, b, op)
        return o

    def ts(self, op, a, s, n=1):
        o = self.new(n)
        self.nc.vector.tensor_scalar(o, a, s, None, op)
        return o

    def stt(self, a, scal, b, op0, op1, n=1):
        """(a op0 scal) op1 b ; scal is float or [32,1] AP"""
        o = self.new(n)
        self.nc.vector.scalar_tensor_tensor(o, a, scal, b, op0=op0, op1=op1)
        return o

    def rsqrt(self, a, n=1):
        t = self.new(n)
        self.nc.scalar.activation(t, a, AF.Sqrt)
        o = self.new(n)
        self.nc.vector.reciprocal(o, t)
        return o

    def cdve(self, op, in0, in1, s0=0.0, s1=0.0, imm2=0.0, n=1, out=None):
        if out is None:
            out = self.new(n)
        self.nc.vector._custom_dve(op, out=out, in0=in0, in1=in1,
                                   s0=s0, s1=s1, imm2=imm2)
        return out

    def sel(self, mask, a, b, n=1):
        """where(mask, a, b) = (a-b)*mask + b ; mask is [32,1] AP, a/b [32,n]"""
        d = self.tt(ALU.subtract, a, b, n)
        return self.stt(d, mask, b, ALU.mult, ALU.add, n)

    def const(self, val, n=1):
        o = self.new(n)
        self.nc.vector.memset(o, val)
        return o


def _bcast_r(ap3):
    """[32,3] -> [32,3,3] broadcasting along the inner (new last) dim."""
    return ap3.unsqueeze(2).broadcast_to([32, 3, 3])


def _bcast_l(ap3):
    """[32,3] -> [32,3,3] broadcasting along the outer dim."""
    return ap3.unsqueeze(1).broadcast_to([32, 3, 3])


def emit_svd_so3(nc, em, m_ap, pose_tile):
    """m_ap: [32,9] raw 3x3 per sample (row-major). Writes the SO(3) projection
    into pose_tile columns (4r+c for r,c in 0..2)."""
    # --- row normalize ---
    sq = em.tt(ALU.mult, m_ap, m_ap, 9)
    t = em.tt(ALU.add, sq[:, 0:9:3], sq[:, 1:9:3], 3)
    r2 = em.tt(ALU.add, t, sq[:, 2:9:3], 3)
    r2c = em.ts(ALU.max, r2, 1e-24, 3)
    rinv = em.rsqrt(r2c, 3)
    A = em.tt3(ALU.mult, m_ap.rearrange("p (r c) -> p r c", r=3, c=3),
               _bcast_r(rinv), 9)

    # --- S = A^T A (full 9, s_ij at 3i+j) ---
    terms = []
    for r in range(3):
        arow = A[:, 3 * r:3 * r + 3]
        terms.append(em.tt3(ALU.mult, _bcast_r(arow), _bcast_l(arow), 9))
    s01 = em.tt(ALU.add, terms[0], terms[1], 9)
    S9 = em.tt(ALU.add, s01, terms[2], 9)

    # S entries as single-col APs, tracked SSA-style
    S = {}
    for i in range(3):
        for j in range(i, 3):
            S[(i, j)] = S9[:, 3 * i + j:3 * i + j + 1]

    def sk(i, j):
        return S[(i, j)] if i <= j else S[(j, i)]

    def set_sk(i, j, ap):
        S[(i, j) if i <= j else (j, i)] = ap

    # V accumulated directly as three [32,3] column blocks, init = identity
    C45 = em.const(float(np.cos(np.pi / 4)))
    Vc = []
    for j in range(3):
        vj = em.new(3)
        nc.vector.memset(vj, 0.0)
        nc.vector.memset(vj[:, j:j + 1], 1.0)
        Vc.append(vj)

    def rotation(p, q_, r_):
        spp, sqq, spq = sk(p, p), sk(q_, q_), sk(p, q_)
        spr, sqr = sk(p, r_), sk(q_, r_)
        # approximate Givens full-angle (c, s) with pi/4 fallback; no sqrt:
        #   c = (ch^2-sh^2)/(ch^2+sh^2), s = 2 ch sh/(ch^2+sh^2)
        ch = em.tt(ALU.subtract, spp, sqq)
        sh = em.ts(ALU.mult, spq, 0.5)
        ch2 = em.tt(ALU.mult, ch, ch)
        sh2 = em.tt(ALU.mult, sh, sh)
        ssum = em.tt(ALU.add, ch2, sh2)
        w2 = em.new(1)
        nc.vector.reciprocal(w2, ssum)
        c = em.cdve(OP_SELC2, ch2, sh2, s0=w2, s1=GAMMA, imm2=CQ45)
        mask = em.stt(sh2, GAMMA, ch2, ALU.mult, ALU.is_lt)
        s_raw = em.cdve(OP_XYW2, ch, sh, s0=w2)
        s = em.cdve(OP_WHERE, s_raw, C45, s0=mask)
        # S update via nested linear combos:
        #   spp' = c*(c*spp + s*spq) + s*(c*spq + s*sqq)
        #   sqq' = s*(s*spp - c*spq) - c*(s*spq - c*sqq)
        #   spq' = c*(s*sqq + c*spq) - s*(c*spp + s*spq)
        A = em.cdve(OP_AXPBY, spp, spq, s0=c, s1=s)
        B = em.cdve(OP_AXPBY, spq, sqq, s0=c, s1=s)
        npp = em.cdve(OP_AXPBY, A, B, s0=c, s1=s)
        A2 = em.cdve(OP_AXMBY, spp, spq, s0=s, s1=c)
        B2 = em.cdve(OP_AXMBY, spq, sqq, s0=s, s1=c)
        nqq = em.cdve(OP_AXMBY, A2, B2, s0=s, s1=c)
        Ce = em.cdve(OP_AXPBY, sqq, spq, s0=s, s1=c)
        npq = em.cdve(OP_AXMBY, Ce, A, s0=c, s1=s)
        npr = em.cdve(OP_AXPBY, spr, sqr, s0=c, s1=s)
        nqr = em.cdve(OP_AXMBY, sqr, spr, s0=c, s1=s)
        set_sk(p, p, npp)
        set_sk(q_, q_, nqq)
        set_sk(p, q_, npq)
        set_sk(p, r_, npr)
        set_sk(q_, r_, nqr)
        # V update: vp' = c vp + s vq ; vq' = c vq - s vp
        nvp = em.cdve(OP_AXPBY, Vc[p], Vc[q_], s0=c, s1=s, n=3)
        nvq = em.cdve(OP_AXMBY, Vc[q_], Vc[p], s0=c, s1=s, n=3)
        Vc[p], Vc[q_] = nvp, nvq

    seq = [(0, 1, 2), (1, 2, 0), (2, 0, 1)] * N_SWEEPS
    for (p, q_, r_) in seq[:N_ROTATIONS]:
        rotation(p, q_, r_)

    # --- sort eigenpairs descending (keep det(V)=+1) ---
    lam = [sk(0, 0), sk(1, 1), sk(2, 2)]

    def cond_swap(i, j):
        mask = em.tt(ALU.is_lt, lam[i], lam[j])
        li = em.cdve(OP_WHERE, lam[j], lam[i], s0=mask)
        lj = em.cdve(OP_WHERE, lam[i], lam[j], s0=mask)
        lam[i], lam[j] = li, lj
        vi = em.cdve(OP_WHERE, Vc[j], Vc[i], s0=mask, n=3)
        vj = em.cdve(OP_WHERENEG, Vc[i], Vc[j], s0=mask, n=3)
        Vc[i], Vc[j] = vi, vj

    cond_swap(0, 1)
    cond_swap(1, 2)
    cond_swap(0, 1)

    # --- B columns (j=0,1): b_j[r] = sum_c A[r][c] * V[c][j] ---
    Astr = [A[:, c:c + 7:3] for c in range(3)]   # A[:,c::3] -> a[r][c] over r

    def bcol(j):
        t0 = em.cdve(OP_AXPBY, Astr[0], Astr[1],
                     s0=Vc[j][:, 0:1], s1=Vc[j][:, 1:2], n=3)
        return em.stt(Astr[2], Vc[j][:, 2:3], t0, ALU.mult, ALU.add, 3)

    b0 = bcol(0)
    b1 = bcol(1)

    def normalize(v3):
        sqv = em.tt(ALU.mult, v3, v3, 3)
        n_ = em.tt(ALU.add, sqv[:, 0:1], sqv[:, 1:2])
        n_ = em.tt(ALU.add, n_, sqv[:, 2:3])
        nc_ = em.ts(ALU.max, n_, 1e-30)
        inv = em.rsqrt(nc_)
        return em.ts(ALU.mult, v3, inv, 3)

    u1 = normalize(b0)
    # Gram-Schmidt b1 against u1
    p_ = em.tt(ALU.mult, u1, b1, 3)
    d_ = em.tt(ALU.add, p_[:, 0:1], p_[:, 1:2])
    d_ = em.tt(ALU.add, d_, p_[:, 2:3])
    dneg = em.ts(ALU.mult, d_, -1.0)
    b2o = em.stt(u1, dneg, b1, ALU.mult, ALU.add, 3)
    u2 = normalize(b2o)
    # u3 = u1 x u2 (one fused op per component)
    u3 = em.new(3)
    for k, (i1, i2) in enumerate(((1, 2), (2, 0), (0, 1))):
        em.cdve(OP_AXMBY, u1[:, i1:i1 + 1], u1[:, i2:i2 + 1],
                s0=u2[:, i2:i2 + 1], s1=u2[:, i1:i1 + 1],
                out=u3[:, k:k + 1])

    # --- R = u1 v1^T + u2 v2^T + u3 v3^T ; write into pose cols 4r+c ---
    t0 = em.tt3(ALU.mult, _bcast_r(u1), _bcast_l(Vc[0]), 9)
    t1 = em.tt3(ALU.mult, _bcast_r(u2), _bcast_l(Vc[1]), 9)
    t01 = em.tt(ALU.add, t0, t1, 9)
    t2 = em.tt3(ALU.mult, _bcast_r(u3), _bcast_l(Vc[2]), 9)
    pose_R = pose_tile[:].rearrange("p (r c) -> p r c", r=4, c=4)[:, 0:3, 0:3]
    nc.vector.tensor_tensor(
        pose_R, t01.rearrange("p (r c) -> p r c", r=3, c=3),
        t2.rearrange("p (r c) -> p r c", r=3, c=3), ALU.add)


# ---------------------------------------------------------------------------
# kernel build
# ---------------------------------------------------------------------------
def build_nc():
    nc = bacc.Bacc("TRN2", target_bir_lowering=False)
    MMDT = F32R if USE_F32R else F32

    xT = nc.dram_tensor("xT", [D, T_CORE], F32, kind="ExternalInput")
    wts = nc.dram_tensor("wts", [6, D, D], F32, kind="ExternalInput")
    bs = nc.dram_tensor("bs", [6, D], F32, kind="ExternalInput")
    mwt = nc.dram_tensor("mwt", [2, D, D], F32, kind="ExternalInput")
    mbs = nc.dram_tensor("mbs", [2, D], F32, kind="ExternalInput")
    hwT = nc.dram_tensor("hwT", [D, 12], F32, kind="ExternalInput")
    hb = nc.dram_tensor("hb", [32, 12], F32, kind="ExternalInput")
    pose = nc.dram_tensor("pose", [32, 16], F32, kind="ExternalOutput")

    def mmview(ap):
        return ap.bitcast(F32R) if USE_F32R else ap

    with tile.TileContext(nc) as tc:
        with (
            tc.tile_pool(name="wp", bufs=1) as wpool,
            tc.tile_pool(name="xp", bufs=4) as xpool,
            tc.tile_pool(name="hp", bufs=2) as hpool,
            tc.tile_pool(name="rp", bufs=3) as rpool,
            tc.tile_pool(name="pp", bufs=1) as ppool,
            tc.tile_pool(name="ps", bufs=6, space="PSUM") as pspool,
            tc.tile_pool(name="pst", bufs=2, space="PSUM") as pstpool,
            tc.tile_pool(name="sm", bufs=1) as smpool,
        ):
            # ---- load first x tile + layer-0 weights FIRST so the PE can
            # start ~6us in; the rest of the weights stream behind ----
            # xt0 on the gpsimd DMA queue so it streams in parallel with w0
            xt0 = xpool.tile([128, 4 * T_TILE], MMDT, tag="xt", name="xt")
            for k in range(4):
                nc.gpsimd.dma_start(
                    xt0[:, T_TILE * k:T_TILE * (k + 1)],
                    mmview(xT[128 * k:128 * (k + 1), 0:T_TILE]))
            w_sb = [wpool.tile([128, 4 * D], MMDT, tag=f"w{l}", name=f"w{l}")
                    for l in range(6)]
            # biases are tiny and gate the first ACT — load right after w0
            b_sb = wpool.tile([128, 24], F32, tag="b", name="b_sb")
            for k in range(4):
                nc.sync.dma_start(
                    w_sb[0][:, D * k:D * (k + 1)],
                    mmview(wts[0, 128 * k:128 * (k + 1), :]))
            for l in range(6):
                nc.sync.dma_start(b_sb[:, 4 * l:4 * l + 4],
                                  bs[l].rearrange("(o p) -> p o", p=128, o=4))
            for l in range(1, 6):
                for k in range(4):
                    nc.sync.dma_start(
                        w_sb[l][:, D * k:D * (k + 1)],
                        mmview(wts[l, 128 * k:128 * (k + 1), :]))
            mw_sb = [wpool.tile([128, 4 * D], F32, tag=f"mw{l}", name=f"mw{l}")
                     for l in range(2)]
            for l in range(2):
                for k in range(4):
                    nc.sync.dma_start(
                        mw_sb[l][:, D * k:D * (k + 1)],
                        mwt[l, 128 * k:128 * (k + 1), :])
            mb_sb = wpool.tile([128, 8], F32, tag="mb", name="mb_sb")
            for l in range(2):
                nc.sync.dma_start(mb_sb[:, 4 * l:4 * l + 4],
                                  mbs[l].rearrange("(o p) -> p o", p=128, o=4))
            hw_sb = wpool.tile([128, 48], F32, tag="hw", name="hw_sb")
            for k in range(4):
                nc.sync.dma_start(hw_sb[:, 12 * k:12 * (k + 1)],
                                  hwT[128 * k:128 * (k + 1), :])
            hb_sb = wpool.tile([32, 12], F32, tag="hbt", name="hb_sb")
            nc.sync.dma_start(hb_sb[:], hb[:])

            # pooled sums accumulator: [128, 4 kchunks * 32 samples]
            pool_acc = ppool.tile([128, 4 * S_CORE], F32, tag="pool",
                                  name="pool_acc")

            # ---- main loop over token tiles ----
            for ti in range(N_TILES):
                if ti == 0:
                    xt = xt0
                else:
                    # gpsimd DMA queue: keeps the x stream from serializing
                    # behind the weight transfers on the in-order Sync queue
                    xt = xpool.tile([128, 4 * T_TILE], MMDT, tag="xt",
                                    name="xt")
                    for k in range(4):
                        nc.gpsimd.dma_start(
                            xt[:, T_TILE * k:T_TILE * (k + 1)],
                            mmview(xT[128 * k:128 * (k + 1),
                                      T_TILE * ti:T_TILE * (ti + 1)]))
                cur = xt
                for blk in range(2):
                    h_in = cur
                    for li in range(3):
                        l = blk * 3 + li
                        # the block's last layer output feeds only the DVE
                        # residual add -> plain F32, relu can run on DVE
                        # (F32R out from tensor_scalar is broken on HW;
                        # ACT F32R out and tensor_add F32R out are fine)
                        last = li == 2
                        h_out = hpool.tile([128, 4 * T_TILE],
                                           F32 if last else MMDT,
                                           tag="hf" if last else "h",
                                           name=f"h{l}")
                        for o in range(4):
                            ps = pspool.tile([128, T_TILE], F32, tag="ps",
                                             name="ps")
                            for k in range(4):
                                nc.tensor.matmul(
                                    ps[:],
                                    w_sb[l][:, D * k + 128 * o:
                                            D * k + 128 * (o + 1)],
                                    xt_chunk(h_in, k),
                                    start=(k == 0), stop=(k == 3))
                            hsl = h_out[:, T_TILE * o:T_TILE * (o + 1)]
                            if last and blk == 1:
                                # blk1's output only feeds pooling; off the
                                # critical path -> DVE relieves ACT. blk0's
                                # output gates block 2 via the residual, so
                                # it stays on ACT (shorter latency chain).
                                nc.vector.tensor_scalar(
                                    hsl, ps[:],
                                    b_sb[:, 4 * l + o:4 * l + o + 1], 0.0,
                                    ALU.add, ALU.max)
                            else:
                                nc.scalar.activation(
                                    hsl, ps[:], AF.Relu,
                                    bias=b_sb[:, 4 * l + o:4 * l + o + 1],
                                    scale=1.0)
                        h_in = h_out
                    res = rpool.tile([128, 4 * T_TILE], MMDT, tag="res",
                                     name=f"res{blk}")
                    # chunk-split: block2's first matmul only needs chunk 0,
                    # and xt is released per-chunk for the next tile's DMA.
                    for k in range(4):
                        sl = slice(T_TILE * k, T_TILE * (k + 1))
                        if blk == 0:
                            # feeds block-2 matmuls: F32R out dtype so the
                            # verifier sees a rounded producer
                            nc.vector.tensor_add(res[:, sl], fview(cur)[:, sl],
                                                 fview(h_in)[:, sl])
                        else:
                            # only feeds pooling (DVE): plain f32 view
                            nc.vector.tensor_add(fview(res)[:, sl],
                                                 fview(cur)[:, sl],
                                                 fview(h_in)[:, sl])
                    cur = res
                # pooling: 2 samples per tile, sum over 256 tokens each
                for k in range(4):
                    nc.vector.tensor_reduce(
                        pool_acc[:, S_CORE * k + S_TILE * ti:
                                 S_CORE * k + S_TILE * (ti + 1)],
                        fview(cur)[:, T_TILE * k:T_TILE * (k + 1)].rearrange(
                            "p (g t) -> p g t", g=S_TILE),
                        axis=AX.X, op=ALU.add)

            # ---- tail MLPs (fp32) ----
            f_prev = pool_acc
            scales = [1.0 / TOK, 1.0]
            f_tiles = []
            for l in range(2):
                f_out = smpool.tile([128, 4 * S_CORE], F32, tag=f"f{l}",
                                    name=f"f{l}")
                for o in range(4):
                    ps = pstpool.tile([128, S_CORE], F32, tag="pst", name="pst")
                    for k in range(4):
                        nc.tensor.matmul(
                            ps[:],
                            mw_sb[l][:, D * k + 128 * o:D * k + 128 * (o + 1)],
                            f_prev[:, S_CORE * k:S_CORE * (k + 1)],
                            start=(k == 0), stop=(k == 3))
                    nc.scalar.activation(
                        f_out[:, S_CORE * o:S_CORE * (o + 1)], ps[:], AF.Relu,
                        bias=mb_sb[:, 4 * l + o:4 * l + o + 1], scale=scales[l])
                f_prev = f_out
                f_tiles.append(f_out)

            # ---- heads: [32 samples, 12] = t(3) ++ rot(9) ----
            psh = pstpool.tile([32, 12], F32, tag="pst", name="psh")
            for k in range(4):
                nc.tensor.matmul(psh[:],
                                 f_prev[:, S_CORE * k:S_CORE * (k + 1)],
                                 hw_sb[:, 12 * k:12 * (k + 1)],
                                 start=(k == 0), stop=(k == 3))
            mm = smpool.tile([32, 12], F32, tag="mm", name="mm")
            nc.vector.tensor_add(mm[:], psh[:], hb_sb[:])

            # ---- pose assembly + SVD ----
            pose_t = smpool.tile([32, 16], F32, tag="pose", name="pose_t")
            nc.vector.memset(pose_t[:], 0.0)
            nc.vector.memset(pose_t[:, 15:16], 1.0)
            nc.vector.tensor_copy(
                pose_t[:].rearrange("p (r c) -> p r c", r=4, c=4)[:, 0:3, 3],
                mm[:, 0:3])

            em = Emit(nc, smpool)
            emit_svd_so3(nc, em, mm[:, 3:12], pose_t)

            nc.sync.dma_start(pose[:], pose_t[:])

    nc.compile()
    return nc


def xt_chunk(t, k):
    return t[:, T_TILE * k:T_TILE * (k + 1)]


def fview(ap):
    """f32 view of a (possibly f32r) tile for DVE ops."""
    return ap.bitcast(F32) if USE_F32R else ap


_NC_CACHE = None


def _get_nc():
    global _NC_CACHE
    if _NC_CACHE is None:
        _NC_CACHE = build_nc()
    return _NC_CACHE


def kernel(**inputs):
    feat = np.asarray(inputs["feat"], dtype=np.float32)
    b_, v_, n_, d_ = feat.shape
    xs = feat.reshape(b_ * v_, n_, d_)

    wts = np.stack([np.ascontiguousarray(
        np.asarray(inputs[f"r{blk}_w{li}"], np.float32).T)
        for blk in (1, 2) for li in (1, 2, 3)])
    bs = np.stack([np.asarray(inputs[f"r{blk}_b{li}"], np.float32)
                   for blk in (1, 2) for li in (1, 2, 3)])
    mwt = np.stack([np.ascontiguousarray(
        np.asarray(inputs[f"m_w{li}"], np.float32).T) for li in (1, 2)])
    mbs = np.stack([np.asarray(inputs[f"m_b{li}"], np.float32)
                    for li in (1, 2)])
    hwT = np.ascontiguousarray(np.concatenate(
        [np.asarray(inputs["t_w"], np.float32).T,
         np.asarray(inputs["rot_w"], np.float32).T], axis=1))
    hb = np.broadcast_to(np.concatenate(
        [np.asarray(inputs["t_b"], np.float32),
         np.asarray(inputs["rot_b"], np.float32)])[None, :],
        (S_CORE, 12)).copy()

    in_maps = []
    for c in range(N_CORES):
        xT = np.ascontiguousarray(
            xs[c * S_CORE:(c + 1) * S_CORE].reshape(T_CORE, D).T)
        in_maps.append({
            "xT": xT, "wts": wts, "bs": bs, "mwt": mwt, "mbs": mbs,
            "hwT": hwT, "hb": hb,
        })

    nc = _get_nc()
    import os
    kwargs = {}
    if os.environ.get("KERNEL_TRACE") == "1":
        kwargs["trace"] = True
    res = run_bass_kernel_spmd(nc, in_maps, core_ids=list(range(N_CORES)),
                               **kwargs)
    if kwargs.get("trace"):
        kernel.last_results = res
    poses = np.concatenate([r["pose"] for r in res.results], axis=0)
    return poses.reshape(b_, v_, 4, 4)



# revision 9
# speedup vs baseline: 1.2747x; 1.2747x over previous
"""CameraHead Trainium2 kernel — data-parallel over b*v across 8 NeuronCores.

fp8(e4m3) DoubleRow edition. Per-core layout: activations feature-major in
SBUF (X^T: 4x128-part chunks x tokens, fp8), all six 512x512 Linears run as
DoubleRow fp8 matmuls (K=256 per pass, 512-token moving operand, 2 passes for
K=512) at 2x the bf16/fp32r PE rate. Relu+bias drains PSUM on three engines
(ACT / DVE / GPSIMD) in parallel. The block-2 first Linear consumes x0 and
h3(block1) as two accumulated matmul streams, so the residual add never
materializes. Pooling = host-exact sum(x0) + on-device tensor_reduce over the
two h3 streams. Tail: bf16 MLP/head matmuls + a lean d-tracking Jacobi
(9 rotations) for the SVD->SO(3) projection, then pose assembly.
"""
import sys
import numpy as np

sys.path.insert(0, '/opt/trn_rl_repo')

import ml_dtypes  # noqa: E402

import concourse.bacc as bacc  # noqa: E402
import concourse.mybir as mybir  # noqa: E402
from concourse import tile  # noqa: E402
from concourse import dve_ops as _dvo  # noqa: E402
from concourse.bass_utils import run_bass_kernel_spmd  # noqa: E402
from concourse.dve_spec import (  # noqa: E402
    C0, C1, C2, One, Spec, Src0, Src1, select as dve_select, sq as dve_sq,
)


def _reg_op(name, body, ref):
    """Register a custom DVE op (per-NEFF uop table; no firmware change).

    The uops sha pin is bootstrapped by parsing compile()'s drift error."""
    for op in _dvo.OPS:
        if op.name == name:
            return op
    import re as _re

    from concourse.dve_table_gen import dve_ver_for

    row = _dvo._CUSTOM_DVE_ROW_BASE + len(_dvo.OPS)
    assert row < 0x20, "custom DVE opcode rows exhausted"
    spec = Spec(body=body, reference=ref)
    op = _dvo.DveOp(name, spec, subdim=False, uops_sha={})
    _dvo.OPS.append(op)
    _dvo._SUB_OPCODE_FOR_NAME[name] = row
    _dvo.CUSTOM_DVE_SPECS[name] = spec
    ver = dve_ver_for("TRN2")
    try:
        op.compile(ver)
    except ValueError as e:
        m = _re.search(r'uops_sha\["' + ver + r'"\]="([0-9a-f]+)"', str(e))
        if not m:
            raise
        op.uops_sha[ver] = m.group(1)
        op.compile(ver)
    return op


_f32 = np.float32
OP_AXPBY = _reg_op(
    "ANT_AXPBY", Src0 * C0 + Src1 * C1,
    lambda in0, in1, s0, s1, imm2: (in0 * s0 + in1 * s1).astype(_f32))
OP_AXMBY = _reg_op(
    "ANT_AXMBY", Src0 * C0 - Src1 * C1,
    lambda in0, in1, s0, s1, imm2: (in0 * s0 - in1 * s1).astype(_f32))
OP_AXPBY2 = _reg_op(
    "ANT_AXPBY2", (Src0 * C0 + Src1 * C1) * C2,
    lambda in0, in1, s0, s1, imm2: ((in0 * s0 + in1 * s1) * imm2).astype(_f32))
OP_AXMBY2 = _reg_op(
    "ANT_AXMBY2", (Src0 * C0 - Src1 * C1) * C2,
    lambda in0, in1, s0, s1, imm2: ((in0 * s0 - in1 * s1) * imm2).astype(_f32))
OP_SQDIFF = _reg_op(
    "ANT_SQDIFF", dve_sq(Src0) - dve_sq(Src1),
    lambda in0, in1, s0, s1, imm2: (in0 * in0 - in1 * in1).astype(_f32))


def _xy2_body():
    t = Src0 * Src1
    return t + t


OP_XY2 = _reg_op(
    "ANT_XY2", _xy2_body(),
    lambda in0, in1, s0, s1, imm2: (2.0 * in0 * in1).astype(_f32))
OP_WHERE = _reg_op(
    "ANT_WHERE", dve_select(C0, Src0, Src1),
    lambda in0, in1, s0, s1, imm2: np.where(
        s0 != 0, in0, in1).astype(_f32))
OP_WHERENEG = _reg_op(
    "ANT_WHERENEG", dve_select(C0, -Src0, Src1),
    lambda in0, in1, s0, s1, imm2: np.where(
        s0 != 0, -in0, in1).astype(_f32))
OP_SQSUM = _reg_op(
    "ANT_SQSUM", dve_sq(Src0) + dve_sq(Src1),
    lambda in0, in1, s0, s1, imm2: (in0 * in0 + in1 * in1).astype(_f32))
# (ch^2 - sh^2) * w2  — raw cosine
OP_CSUBW = _reg_op(
    "ANT_CSUBW", (dve_sq(Src0) - dve_sq(Src1)) * C0,
    lambda in0, in1, s0, s1, imm2: (
        (in0 * in0 - in1 * in1) * s0).astype(_f32))


def _xyw_body():
    t = Src0 * Src1
    return (t + t) * C0


# 2 * ch * sh * w2 — raw sine
OP_XYW2 = _reg_op(
    "ANT_XYW2", _xyw_body(),
    lambda in0, in1, s0, s1, imm2: (2.0 * in0 * in1 * s0).astype(_f32))
# gate value: ch^2 - gamma*sh^2 (>0 -> use raw angle, else pi/4 fallback)
OP_GATE = _reg_op(
    "ANT_GATE", dve_sq(Src0) - dve_sq(Src1) * C1,
    lambda in0, in1, s0, s1, imm2: (
        in0 * in0 - in1 * in1 * s1).astype(_f32))
# select(g < 0, imm2, raw)
OP_SELPOS = _reg_op(
    "ANT_SELPOS", dve_select(Src1 < C1, C2, Src0),
    lambda in0, in1, s0, s1, imm2: np.where(
        in1 < s1, imm2, in0).astype(_f32))


def _axpb4y_body():
    t = Src1 * C1
    u = t + t
    return Src0 * C0 + (u + u)


# d' = c2*d + 4*s2*pt
OP_AXPB4Y = _reg_op(
    "ANT_AXPB4Y", _axpb4y_body(),
    lambda in0, in1, s0, s1, imm2: (in0 * s0 + 4.0 * in1 * s1).astype(_f32))
# pt' = c2*pt - 0.25*s2*d   (imm2 carries the 0.25)
OP_AXMBYC = _reg_op(
    "ANT_AXMBYC", Src0 * C0 - (Src1 * C1) * C2,
    lambda in0, in1, s0, s1, imm2: (in0 * s0 - in1 * s1 * imm2).astype(_f32))

F32 = mybir.dt.float32
BF16 = mybir.dt.bfloat16
F8 = mybir.dt.float8e4
AF = mybir.ActivationFunctionType
ALU = mybir.AluOpType
AX = mybir.AxisListType
DR = mybir.MatmulPerfMode.DoubleRow

N_CORES = 8
D = 512
SAMPLES = 256          # b*v
TOK = 256              # tokens per sample
S_CORE = SAMPLES // N_CORES       # 32 samples per core
T_CORE = S_CORE * TOK             # 8192 token rows per core
T_TILE = 512
N_TILES = T_CORE // T_TILE        # 16
S_TILE = T_TILE // TOK            # 2 samples per token tile

T_SUP = 1024           # supertile: 2 token-tiles drained with wide ops
N_SUP = T_CORE // T_SUP           # 8
S_SUP = T_SUP // TOK              # 4 samples per supertile

N_ROT = 9              # lean Jacobi rotations (sim: rel_err 3.3e-3)

GAMMA = float(3.0 + 2.0 * np.sqrt(2.0))
CQ45 = float(np.cos(np.pi / 4))


# ---------------------------------------------------------------------------
# small-op emitter for the SVD tail: SSA-style column allocation on a scratch
# tile; every value is an AP (or list of APs).
# ---------------------------------------------------------------------------
class Emit:
    def __init__(self, nc, pool):
        self.nc = nc
        self.scr = pool.tile([32, 2048], F32, tag="svd_scratch",
                             name="svd_scratch")
        self.ptr = 0

    def new(self, n=1):
        c = self.ptr
        self.ptr += n
        assert self.ptr <= 2048, "svd scratch overflow"
        return self.scr[:, c:c + n]

    def tt(self, op, a, b, n=1):
        o = self.new(n)
        self.nc.vector.tensor_tensor(o, a, b, op)
        return o

    def tt3(self, op, a, b, n=9):
        o = self.new(n)
        self.nc.vector.tensor_tensor(
            o.rearrange("p (i j) -> p i j", i=3, j=n // 3), a, b, op)
        return o

    def ts(self, op, a, s, n=1):
        o = self.new(n)
        self.nc.vector.tensor_scalar(o, a, s, None, op)
        return o

    def stt(self, a, scal, b, op0, op1, n=1):
        o = self.new(n)
        self.nc.vector.scalar_tensor_tensor(o, a, scal, b, op0=op0, op1=op1)
        return o

    def rsqrt(self, a, n=1):
        t = self.new(n)
        self.nc.scalar.activation(t, a, AF.Sqrt)
        o = self.new(n)
        self.nc.vector.reciprocal(o, t)
        return o

    def cdve(self, op, in0, in1, s0=0.0, s1=0.0, imm2=0.0, n=1, out=None):
        if out is None:
            out = self.new(n)
        self.nc.vector._custom_dve(op, out=out, in0=in0, in1=in1,
                                   s0=s0, s1=s1, imm2=imm2)
        return out

    def const(self, val, n=1):
        o = self.new(n)
        self.nc.vector.memset(o, val)
        return o


def _bcast_r(ap3):
    return ap3.unsqueeze(2).broadcast_to([32, 3, 3])


def _bcast_l(ap3):
    return ap3.unsqueeze(1).broadcast_to([32, 3, 3])


def emit_svd_so3(nc, em, m_ap, pose_tile):
    """m_ap: [32,9] raw 3x3 per sample (row-major). Writes the SO(3)
    projection into pose_tile columns (4r+c for r,c in 0..2).

    Lean d-tracking Jacobi: state is (d01,d12,d02) eigenvalue differences and
    (p01,p12,p02) halved off-diagonals; 15 DVE ops per rotation."""
    # --- row normalize ---
    sq = em.tt(ALU.mult, m_ap, m_ap, 9)
    t = em.tt(ALU.add, sq[:, 0:9:3], sq[:, 1:9:3], 3)
    r2 = em.tt(ALU.add, t, sq[:, 2:9:3], 3)
    r2c = em.ts(ALU.max, r2, 1e-24, 3)
    rinv = em.rsqrt(r2c, 3)
    A = em.tt3(ALU.mult, m_ap.rearrange("p (r c) -> p r c", r=3, c=3),
               _bcast_r(rinv), 9)

    # --- S = A^T A (s_ij at col 3i+j) ---
    terms = []
    for r in range(3):
        arow = A[:, 3 * r:3 * r + 3]
        terms.append(em.tt3(ALU.mult, _bcast_r(arow), _bcast_l(arow), 9))
    s01 = em.tt(ALU.add, terms[0], terms[1], 9)
    S9 = em.tt(ALU.add, s01, terms[2], 9)

    # d/pt state (SSA-tracked APs)
    d01 = em.tt(ALU.subtract, S9[:, 0:1], S9[:, 4:5])
    d12 = em.tt(ALU.subtract, S9[:, 4:5], S9[:, 8:9])
    d02 = em.tt(ALU.add, d01, d12)
    p01 = em.ts(ALU.mult, S9[:, 1:2], 0.5)
    p12 = em.ts(ALU.mult, S9[:, 5:6], 0.5)
    p02 = em.ts(ALU.mult, S9[:, 2:3], 0.5)

    # V columns as [32,3] blocks, init = identity
    Vc = []
    for j in range(3):
        vj = em.new(3)
        nc.vector.memset(vj, 0.0)
        nc.vector.memset(vj[:, j:j + 1], 1.0)
        Vc.append(vj)

    st = {'d01': d01, 'd12': d12, 'd02': d02,
          'p01': p01, 'p12': p12, 'p02': p02}

    def angle(ch, sh):
        ssum = em.cdve(OP_SQSUM, ch, sh)
        w2 = em.new(1)
        nc.vector.reciprocal(w2, ssum)
        craw = em.cdve(OP_CSUBW, ch, sh, s0=w2)
        sraw = em.cdve(OP_XYW2, ch, sh, s0=w2)
        g = em.cdve(OP_GATE, ch, sh, s1=GAMMA)
        c = em.cdve(OP_SELPOS, craw, g, s1=0.0, imm2=CQ45)
        s = em.cdve(OP_SELPOS, sraw, g, s1=0.0, imm2=CQ45)
        c2 = em.cdve(OP_SQDIFF, c, s)
        s2 = em.cdve(OP_XY2, c, s)
        return c, s, c2, s2

    def vup(p, q, c, s):
        nvp = em.cdve(OP_AXPBY, Vc[p], Vc[q], s0=c, s1=s, n=3)
        nvq = em.cdve(OP_AXMBY, Vc[q], Vc[p], s0=c, s1=s, n=3)
        Vc[p], Vc[q] = nvp, nvq

    for k in range(N_ROT):
        rt = k % 3
        if rt == 0:     # (p,q,r) = (0,1,2)
            c, s, c2, s2 = angle(st['d01'], st['p01'])
            dd = em.cdve(OP_AXPB4Y, st['d01'], st['p01'], s0=c2, s1=s2)
            pn = em.cdve(OP_AXMBYC, st['p01'], st['d01'], s0=c2, s1=s2,
                         imm2=0.25)
            t_ = em.tt(ALU.add, st['d02'], st['d12'])
            nd02 = em.cdve(OP_AXPBY, t_, dd, s0=0.5, s1=0.5)
            nd12 = em.cdve(OP_AXMBY, t_, dd, s0=0.5, s1=0.5)
            np02 = em.cdve(OP_AXPBY, st['p02'], st['p12'], s0=c, s1=s)
            np12 = em.cdve(OP_AXMBY, st['p12'], st['p02'], s0=c, s1=s)
            st.update(d01=dd, p01=pn, d02=nd02, d12=nd12, p02=np02, p12=np12)
            vup(0, 1, c, s)
        elif rt == 1:   # (1,2,0)
            c, s, c2, s2 = angle(st['d12'], st['p12'])
            dd = em.cdve(OP_AXPB4Y, st['d12'], st['p12'], s0=c2, s1=s2)
            pn = em.cdve(OP_AXMBYC, st['p12'], st['d12'], s0=c2, s1=s2,
                         imm2=0.25)
            t_ = em.tt(ALU.add, st['d01'], st['d02'])
            nd01 = em.cdve(OP_AXMBY, t_, dd, s0=0.5, s1=0.5)
            nd02 = em.cdve(OP_AXPBY, t_, dd, s0=0.5, s1=0.5)
            np01 = em.cdve(OP_AXPBY, st['p01'], st['p02'], s0=c, s1=s)
            np02 = em.cdve(OP_AXMBY, st['p02'], st['p01'], s0=c, s1=s)
            st.update(d12=dd, p12=pn, d01=nd01, d02=nd02, p01=np01, p02=np02)
            vup(1, 2, c, s)
        else:           # (0,2,1)
            c, s, c2, s2 = angle(st['d02'], st['p02'])
            dd = em.cdve(OP_AXPB4Y, st['d02'], st['p02'], s0=c2, s1=s2)
            pn = em.cdve(OP_AXMBYC, st['p02'], st['d02'], s0=c2, s1=s2,
                         imm2=0.25)
            t_ = em.tt(ALU.subtract, st['d01'], st['d12'])
            nd01 = em.cdve(OP_AXPBY, t_, dd, s0=0.5, s1=0.5)
            nd12 = em.cdve(OP_AXMBY, dd, t_, s0=0.5, s1=0.5)
            np01 = em.cdve(OP_AXPBY, st['p01'], st['p12'], s0=c, s1=s)
            np12 = em.cdve(OP_AXMBY, st['p12'], st['p01'], s0=c, s1=s)
            st.update(d02=dd, p02=pn, d01=nd01, d12=nd12, p01=np01, p12=np12)
            vup(0, 2, c, s)

    # --- sort eigenpairs descending (det(V) stays +1 via column negation) ---
    def cond_swap(i, j):
        key_ij = f'd{i}{j}'
        k = 3 - i - j
        key_ik = f'd{min(i,k)}{max(i,k)}'
        key_jk = f'd{min(j,k)}{max(j,k)}'
        mask = em.ts(ALU.is_lt, st[key_ij], 0.0)
        nij = em.cdve(OP_WHERENEG, st[key_ij], st[key_ij], s0=mask)
        nik = em.cdve(OP_WHERE, st[key_jk], st[key_ik], s0=mask)
        njk = em.cdve(OP_WHERE, st[key_ik], st[key_jk], s0=mask)
        st[key_ij], st[key_ik], st[key_jk] = nij, nik, njk
        vi = em.cdve(OP_WHERE, Vc[j], Vc[i], s0=mask, n=3)
        vj = em.cdve(OP_WHERENEG, Vc[i], Vc[j], s0=mask, n=3)
        Vc[i], Vc[j] = vi, vj

    cond_swap(0, 1)
    cond_swap(1, 2)
    cond_swap(0, 1)

    # --- B columns (j=0,1): b_j[r] = sum_c A[r][c] * V[c][j] ---
    Astr = [A[:, c:c + 7:3] for c in range(3)]

    def bcol(j):
        t0 = em.cdve(OP_AXPBY, Astr[0], Astr[1],
                     s0=Vc[j][:, 0:1], s1=Vc[j][:, 1:2], n=3)
        return em.stt(Astr[2], Vc[j][:, 2:3], t0, ALU.mult, ALU.add, 3)

    b0 = bcol(0)
    b1 = bcol(1)

    def normalize(v3):
        sqv = em.tt(ALU.mult, v3, v3, 3)
        n_ = em.tt(ALU.add, sqv[:, 0:1], sqv[:, 1:2])
        n_ = em.tt(ALU.add, n_, sqv[:, 2:3])
        nc_ = em.ts(ALU.max, n_, 1e-30)
        inv = em.rsqrt(nc_)
        return em.ts(ALU.mult, v3, inv, 3)

    u1 = normalize(b0)
    p_ = em.tt(ALU.mult, u1, b1, 3)
    d_ = em.tt(ALU.add, p_[:, 0:1], p_[:, 1:2])
    d_ = em.tt(ALU.add, d_, p_[:, 2:3])
    dneg = em.ts(ALU.mult, d_, -1.0)
    b2o = em.stt(u1, dneg, b1, ALU.mult, ALU.add, 3)
    u2 = normalize(b2o)
    u3 = em.new(3)
    for k, (i1, i2) in enumerate(((1, 2), (2, 0), (0, 1))):
        em.cdve(OP_AXMBY, u1[:, i1:i1 + 1], u1[:, i2:i2 + 1],
                s0=u2[:, i2:i2 + 1], s1=u2[:, i1:i1 + 1],
                out=u3[:, k:k + 1])

    # --- R = u1 v1^T + u2 v2^T + u3 v3^T ---
    t0 = em.tt3(ALU.mult, _bcast_r(u1), _bcast_l(Vc[0]), 9)
    t1 = em.tt3(ALU.mult, _bcast_r(u2), _bcast_l(Vc[1]), 9)
    t01 = em.tt(ALU.add, t0, t1, 9)
    t2 = em.tt3(ALU.mult, _bcast_r(u3), _bcast_l(Vc[2]), 9)
    pose_R = pose_tile[:].rearrange("p (r c) -> p r c", r=4, c=4)[:, 0:3, 0:3]
    nc.vector.tensor_tensor(
        pose_R, t01.rearrange("p (r c) -> p r c", r=3, c=3),
        t2.rearrange("p (r c) -> p r c", r=3, c=3), ALU.add)


# ---------------------------------------------------------------------------
# kernel build
# ---------------------------------------------------------------------------
def build_nc():
    nc = bacc.Bacc("TRN2", target_bir_lowering=False)

    xT8 = nc.dram_tensor("xT8", [D, T_CORE], F8, kind="ExternalInput")
    w8 = nc.dram_tensor("w8", [6, 128, 2048], F8, kind="ExternalInput")
    bs = nc.dram_tensor("bs", [6, D], F32, kind="ExternalInput")
    x0s = nc.dram_tensor("x0s", [128, 4 * S_CORE], F32, kind="ExternalInput")
    mwt = nc.dram_tensor("mwt", [2, D, D], BF16, kind="ExternalInput")
    mbs = nc.dram_tensor("mbs", [2, D], F32, kind="ExternalInput")
    hwT = nc.dram_tensor("hwT", [D, 12], BF16, kind="ExternalInput")
    hb = nc.dram_tensor("hb", [S_CORE, 12], F32, kind="ExternalInput")
    pose = nc.dram_tensor("pose", [S_CORE, 16], F32, kind="ExternalOutput")

    with tile.TileContext(nc) as tc:
        with (
            tc.tile_pool(name="wp", bufs=1) as wpool,
            tc.tile_pool(name="xp", bufs=3) as xpool,
            tc.tile_pool(name="hp", bufs=2) as hpool,
            tc.tile_pool(name="h3p", bufs=2) as h3pool,
            tc.tile_pool(name="pp", bufs=1) as ppool,
            tc.tile_pool(name="ps", bufs=3, space="PSUM") as pspool,
            tc.tile_pool(name="pst", bufs=2, space="PSUM") as pstpool,
            tc.tile_pool(name="sm", bufs=1) as smpool,
        ):
            # warm the ACT function table while DMAs stream
            warm = smpool.tile([32, 1], F32, tag="warm", name="warm")
            nc.vector.memset(warm[:], 0.0)
            nc.scalar.activation(warm[:], warm[:], AF.Relu)

            # ---- first x supertile + layer-0 weights first; rest streams
            xt0 = xpool.tile([128, 4 * T_SUP], F8, tag="xt", name="xt")
            for k in range(4):
                nc.gpsimd.dma_start(
                    xt0[:, T_SUP * k:T_SUP * (k + 1)],
                    xT8[128 * k:128 * (k + 1), 0:T_SUP])
            w_sb = [wpool.tile([128, 2048], F8, tag=f"w{l}", name=f"w{l}")
                    for l in range(6)]
            nc.sync.dma_start(w_sb[0][:], w8[0])
            b_sb = wpool.tile([128, 24], F32, tag="b", name="b_sb")
            for l in range(6):
                nc.sync.dma_start(b_sb[:, 4 * l:4 * l + 4],
                                  bs[l].rearrange("(o p) -> p o", p=128, o=4))
            for l in range(1, 6):
                nc.sync.dma_start(w_sb[l][:], w8[l])
            x0s_sb = wpool.tile([128, 4 * S_CORE], F32, tag="x0s",
                                name="x0s_sb")
            nc.sync.dma_start(x0s_sb[:], x0s[:])
            mw_sb = [wpool.tile([128, 2048], BF16, tag=f"mw{l}",
                                name=f"mw{l}") for l in range(2)]
            for l in range(2):
                for k in range(4):
                    nc.sync.dma_start(
                        mw_sb[l][:, D * k:D * (k + 1)],
                        mwt[l, 128 * k:128 * (k + 1), :])
            mb_sb = wpool.tile([128, 8], F32, tag="mb", name="mb_sb")
            for l in range(2):
                nc.sync.dma_start(mb_sb[:, 4 * l:4 * l + 4],
                                  mbs[l].rearrange("(o p) -> p o", p=128, o=4))
            hw_sb = wpool.tile([128, 48], BF16, tag="hw", name="hw_sb")
            for k in range(4):
                nc.sync.dma_start(hw_sb[:, 12 * k:12 * (k + 1)],
                                  hwT[128 * k:128 * (k + 1), :])
            hb_sb = wpool.tile([32, 12], F32, tag="hbt", name="hb_sb")
            nc.sync.dma_start(hb_sb[:], hb[:])

            # per-tile h3 sums for the two blocks: [128, 4k x 32 samples]
            pb1 = ppool.tile([128, 4 * S_CORE], F32, tag="pb1", name="pb1")
            pb2 = ppool.tile([128, 4 * S_CORE], F32, tag="pb2", name="pb2")

            def wap(l, o, kp):
                c0 = (o * 2 + kp) * 256
                return w_sb[l][:, c0:c0 + 256].rearrange(
                    "p (i m) -> p i m", i=2)

            def rhs(t, kp, th):
                # kp-pair chunks of a [128, 4*T_SUP] supertile, token half th
                return t[:, 2 * T_SUP * kp:2 * T_SUP * (kp + 1)].rearrange(
                    "p (i n) -> p i n", i=2)[:, :, 512 * th:512 * (th + 1)]

            def relu_drain(engine, h_slice, ps, bias_ap):
                if engine == 'act':
                    nc.scalar.activation(h_slice, ps[:], AF.Relu,
                                         bias=bias_ap, scale=1.0)
                else:
                    nc.vector.tensor_scalar(h_slice, ps[:], bias_ap, 0.0,
                                            ALU.add, ALU.max)

            # engine assignment per layer (ACT 14 / DVE 10 drains per sup)
            ENG = {
                0: ['act'] * 4,
                1: ['dve'] * 4,
                2: ['act'] * 4,
                3: ['act'] * 4,
                4: ['act', 'act', 'dve', 'dve'],
                5: ['dve'] * 4,
            }

            # ---- main loop over supertiles (1024 tokens each) ----
            for ti in range(N_SUP):
                if ti == 0:
                    xt = xt0
                else:
                    xt = xpool.tile([128, 4 * T_SUP], F8, tag="xt",
                                    name="xt")
                    for k in range(4):
                        nc.gpsimd.dma_start(
                            xt[:, T_SUP * k:T_SUP * (k + 1)],
                            xT8[128 * k:128 * (k + 1),
                                T_SUP * ti:T_SUP * (ti + 1)])

                def run_layer(l, src, out_dtype=F8, tag="h", extra_src=None):
                    pool_ = h3pool if l in (2, 5) else hpool
                    h_out = pool_.tile([128, 4 * T_SUP], out_dtype,
                                       tag=tag, name=f"h{l}")
                    for o in range(4):
                        ps = pspool.tile([128, T_SUP], F32, tag="ps",
                                         name="ps")
                        srcs = [src] if extra_src is None else [src,
                                                                extra_src]
                        n_mm = 4 * len(srcs)
                        mi = 0
                        for s_ in srcs:
                            for kp in range(2):
                                for th in range(2):
                                    nc.tensor.matmul(
                                        ps[:, 512 * th:512 * (th + 1)],
                                        wap(l, o, kp), rhs(s_, kp, th),
                                        start=(mi < 2),
                                        stop=(mi >= n_mm - 2),
                                        perf_mode=DR)
                                    mi += 1
                        relu_drain(ENG[l][o],
                                   h_out[:, T_SUP * o:T_SUP * (o + 1)],
                                   ps, b_sb[:, 4 * l + o:4 * l + o + 1])
                    return h_out

                h1 = run_layer(0, xt)
                h2 = run_layer(1, h1)
                h3a = run_layer(2, h2, tag="h3a")          # fp8: matmul input
                g1 = run_layer(3, xt, extra_src=h3a)        # fused residual
                g2 = run_layer(4, g1)
                h3b = run_layer(5, g2, out_dtype=BF16, tag="h3b")

                # pooling reduces (DVE only supports axis=X)
                for h3t, pb in ((h3a, pb1), (h3b, pb2)):
                    nc.vector.tensor_reduce(
                        pb[:].rearrange("p (o s) -> p o s", o=4,
                                        s=S_CORE)[:, :, S_SUP * ti:
                                                  S_SUP * (ti + 1)],
                        h3t[:].rearrange("p (o g t) -> p o g t", o=4,
                                         g=S_SUP),
                        axis=AX.X, op=ALU.add)

            # ---- pooled = x0s + pb1 + pb2 (bf16 out for the tail mms) ----
            pool_f32 = smpool.tile([128, 4 * S_CORE], F32, tag="poolf",
                                   name="pool_f32")
            nc.vector.tensor_add(pool_f32[:], pb1[:], pb2[:])
            pool_bf = smpool.tile([128, 4 * S_CORE], BF16, tag="poolb",
                                  name="pool_bf")
            nc.vector.tensor_add(pool_bf[:], pool_f32[:], x0s_sb[:])

            # ---- tail MLPs (bf16) ----
            f_prev = pool_bf
            scales = [1.0 / TOK, 1.0]
            for l in range(2):
                f_out = smpool.tile([128, 4 * S_CORE], BF16, tag=f"f{l}",
                                    name=f"f{l}")
                for o in range(4):
                    ps = pstpool.tile([128, S_CORE], F32, tag="pst",
                                      name="pst")
                    for k in range(4):
                        nc.tensor.matmul(
                            ps[:],
                            mw_sb[l][:, D * k + 128 * o:D * k + 128 * (o + 1)],
                            f_prev[:, S_CORE * k:S_CORE * (k + 1)],
                            start=(k == 0), stop=(k == 3))
                    nc.scalar.activation(
                        f_out[:, S_CORE * o:S_CORE * (o + 1)], ps[:], AF.Relu,
                        bias=mb_sb[:, 4 * l + o:4 * l + o + 1],
                        scale=scales[l])
                f_prev = f_out

            # ---- heads: [32 samples, 12] = t(3) ++ rot(9) ----
            psh = pstpool.tile([32, 12], F32, tag="pst", name="psh")
            for k in range(4):
                nc.tensor.matmul(psh[:],
                                 f_prev[:, S_CORE * k:S_CORE * (k + 1)],
                                 hw_sb[:, 12 * k:12 * (k + 1)],
                                 start=(k == 0), stop=(k == 3))
            mm = smpool.tile([32, 12], F32, tag="mm", name="mm")
            nc.vector.tensor_add(mm[:], psh[:], hb_sb[:])

            # ---- pose assembly + SVD ----
            pose_t = smpool.tile([32, 16], F32, tag="pose", name="pose_t")
            nc.vector.memset(pose_t[:], 0.0)
            nc.vector.memset(pose_t[:, 15:16], 1.0)
            nc.vector.tensor_copy(
                pose_t[:].rearrange("p (r c) -> p r c", r=4, c=4)[:, 0:3, 3],
                mm[:, 0:3])

            em = Emit(nc, smpool)
            emit_svd_so3(nc, em, mm[:, 3:12], pose_t)

            nc.sync.dma_start(pose[:], pose_t[:])

    nc.compile()
    return nc


_NC_CACHE = None


def _get_nc():
    global _NC_CACHE
    if _NC_CACHE is None:
        _NC_CACHE = build_nc()
    return _NC_CACHE


F8NP = ml_dtypes.float8_e4m3fn
BF16NP = ml_dtypes.bfloat16


def kernel(**inputs):
    feat = np.asarray(inputs["feat"], dtype=np.float32)
    b_, v_, n_, d_ = feat.shape
    xs = feat.reshape(b_ * v_, n_, d_)
    x0sum = xs.sum(axis=1, dtype=np.float32)          # (256, 512)

    # DoubleRow weight prepack: [p, o, kp, i, m] <- wT[128*(2kp+i)+p, 128o+m]
    w8_list = []
    for blk in (1, 2):
        for li in (1, 2, 3):
            wT = np.asarray(inputs[f"r{blk}_w{li}"], np.float32).T
            arr = wT.astype(F8NP).reshape(2, 2, 128, 4, 128)
            arr = np.ascontiguousarray(arr.transpose(2, 3, 0, 1, 4))
            w8_list.append(arr.reshape(128, 2048))
    w8 = np.stack(w8_list)
    bs = np.stack([np.asarray(inputs[f"r{blk}_b{li}"], np.float32)
                   for blk in (1, 2) for li in (1, 2, 3)])
    mwt = np.stack([np.ascontiguousarray(
        np.asarray(inputs[f"m_w{li}"], np.float32).T).astype(BF16NP)
        for li in (1, 2)])
    mbs = np.stack([np.asarray(inputs[f"m_b{li}"], np.float32)
                    for li in (1, 2)])
    hwT = np.ascontiguousarray(np.concatenate(
        [np.asarray(inputs["t_w"], np.float32).T,
         np.asarray(inputs["rot_w"], np.float32).T], axis=1)).astype(BF16NP)
    hb = np.broadcast_to(np.concatenate(
        [np.asarray(inputs["t_b"], np.float32),
         np.asarray(inputs["rot_b"], np.float32)])[None, :],
        (S_CORE, 12)).copy()

    in_maps = []
    for c in range(N_CORES):
        xT8 = np.ascontiguousarray(
            xs[c * S_CORE:(c + 1) * S_CORE].reshape(T_CORE, D).T).astype(F8NP)
        xs_c = x0sum[c * S_CORE:(c + 1) * S_CORE]     # (32, 512)
        x0s = np.ascontiguousarray(
            xs_c.T.reshape(4, 128, S_CORE).transpose(1, 0, 2).reshape(
                128, 4 * S_CORE))
        in_maps.append({
            "xT8": xT8, "w8": w8, "bs": bs, "x0s": x0s, "mwt": mwt,
            "mbs": mbs, "hwT": hwT, "hb": hb,
        })

    nc = _get_nc()
    import os
    kwargs = {}
    if os.environ.get("KERNEL_TRACE") == "1":
        kwargs["trace"] = True
    res = run_bass_kernel_spmd(nc, in_maps, core_ids=list(range(N_CORES)),
                               **kwargs)
    if kwargs.get("trace"):
        kernel.last_results = res
    poses = np.concatenate([r["pose"] for r in res.results], axis=0)
    return poses.reshape(b_, v_, 4, 4)


# revision 23
# speedup vs baseline: 1.4710x; 1.1539x over previous
"""CameraHead Trainium2 kernel — data-parallel over b*v across 8 NeuronCores.

fp8(e4m3) DoubleRow edition. Per-core layout: activations feature-major in
SBUF (X^T: 4x128-part chunks x tokens, fp8), all six 512x512 Linears run as
DoubleRow fp8 matmuls (K=256 per pass, 512-token moving operand, 2 passes for
K=512) at 2x the bf16/fp32r PE rate. Relu+bias drains PSUM on three engines
(ACT / DVE / GPSIMD) in parallel. The block-2 first Linear consumes x0 and
h3(block1) as two accumulated matmul streams, so the residual add never
materializes. Pooling = host-exact sum(x0) + on-device tensor_reduce over the
two h3 streams. Tail: bf16 MLP/head matmuls + a lean d-tracking Jacobi
(9 rotations) for the SVD->SO(3) projection, then pose assembly.
"""
import sys
import numpy as np

sys.path.insert(0, '/opt/trn_rl_repo')

import ml_dtypes  # noqa: E402

import concourse.bacc as bacc  # noqa: E402
import concourse.mybir as mybir  # noqa: E402
from concourse import tile  # noqa: E402
from concourse import dve_ops as _dvo  # noqa: E402
from concourse.bass_utils import run_bass_kernel_spmd  # noqa: E402
from concourse.dve_spec import (  # noqa: E402
    C0, C1, C2, One, Spec, Src0, Src1, select as dve_select, sq as dve_sq,
)


def _reg_op(name, body, ref):
    """Register a custom DVE op (per-NEFF uop table; no firmware change).

    The uops sha pin is bootstrapped by parsing compile()'s drift error."""
    for op in _dvo.OPS:
        if op.name == name:
            return op
    import re as _re

    from concourse.dve_table_gen import dve_ver_for

    row = _dvo._CUSTOM_DVE_ROW_BASE + len(_dvo.OPS)
    assert row < 0x20, "custom DVE opcode rows exhausted"
    spec = Spec(body=body, reference=ref)
    op = _dvo.DveOp(name, spec, subdim=False, uops_sha={})
    _dvo.OPS.append(op)
    _dvo._SUB_OPCODE_FOR_NAME[name] = row
    _dvo.CUSTOM_DVE_SPECS[name] = spec
    ver = dve_ver_for("TRN2")
    try:
        op.compile(ver)
    except ValueError as e:
        m = _re.search(r'uops_sha\["' + ver + r'"\]="([0-9a-f]+)"', str(e))
        if not m:
            raise
        op.uops_sha[ver] = m.group(1)
        op.compile(ver)
    return op


_f32 = np.float32
OP_AXPBY = _reg_op(
    "ANT_AXPBY", Src0 * C0 + Src1 * C1,
    lambda in0, in1, s0, s1, imm2: (in0 * s0 + in1 * s1).astype(_f32))
OP_AXMBY = _reg_op(
    "ANT_AXMBY", Src0 * C0 - Src1 * C1,
    lambda in0, in1, s0, s1, imm2: (in0 * s0 - in1 * s1).astype(_f32))
OP_AXPBY2 = _reg_op(
    "ANT_AXPBY2", (Src0 * C0 + Src1 * C1) * C2,
    lambda in0, in1, s0, s1, imm2: ((in0 * s0 + in1 * s1) * imm2).astype(_f32))
OP_AXMBY2 = _reg_op(
    "ANT_AXMBY2", (Src0 * C0 - Src1 * C1) * C2,
    lambda in0, in1, s0, s1, imm2: ((in0 * s0 - in1 * s1) * imm2).astype(_f32))
OP_SQDIFF = _reg_op(
    "ANT_SQDIFF", dve_sq(Src0) - dve_sq(Src1),
    lambda in0, in1, s0, s1, imm2: (in0 * in0 - in1 * in1).astype(_f32))


def _xy2_body():
    t = Src0 * Src1
    return t + t


OP_XY2 = _reg_op(
    "ANT_XY2", _xy2_body(),
    lambda in0, in1, s0, s1, imm2: (2.0 * in0 * in1).astype(_f32))
OP_WHERE = _reg_op(
    "ANT_WHERE", dve_select(C0, Src0, Src1),
    lambda in0, in1, s0, s1, imm2: np.where(
        s0 != 0, in0, in1).astype(_f32))
OP_WHERENEG = _reg_op(
    "ANT_WHERENEG", dve_select(C0, -Src0, Src1),
    lambda in0, in1, s0, s1, imm2: np.where(
        s0 != 0, -in0, in1).astype(_f32))
OP_SQSUM = _reg_op(
    "ANT_SQSUM", dve_sq(Src0) + dve_sq(Src1),
    lambda in0, in1, s0, s1, imm2: (in0 * in0 + in1 * in1).astype(_f32))
# (ch^2 - sh^2) * w2  — raw cosine
OP_CSUBW = _reg_op(
    "ANT_CSUBW", (dve_sq(Src0) - dve_sq(Src1)) * C0,
    lambda in0, in1, s0, s1, imm2: (
        (in0 * in0 - in1 * in1) * s0).astype(_f32))


def _xyw_body():
    t = Src0 * Src1
    return (t + t) * C0


# 2 * ch * sh * w2 — raw sine
OP_XYW2 = _reg_op(
    "ANT_XYW2", _xyw_body(),
    lambda in0, in1, s0, s1, imm2: (2.0 * in0 * in1 * s0).astype(_f32))
# gate value: ch^2 - gamma*sh^2 (>0 -> use raw angle, else pi/4 fallback)
OP_GATE = _reg_op(
    "ANT_GATE", dve_sq(Src0) - dve_sq(Src1) * C1,
    lambda in0, in1, s0, s1, imm2: (
        in0 * in0 - in1 * in1 * s1).astype(_f32))
# select(g < 0, imm2, raw)
OP_SELPOS = _reg_op(
    "ANT_SELPOS", dve_select(Src1 < C1, C2, Src0),
    lambda in0, in1, s0, s1, imm2: np.where(
        in1 < s1, imm2, in0).astype(_f32))


def _axpb4y_body():
    t = Src1 * C1
    u = t + t
    return Src0 * C0 + (u + u)


# d' = c2*d + 4*s2*pt
OP_AXPB4Y = _reg_op(
    "ANT_AXPB4Y", _axpb4y_body(),
    lambda in0, in1, s0, s1, imm2: (in0 * s0 + 4.0 * in1 * s1).astype(_f32))
# pt' = c2*pt - 0.25*s2*d   (imm2 carries the 0.25)
OP_AXMBYC = _reg_op(
    "ANT_AXMBYC", Src0 * C0 - (Src1 * C1) * C2,
    lambda in0, in1, s0, s1, imm2: (in0 * s0 - in1 * s1 * imm2).astype(_f32))

F32 = mybir.dt.float32
BF16 = mybir.dt.bfloat16
F8 = mybir.dt.float8e4
AF = mybir.ActivationFunctionType
ALU = mybir.AluOpType
AX = mybir.AxisListType
DR = mybir.MatmulPerfMode.DoubleRow

N_CORES = 8
D = 512
SAMPLES = 256          # b*v
TOK = 256              # tokens per sample
S_CORE = SAMPLES // N_CORES       # 32 samples per core
T_CORE = S_CORE * TOK             # 8192 token rows per core
T_TILE = 512
N_TILES = T_CORE // T_TILE        # 16
S_TILE = T_TILE // TOK            # 2 samples per token tile

T_SUP = 1024           # supertile: 2 token-tiles drained with wide ops
N_SUP = T_CORE // T_SUP           # 8
S_SUP = T_SUP // TOK              # 4 samples per supertile

N_ROT = 9              # lean Jacobi rotations (sim: rel_err 3.3e-3)

GAMMA = float(3.0 + 2.0 * np.sqrt(2.0))
CQ45 = float(np.cos(np.pi / 4))


# ---------------------------------------------------------------------------
# small-op emitter for the SVD tail: SSA-style column allocation on a scratch
# tile; every value is an AP (or list of APs).
# ---------------------------------------------------------------------------
class Emit:
    def __init__(self, nc, pool):
        self.nc = nc
        self.scr = pool.tile([32, 2048], F32, tag="svd_scratch",
                             name="svd_scratch")
        self.ptr = 0

    def new(self, n=1):
        c = self.ptr
        self.ptr += n
        assert self.ptr <= 2048, "svd scratch overflow"
        return self.scr[:, c:c + n]

    def tt(self, op, a, b, n=1):
        o = self.new(n)
        self.nc.vector.tensor_tensor(o, a, b, op)
        return o

    def tt3(self, op, a, b, n=9):
        o = self.new(n)
        self.nc.vector.tensor_tensor(
            o.rearrange("p (i j) -> p i j", i=3, j=n // 3), a, b, op)
        return o

    def ts(self, op, a, s, n=1):
        o = self.new(n)
        self.nc.vector.tensor_scalar(o, a, s, None, op)
        return o

    def stt(self, a, scal, b, op0, op1, n=1):
        o = self.new(n)
        self.nc.vector.scalar_tensor_tensor(o, a, scal, b, op0=op0, op1=op1)
        return o

    # --- gpsimd variant (tensor_tensor only; Pool supports no Ptr ops) ---
    def gtt(self, op, a, b, n=1):
        o = self.new(n)
        self.nc.gpsimd.tensor_tensor(o, a, b, op)
        return o

    def rsqrt(self, a, n=1):
        t = self.new(n)
        self.nc.scalar.activation(t, a, AF.Sqrt)
        o = self.new(n)
        self.nc.vector.reciprocal(o, t)
        return o

    def cdve(self, op, in0, in1, s0=0.0, s1=0.0, imm2=0.0, n=1, out=None):
        if out is None:
            out = self.new(n)
        self.nc.vector._custom_dve(op, out=out, in0=in0, in1=in1,
                                   s0=s0, s1=s1, imm2=imm2)
        return out

    def const(self, val, n=1):
        o = self.new(n)
        self.nc.vector.memset(o, val)
        return o


def _bcast_r(ap3):
    return ap3.unsqueeze(2).broadcast_to([32, 3, 3])


def _bcast_l(ap3):
    return ap3.unsqueeze(1).broadcast_to([32, 3, 3])


def emit_svd_so3(nc, em, m_ap, pose_tile):
    """m_ap: [32,9] raw 3x3 per sample (row-major). Writes the SO(3)
    projection into pose_tile columns (4r+c for r,c in 0..2).

    Lean d-tracking Jacobi: state is (d01,d12,d02) eigenvalue differences and
    (p01,p12,p02) halved off-diagonals; 15 DVE ops per rotation."""
    # --- row normalize ---
    sq = em.tt(ALU.mult, m_ap, m_ap, 9)
    t = em.tt(ALU.add, sq[:, 0:9:3], sq[:, 1:9:3], 3)
    r2 = em.tt(ALU.add, t, sq[:, 2:9:3], 3)
    r2c = em.ts(ALU.max, r2, 1e-24, 3)
    rinv = em.rsqrt(r2c, 3)
    A = em.tt3(ALU.mult, m_ap.rearrange("p (r c) -> p r c", r=3, c=3),
               _bcast_r(rinv), 9)

    # --- S = A^T A (s_ij at col 3i+j) ---
    terms = []
    for r in range(3):
        arow = A[:, 3 * r:3 * r + 3]
        terms.append(em.tt3(ALU.mult, _bcast_r(arow), _bcast_l(arow), 9))
    s01 = em.tt(ALU.add, terms[0], terms[1], 9)
    S9 = em.tt(ALU.add, s01, terms[2], 9)

    # d/pt state (SSA-tracked APs)
    d01 = em.tt(ALU.subtract, S9[:, 0:1], S9[:, 4:5])
    d12 = em.tt(ALU.subtract, S9[:, 4:5], S9[:, 8:9])
    d02 = em.tt(ALU.add, d01, d12)
    p01 = em.ts(ALU.mult, S9[:, 1:2], 0.5)
    p12 = em.ts(ALU.mult, S9[:, 5:6], 0.5)
    p02 = em.ts(ALU.mult, S9[:, 2:3], 0.5)

    # V columns as [32,3] blocks, init = identity
    Vc = []
    for j in range(3):
        vj = em.new(3)
        nc.vector.memset(vj, 0.0)
        nc.vector.memset(vj[:, j:j + 1], 1.0)
        Vc.append(vj)

    st = {'d01': d01, 'd12': d12, 'd02': d02,
          'p01': p01, 'p12': p12, 'p02': p02}

    def angle(ch, sh):
        ssum = em.cdve(OP_SQSUM, ch, sh)
        w2 = em.new(1)
        nc.vector.reciprocal(w2, ssum)
        craw = em.cdve(OP_CSUBW, ch, sh, s0=w2)
        sraw = em.cdve(OP_XYW2, ch, sh, s0=w2)
        g = em.cdve(OP_GATE, ch, sh, s1=GAMMA)
        c = em.cdve(OP_SELPOS, craw, g, s1=0.0, imm2=CQ45)
        s = em.cdve(OP_SELPOS, sraw, g, s1=0.0, imm2=CQ45)
        c2 = em.cdve(OP_SQDIFF, c, s)
        s2 = em.cdve(OP_XY2, c, s)
        return c, s, c2, s2

    def vup(p, q, c, s):
        # V rotation runs on GPSIMD, off the DVE critical chain (the V
        # columns are only consumed after the rotation loop).
        cb = c.broadcast_to([32, 3])
        sb = s.broadcast_to([32, 3])
        t1 = em.gtt(ALU.mult, Vc[q], sb, 3)
        t2 = em.gtt(ALU.mult, Vc[p], cb, 3)
        nvp = em.gtt(ALU.add, t2, t1, 3)
        t3 = em.gtt(ALU.mult, Vc[p], sb, 3)
        t4 = em.gtt(ALU.mult, Vc[q], cb, 3)
        nvq = em.gtt(ALU.subtract, t4, t3, 3)
        Vc[p], Vc[q] = nvp, nvq

    for k in range(N_ROT):
        rt = k % 3
        if rt == 0:     # (p,q,r) = (0,1,2)
            c, s, c2, s2 = angle(st['d01'], st['p01'])
            dd = em.cdve(OP_AXPB4Y, st['d01'], st['p01'], s0=c2, s1=s2)
            pn = em.cdve(OP_AXMBYC, st['p01'], st['d01'], s0=c2, s1=s2,
                         imm2=0.25)
            t_ = em.tt(ALU.add, st['d02'], st['d12'])
            nd02 = em.cdve(OP_AXPBY, t_, dd, s0=0.5, s1=0.5)
            nd12 = em.cdve(OP_AXMBY, t_, dd, s0=0.5, s1=0.5)
            np02 = em.cdve(OP_AXPBY, st['p02'], st['p12'], s0=c, s1=s)
            np12 = em.cdve(OP_AXMBY, st['p12'], st['p02'], s0=c, s1=s)
            st.update(d01=dd, p01=pn, d02=nd02, d12=nd12, p02=np02, p12=np12)
            vup(0, 1, c, s)
        elif rt == 1:   # (1,2,0)
            c, s, c2, s2 = angle(st['d12'], st['p12'])
            dd = em.cdve(OP_AXPB4Y, st['d12'], st['p12'], s0=c2, s1=s2)
            pn = em.cdve(OP_AXMBYC, st['p12'], st['d12'], s0=c2, s1=s2,
                         imm2=0.25)
            t_ = em.tt(ALU.add, st['d01'], st['d02'])
            nd01 = em.cdve(OP_AXMBY, t_, dd, s0=0.5, s1=0.5)
            nd02 = em.cdve(OP_AXPBY, t_, dd, s0=0.5, s1=0.5)
            np01 = em.cdve(OP_AXPBY, st['p01'], st['p02'], s0=c, s1=s)
            np02 = em.cdve(OP_AXMBY, st['p02'], st['p01'], s0=c, s1=s)
            st.update(d12=dd, p12=pn, d01=nd01, d02=nd02, p01=np01, p02=np02)
            vup(1, 2, c, s)
        else:           # (0,2,1)
            c, s, c2, s2 = angle(st['d02'], st['p02'])
            dd = em.cdve(OP_AXPB4Y, st['d02'], st['p02'], s0=c2, s1=s2)
            pn = em.cdve(OP_AXMBYC, st['p02'], st['d02'], s0=c2, s1=s2,
                         imm2=0.25)
            t_ = em.tt(ALU.subtract, st['d01'], st['d12'])
            nd01 = em.cdve(OP_AXPBY, t_, dd, s0=0.5, s1=0.5)
            nd12 = em.cdve(OP_AXMBY, dd, t_, s0=0.5, s1=0.5)
            np01 = em.cdve(OP_AXPBY, st['p01'], st['p12'], s0=c, s1=s)
            np12 = em.cdve(OP_AXMBY, st['p12'], st['p01'], s0=c, s1=s)
            st.update(d02=dd, p02=pn, d01=nd01, d12=nd12, p01=np01, p12=np12)
            vup(0, 2, c, s)

    # --- sort eigenpairs descending (det(V) stays +1 via column negation) ---
    def cond_swap(i, j):
        key_ij = f'd{i}{j}'
        k = 3 - i - j
        key_ik = f'd{min(i,k)}{max(i,k)}'
        key_jk = f'd{min(j,k)}{max(j,k)}'
        mask = em.ts(ALU.is_lt, st[key_ij], 0.0)
        nij = em.cdve(OP_WHERENEG, st[key_ij], st[key_ij], s0=mask)
        nik = em.cdve(OP_WHERE, st[key_jk], st[key_ik], s0=mask)
        njk = em.cdve(OP_WHERE, st[key_ik], st[key_jk], s0=mask)
        st[key_ij], st[key_ik], st[key_jk] = nij, nik, njk
        # V swap on GPSIMD: vi' = vi + m*(vj-vi); vj' = vj - m*(vi+vj)
        m3 = mask.broadcast_to([32, 3])
        dv = em.gtt(ALU.subtract, Vc[j], Vc[i], 3)
        md = em.gtt(ALU.mult, dv, m3, 3)
        vi = em.gtt(ALU.add, Vc[i], md, 3)
        sv = em.gtt(ALU.add, Vc[i], Vc[j], 3)
        ms = em.gtt(ALU.mult, sv, m3, 3)
        vj = em.gtt(ALU.subtract, Vc[j], ms, 3)
        Vc[i], Vc[j] = vi, vj

    cond_swap(0, 1)
    cond_swap(1, 2)
    cond_swap(0, 1)

    # --- B columns (j=0,1): b_j[r] = sum_c A[r][c] * V[c][j] ---
    Astr = [A[:, c:c + 7:3] for c in range(3)]

    def bcol(j):
        t0 = em.cdve(OP_AXPBY, Astr[0], Astr[1],
                     s0=Vc[j][:, 0:1], s1=Vc[j][:, 1:2], n=3)
        return em.stt(Astr[2], Vc[j][:, 2:3], t0, ALU.mult, ALU.add, 3)

    def bcol_gps(j):
        t0 = em.gtt(ALU.mult, Astr[0],
                    Vc[j][:, 0:1].broadcast_to([32, 3]), 3)
        t1 = em.gtt(ALU.mult, Astr[1],
                    Vc[j][:, 1:2].broadcast_to([32, 3]), 3)
        t01 = em.gtt(ALU.add, t0, t1, 3)
        t2 = em.gtt(ALU.mult, Astr[2],
                    Vc[j][:, 2:3].broadcast_to([32, 3]), 3)
        return em.gtt(ALU.add, t01, t2, 3)

    b0 = bcol(0)
    b1 = bcol_gps(1)

    def normalize(v3):
        sqv = em.tt(ALU.mult, v3, v3, 3)
        n_ = em.tt(ALU.add, sqv[:, 0:1], sqv[:, 1:2])
        n_ = em.tt(ALU.add, n_, sqv[:, 2:3])
        nc_ = em.ts(ALU.max, n_, 1e-30)
        inv = em.rsqrt(nc_)
        return em.ts(ALU.mult, v3, inv, 3)

    u1 = normalize(b0)
    p_ = em.tt(ALU.mult, u1, b1, 3)
    d_ = em.tt(ALU.add, p_[:, 0:1], p_[:, 1:2])
    d_ = em.tt(ALU.add, d_, p_[:, 2:3])
    dneg = em.ts(ALU.mult, d_, -1.0)
    b2o = em.stt(u1, dneg, b1, ALU.mult, ALU.add, 3)
    u2 = normalize(b2o)
    u3 = em.new(3)
    for k, (i1, i2) in enumerate(((1, 2), (2, 0), (0, 1))):
        em.cdve(OP_AXMBY, u1[:, i1:i1 + 1], u1[:, i2:i2 + 1],
                s0=u2[:, i2:i2 + 1], s1=u2[:, i1:i1 + 1],
                out=u3[:, k:k + 1])

    # --- R = u1 v1^T + u2 v2^T + u3 v3^T ---
    t0 = em.tt3(ALU.mult, _bcast_r(u1), _bcast_l(Vc[0]), 9)
    t1 = em.tt3(ALU.mult, _bcast_r(u2), _bcast_l(Vc[1]), 9)
    t01 = em.tt(ALU.add, t0, t1, 9)
    t2 = em.tt3(ALU.mult, _bcast_r(u3), _bcast_l(Vc[2]), 9)
    pose_R = pose_tile[:].rearrange("p (r c) -> p r c", r=4, c=4)[:, 0:3, 0:3]
    nc.vector.tensor_tensor(
        pose_R, t01.rearrange("p (r c) -> p r c", r=3, c=3),
        t2.rearrange("p (r c) -> p r c", r=3, c=3), ALU.add)


# ---------------------------------------------------------------------------
# kernel build
# ---------------------------------------------------------------------------
def build_nc():
    nc = bacc.Bacc("TRN2", target_bir_lowering=False)

    xT8 = nc.dram_tensor("xT8", [D, T_CORE], F8, kind="ExternalInput")
    w8 = nc.dram_tensor("w8", [6, 128, 2048], F8, kind="ExternalInput")
    bs = nc.dram_tensor("bs", [6, D], F32, kind="ExternalInput")
    x0s = nc.dram_tensor("x0s", [128, 4 * S_CORE], F32, kind="ExternalInput")
    mwt = nc.dram_tensor("mwt", [2, D, D], BF16, kind="ExternalInput")
    mbs = nc.dram_tensor("mbs", [2, D], F32, kind="ExternalInput")
    hwT = nc.dram_tensor("hwT", [D, 12], BF16, kind="ExternalInput")
    hb = nc.dram_tensor("hb", [S_CORE, 12], F32, kind="ExternalInput")
    pose = nc.dram_tensor("pose", [S_CORE, 16], F32, kind="ExternalOutput")

    with tile.TileContext(nc) as tc:
        with (
            tc.tile_pool(name="wp", bufs=1) as wpool,
            tc.tile_pool(name="xp", bufs=3) as xpool,
            tc.tile_pool(name="hp", bufs=2) as hpool,
            tc.tile_pool(name="h3p", bufs=2) as h3pool,
            tc.tile_pool(name="pp", bufs=1) as ppool,
            tc.tile_pool(name="ps", bufs=4, space="PSUM") as pspool,
            tc.tile_pool(name="sm", bufs=1) as smpool,
        ):
            # warm the ACT function table while DMAs stream
            warm = smpool.tile([32, 1], F32, tag="warm", name="warm")
            nc.vector.memset(warm[:], 0.0)
            nc.scalar.activation(warm[:], warm[:], AF.Relu)

            # ---- first x supertile + layer-0 weights first, spread across
            # four DMA queues so the PE can start ~4us in
            xt0 = xpool.tile([128, 4 * T_SUP], F8, tag="xt", name="xt")
            w_sb = [wpool.tile([128, 2048], F8, tag=f"w{l}", name=f"w{l}")
                    for l in range(6)]
            b_sb = wpool.tile([128, 24], F32, tag="b", name="b_sb")
            nc.sync.dma_start(w_sb[0][:], w8[0])
            for k, eng in enumerate((nc.gpsimd, nc.scalar, nc.sync,
                                     nc.gpsimd)):
                eng.dma_start(xt0[:, T_SUP * k:T_SUP * (k + 1)],
                              xT8[128 * k:128 * (k + 1), 0:T_SUP])
            # all 24 bias columns in one strided DMA
            nc.scalar.dma_start(
                b_sb[:].rearrange("p (l o) -> p l o", l=6, o=4),
                bs[:].rearrange("l (o p) -> p l o", p=128, o=4))
            for l in range(1, 6):
                nc.sync.dma_start(w_sb[l][:], w8[l])
            x0s_sb = wpool.tile([128, 4 * S_CORE], F32, tag="x0s",
                                name="x0s_sb")
            nc.sync.dma_start(x0s_sb[:], x0s[:])
            mw_sb = [wpool.tile([128, 2048], BF16, tag=f"mw{l}",
                                name=f"mw{l}") for l in range(2)]
            for l in range(2):
                for k in range(4):
                    nc.sync.dma_start(
                        mw_sb[l][:, D * k:D * (k + 1)],
                        mwt[l, 128 * k:128 * (k + 1), :])
            mb_sb = wpool.tile([128, 8], F32, tag="mb", name="mb_sb")
            for l in range(2):
                nc.sync.dma_start(mb_sb[:, 4 * l:4 * l + 4],
                                  mbs[l].rearrange("(o p) -> p o", p=128, o=4))
            hw_sb = wpool.tile([128, 48], BF16, tag="hw", name="hw_sb")
            for k in range(4):
                nc.sync.dma_start(hw_sb[:, 12 * k:12 * (k + 1)],
                                  hwT[128 * k:128 * (k + 1), :])
            hb_sb = wpool.tile([32, 12], F32, tag="hbt", name="hb_sb")
            nc.sync.dma_start(hb_sb[:], hb[:])

            # per-tile h3 sums for the two blocks: [128, 4k x 32 samples]
            pb1 = ppool.tile([128, 4 * S_CORE], F32, tag="pb1", name="pb1")
            pb2 = ppool.tile([128, 4 * S_CORE], F32, tag="pb2", name="pb2")

            def wap(l, o, kp):
                c0 = (o * 2 + kp) * 256
                return w_sb[l][:, c0:c0 + 256].rearrange(
                    "p (i m) -> p i m", i=2)

            def rhs(t, kp, th):
                # kp-pair chunks of a [128, 4*T_SUP] supertile, token half th
                return t[:, 2 * T_SUP * kp:2 * T_SUP * (kp + 1)].rearrange(
                    "p (i n) -> p i n", i=2)[:, :, 512 * th:512 * (th + 1)]

            def relu_drain(engine, h_slice, ps, bias_ap):
                if engine == 'act':
                    nc.scalar.activation(h_slice, ps[:], AF.Relu,
                                         bias=bias_ap, scale=1.0)
                else:
                    nc.vector.tensor_scalar(h_slice, ps[:], bias_ap, 0.0,
                                            ALU.add, ALU.max)

            # engine assignment per layer (ACT 16 / DVE 8 drains per sup;
            # DVE also owns the two pooling reduces)
            ENG = {
                0: ['act'] * 4,
                1: ['dve'] * 4,
                2: ['act'] * 4,
                3: ['act'] * 4,
                4: ['act'] * 4,
                5: ['dve'] * 4,
            }

            # ---- main loop over supertiles (1024 tokens each) ----
            for ti in range(N_SUP):
                if ti == 0:
                    xt = xt0
                else:
                    xt = xpool.tile([128, 4 * T_SUP], F8, tag="xt",
                                    name="xt")
                    for k in range(4):
                        nc.gpsimd.dma_start(
                            xt[:, T_SUP * k:T_SUP * (k + 1)],
                            xT8[128 * k:128 * (k + 1),
                                T_SUP * ti:T_SUP * (ti + 1)])

                def run_layer(l, src, out_dtype=F8, tag="h", extra_src=None):
                    pool_ = h3pool if l in (2, 5) else hpool
                    h_out = pool_.tile([128, 4 * T_SUP], out_dtype,
                                       tag=tag, name=f"h{l}")
                    for o in range(4):
                        ps = pspool.tile([128, T_SUP], F32, tag="ps",
                                         name="ps")
                        srcs = [src] if extra_src is None else [src,
                                                                extra_src]
                        n_mm = 4 * len(srcs)
                        mi = 0
                        for s_ in srcs:
                            for kp in range(2):
                                for th in range(2):
                                    nc.tensor.matmul(
                                        ps[:, 512 * th:512 * (th + 1)],
                                        wap(l, o, kp), rhs(s_, kp, th),
                                        start=(mi < 2),
                                        stop=(mi >= n_mm - 2),
                                        perf_mode=DR)
                                    mi += 1
                        relu_drain(ENG[l][o],
                                   h_out[:, T_SUP * o:T_SUP * (o + 1)],
                                   ps, b_sb[:, 4 * l + o:4 * l + o + 1])
                    return h_out

                h1 = run_layer(0, xt)
                h2 = run_layer(1, h1)
                h3a = run_layer(2, h2, tag="h3a")          # fp8: matmul input
                g1 = run_layer(3, xt, extra_src=h3a)        # fused residual
                g2 = run_layer(4, g1)
                h3b = run_layer(5, g2, out_dtype=BF16, tag="h3b")

                # pooling reduces (DVE only supports axis=X)
                for h3t, pb in ((h3a, pb1), (h3b, pb2)):
                    nc.vector.tensor_reduce(
                        pb[:].rearrange("p (o s) -> p o s", o=4,
                                        s=S_CORE)[:, :, S_SUP * ti:
                                                  S_SUP * (ti + 1)],
                        h3t[:].rearrange("p (o g t) -> p o g t", o=4,
                                         g=S_SUP),
                        axis=AX.X, op=ALU.add)

            # ---- pooled = x0s + pb1 + pb2, per k-chunk so the first tail
            # matmuls start before the whole combine finishes ----
            pool_f32 = smpool.tile([128, 4 * S_CORE], F32, tag="poolf",
                                   name="pool_f32")
            pool_bf = smpool.tile([128, 4 * S_CORE], BF16, tag="poolb",
                                  name="pool_bf")
            for k in range(4):
                sl = slice(S_CORE * k, S_CORE * (k + 1))
                nc.vector.tensor_add(pool_f32[:, sl], pb1[:, sl], pb2[:, sl])
                nc.vector.tensor_add(pool_bf[:, sl], pool_f32[:, sl],
                                     x0s_sb[:, sl])

            # ---- tail MLPs (bf16), psum reused from the main pool ----
            f_prev = pool_bf
            scales = [1.0 / TOK, 1.0]
            for l in range(2):
                f_out = smpool.tile([128, 4 * S_CORE], BF16, tag=f"f{l}",
                                    name=f"f{l}")
                for o in range(4):
                    ps_w = pspool.tile([128, T_SUP], F32, tag="ps",
                                       name="pst")
                    ps = ps_w[:, 0:S_CORE]
                    for k in range(4):
                        nc.tensor.matmul(
                            ps,
                            mw_sb[l][:, D * k + 128 * o:D * k + 128 * (o + 1)],
                            f_prev[:, S_CORE * k:S_CORE * (k + 1)],
                            start=(k == 0), stop=(k == 3))
                    nc.scalar.activation(
                        f_out[:, S_CORE * o:S_CORE * (o + 1)], ps, AF.Relu,
                        bias=mb_sb[:, 4 * l + o:4 * l + o + 1],
                        scale=scales[l])
                f_prev = f_out

            # ---- heads: [32 samples, 12] = t(3) ++ rot(9) ----
            psh_w = pspool.tile([128, T_SUP], F32, tag="ps", name="psh")
            psh = psh_w[0:32, 0:12]
            for k in range(4):
                nc.tensor.matmul(psh,
                                 f_prev[:, S_CORE * k:S_CORE * (k + 1)],
                                 hw_sb[:, 12 * k:12 * (k + 1)],
                                 start=(k == 0), stop=(k == 3))
            mm = smpool.tile([32, 12], F32, tag="mm", name="mm")
            nc.vector.tensor_add(mm[:], psh, hb_sb[:])

            # ---- pose assembly + SVD ----
            pose_t = smpool.tile([32, 16], F32, tag="pose", name="pose_t")
            nc.vector.memset(pose_t[:], 0.0)
            nc.vector.memset(pose_t[:, 15:16], 1.0)
            nc.vector.tensor_copy(
                pose_t[:].rearrange("p (r c) -> p r c", r=4, c=4)[:, 0:3, 3],
                mm[:, 0:3])

            em = Emit(nc, smpool)
            emit_svd_so3(nc, em, mm[:, 3:12], pose_t)

            nc.sync.dma_start(pose[:], pose_t[:])

    nc.compile()
    return nc


_NC_CACHE = None


def _get_nc():
    global _NC_CACHE
    if _NC_CACHE is None:
        _NC_CACHE = build_nc()
    return _NC_CACHE


F8NP = ml_dtypes.float8_e4m3fn
BF16NP = ml_dtypes.bfloat16


def kernel(**inputs):
    feat = np.asarray(inputs["feat"], dtype=np.float32)
    b_, v_, n_, d_ = feat.shape
    xs = feat.reshape(b_ * v_, n_, d_)
    x0sum = xs.sum(axis=1, dtype=np.float32)          # (256, 512)

    # DoubleRow weight prepack: [p, o, kp, i, m] <- wT[128*(2kp+i)+p, 128o+m]
    w8_list = []
    for blk in (1, 2):
        for li in (1, 2, 3):
            wT = np.asarray(inputs[f"r{blk}_w{li}"], np.float32).T
            arr = wT.astype(F8NP).reshape(2, 2, 128, 4, 128)
            arr = np.ascontiguousarray(arr.transpose(2, 3, 0, 1, 4))
            w8_list.append(arr.reshape(128, 2048))
    w8 = np.stack(w8_list)
    bs = np.stack([np.asarray(inputs[f"r{blk}_b{li}"], np.float32)
                   for blk in (1, 2) for li in (1, 2, 3)])
    mwt = np.stack([np.ascontiguousarray(
        np.asarray(inputs[f"m_w{li}"], np.float32).T).astype(BF16NP)
        for li in (1, 2)])
    mbs = np.stack([np.asarray(inputs[f"m_b{li}"], np.float32)
                    for li in (1, 2)])
    hwT = np.ascontiguousarray(np.concatenate(
        [np.asarray(inputs["t_w"], np.float32).T,
         np.asarray(inputs["rot_w"], np.float32).T], axis=1)).astype(BF16NP)
    hb = np.broadcast_to(np.concatenate(
        [np.asarray(inputs["t_b"], np.float32),
         np.asarray(inputs["rot_b"], np.float32)])[None, :],
        (S_CORE, 12)).copy()

    in_maps = []
    for c in range(N_CORES):
        xT8 = np.ascontiguousarray(
            xs[c * S_CORE:(c + 1) * S_CORE].reshape(T_CORE, D).T).astype(F8NP)
        xs_c = x0sum[c * S_CORE:(c + 1) * S_CORE]     # (32, 512)
        x0s = np.ascontiguousarray(
            xs_c.T.reshape(4, 128, S_CORE).transpose(1, 0, 2).reshape(
                128, 4 * S_CORE))
        in_maps.append({
            "xT8": xT8, "w8": w8, "bs": bs, "x0s": x0s, "mwt": mwt,
            "mbs": mbs, "hwT": hwT, "hb": hb,
        })

    nc = _get_nc()
    import os
    kwargs = {}
    if os.environ.get("KERNEL_TRACE") == "1":
        kwargs["trace"] = True
    res = run_bass_kernel_spmd(nc, in_maps, core_ids=list(range(N_CORES)),
                               **kwargs)
    if kwargs.get("trace"):
        kernel.last_results = res
    poses = np.concatenate([r["pose"] for r in res.results], axis=0)
    return poses.reshape(b_, v_, 4, 4)


# revision 26
# speedup vs baseline: 1.4788x; 1.0053x over previous
"""CameraHead Trainium2 kernel — data-parallel over b*v across 8 NeuronCores.

fp8(e4m3) DoubleRow edition. Per-core layout: activations feature-major in
SBUF (X^T: 4x128-part chunks x tokens, fp8), all six 512x512 Linears run as
DoubleRow fp8 matmuls (K=256 per pass, 512-token moving operand, 2 passes for
K=512) at 2x the bf16/fp32r PE rate. Relu+bias drains PSUM on three engines
(ACT / DVE / GPSIMD) in parallel. The block-2 first Linear consumes x0 and
h3(block1) as two accumulated matmul streams, so the residual add never
materializes. Pooling = host-exact sum(x0) + on-device tensor_reduce over the
two h3 streams. Tail: bf16 MLP/head matmuls + a lean d-tracking Jacobi
(9 rotations) for the SVD->SO(3) projection, then pose assembly.
"""
import sys
import numpy as np

sys.path.insert(0, '/opt/trn_rl_repo')

import ml_dtypes  # noqa: E402

import concourse.bacc as bacc  # noqa: E402
import concourse.mybir as mybir  # noqa: E402
from concourse import tile  # noqa: E402
from concourse import dve_ops as _dvo  # noqa: E402
from concourse.bass_utils import run_bass_kernel_spmd  # noqa: E402
from concourse.dve_spec import (  # noqa: E402
    C0, C1, C2, One, Spec, Src0, Src1, select as dve_select, sq as dve_sq,
)


def _reg_op(name, body, ref):
    """Register a custom DVE op (per-NEFF uop table; no firmware change).

    The uops sha pin is bootstrapped by parsing compile()'s drift error."""
    for op in _dvo.OPS:
        if op.name == name:
            return op
    import re as _re

    from concourse.dve_table_gen import dve_ver_for

    row = _dvo._CUSTOM_DVE_ROW_BASE + len(_dvo.OPS)
    assert row < 0x20, "custom DVE opcode rows exhausted"
    spec = Spec(body=body, reference=ref)
    op = _dvo.DveOp(name, spec, subdim=False, uops_sha={})
    _dvo.OPS.append(op)
    _dvo._SUB_OPCODE_FOR_NAME[name] = row
    _dvo.CUSTOM_DVE_SPECS[name] = spec
    ver = dve_ver_for("TRN2")
    try:
        op.compile(ver)
    except ValueError as e:
        m = _re.search(r'uops_sha\["' + ver + r'"\]="([0-9a-f]+)"', str(e))
        if not m:
            raise
        op.uops_sha[ver] = m.group(1)
        op.compile(ver)
    return op


_f32 = np.float32
OP_AXPBY = _reg_op(
    "ANT_AXPBY", Src0 * C0 + Src1 * C1,
    lambda in0, in1, s0, s1, imm2: (in0 * s0 + in1 * s1).astype(_f32))
OP_AXMBY = _reg_op(
    "ANT_AXMBY", Src0 * C0 - Src1 * C1,
    lambda in0, in1, s0, s1, imm2: (in0 * s0 - in1 * s1).astype(_f32))
OP_AXPBY2 = _reg_op(
    "ANT_AXPBY2", (Src0 * C0 + Src1 * C1) * C2,
    lambda in0, in1, s0, s1, imm2: ((in0 * s0 + in1 * s1) * imm2).astype(_f32))
OP_AXMBY2 = _reg_op(
    "ANT_AXMBY2", (Src0 * C0 - Src1 * C1) * C2,
    lambda in0, in1, s0, s1, imm2: ((in0 * s0 - in1 * s1) * imm2).astype(_f32))
OP_SQDIFF = _reg_op(
    "ANT_SQDIFF", dve_sq(Src0) - dve_sq(Src1),
    lambda in0, in1, s0, s1, imm2: (in0 * in0 - in1 * in1).astype(_f32))


def _xy2_body():
    t = Src0 * Src1
    return t + t


OP_XY2 = _reg_op(
    "ANT_XY2", _xy2_body(),
    lambda in0, in1, s0, s1, imm2: (2.0 * in0 * in1).astype(_f32))
OP_WHERE = _reg_op(
    "ANT_WHERE", dve_select(C0, Src0, Src1),
    lambda in0, in1, s0, s1, imm2: np.where(
        s0 != 0, in0, in1).astype(_f32))
OP_WHERENEG = _reg_op(
    "ANT_WHERENEG", dve_select(C0, -Src0, Src1),
    lambda in0, in1, s0, s1, imm2: np.where(
        s0 != 0, -in0, in1).astype(_f32))
OP_SQSUM = _reg_op(
    "ANT_SQSUM", dve_sq(Src0) + dve_sq(Src1),
    lambda in0, in1, s0, s1, imm2: (in0 * in0 + in1 * in1).astype(_f32))
# (ch^2 - sh^2) * w2  — raw cosine
OP_CSUBW = _reg_op(
    "ANT_CSUBW", (dve_sq(Src0) - dve_sq(Src1)) * C0,
    lambda in0, in1, s0, s1, imm2: (
        (in0 * in0 - in1 * in1) * s0).astype(_f32))


def _xyw_body():
    t = Src0 * Src1
    return (t + t) * C0


# 2 * ch * sh * w2 — raw sine
OP_XYW2 = _reg_op(
    "ANT_XYW2", _xyw_body(),
    lambda in0, in1, s0, s1, imm2: (2.0 * in0 * in1 * s0).astype(_f32))
# gate value: ch^2 - gamma*sh^2 (>0 -> use raw angle, else pi/4 fallback)
OP_GATE = _reg_op(
    "ANT_GATE", dve_sq(Src0) - dve_sq(Src1) * C1,
    lambda in0, in1, s0, s1, imm2: (
        in0 * in0 - in1 * in1 * s1).astype(_f32))
# select(g < 0, imm2, raw)
OP_SELPOS = _reg_op(
    "ANT_SELPOS", dve_select(Src1 < C1, C2, Src0),
    lambda in0, in1, s0, s1, imm2: np.where(
        in1 < s1, imm2, in0).astype(_f32))


def _axpb4y_body():
    t = Src1 * C1
    u = t + t
    return Src0 * C0 + (u + u)


# d' = c2*d + 4*s2*pt
OP_AXPB4Y = _reg_op(
    "ANT_AXPB4Y", _axpb4y_body(),
    lambda in0, in1, s0, s1, imm2: (in0 * s0 + 4.0 * in1 * s1).astype(_f32))
# pt' = c2*pt - 0.25*s2*d   (imm2 carries the 0.25)
OP_AXMBYC = _reg_op(
    "ANT_AXMBYC", Src0 * C0 - (Src1 * C1) * C2,
    lambda in0, in1, s0, s1, imm2: (in0 * s0 - in1 * s1 * imm2).astype(_f32))

F32 = mybir.dt.float32
BF16 = mybir.dt.bfloat16
F8 = mybir.dt.float8e4
AF = mybir.ActivationFunctionType
ALU = mybir.AluOpType
AX = mybir.AxisListType
DR = mybir.MatmulPerfMode.DoubleRow

N_CORES = 8
D = 512
SAMPLES = 256          # b*v
TOK = 256              # tokens per sample
S_CORE = SAMPLES // N_CORES       # 32 samples per core
T_CORE = S_CORE * TOK             # 8192 token rows per core
T_TILE = 512
N_TILES = T_CORE // T_TILE        # 16
S_TILE = T_TILE // TOK            # 2 samples per token tile

T_SUP = 1024           # supertile: 2 token-tiles drained with wide ops
N_SUP = T_CORE // T_SUP           # 8
S_SUP = T_SUP // TOK              # 4 samples per supertile

N_ROT = 9              # lean Jacobi rotations (sim: rel_err 3.3e-3)

GAMMA = float(3.0 + 2.0 * np.sqrt(2.0))
CQ45 = float(np.cos(np.pi / 4))


# ---------------------------------------------------------------------------
# small-op emitter for the SVD tail: SSA-style column allocation on a scratch
# tile; every value is an AP (or list of APs).
# ---------------------------------------------------------------------------
class Emit:
    def __init__(self, nc, pool):
        self.nc = nc
        self.scr = pool.tile([32, 2048], F32, tag="svd_scratch",
                             name="svd_scratch")
        self.ptr = 0

    def new(self, n=1):
        c = self.ptr
        self.ptr += n
        assert self.ptr <= 2048, "svd scratch overflow"
        return self.scr[:, c:c + n]

    def tt(self, op, a, b, n=1):
        o = self.new(n)
        self.nc.vector.tensor_tensor(o, a, b, op)
        return o

    def tt3(self, op, a, b, n=9):
        o = self.new(n)
        self.nc.vector.tensor_tensor(
            o.rearrange("p (i j) -> p i j", i=3, j=n // 3), a, b, op)
        return o

    def ts(self, op, a, s, n=1):
        o = self.new(n)
        self.nc.vector.tensor_scalar(o, a, s, None, op)
        return o

    def stt(self, a, scal, b, op0, op1, n=1):
        o = self.new(n)
        self.nc.vector.scalar_tensor_tensor(o, a, scal, b, op0=op0, op1=op1)
        return o

    # --- gpsimd variant (tensor_tensor only; Pool supports no Ptr ops) ---
    def gtt(self, op, a, b, n=1):
        o = self.new(n)
        self.nc.gpsimd.tensor_tensor(o, a, b, op)
        return o

    def rsqrt(self, a, n=1):
        t = self.new(n)
        self.nc.scalar.activation(t, a, AF.Sqrt)
        o = self.new(n)
        self.nc.vector.reciprocal(o, t)
        return o

    def cdve(self, op, in0, in1, s0=0.0, s1=0.0, imm2=0.0, n=1, out=None):
        if out is None:
            out = self.new(n)
        self.nc.vector._custom_dve(op, out=out, in0=in0, in1=in1,
                                   s0=s0, s1=s1, imm2=imm2)
        return out

    def const(self, val, n=1):
        o = self.new(n)
        self.nc.vector.memset(o, val)
        return o


def _bcast_r(ap3):
    return ap3.unsqueeze(2).broadcast_to([32, 3, 3])


def _bcast_l(ap3):
    return ap3.unsqueeze(1).broadcast_to([32, 3, 3])


def emit_svd_so3(nc, em, m_ap, pose_tile):
    """m_ap: [32,9] raw 3x3 per sample (row-major). Writes the SO(3)
    projection into pose_tile columns (4r+c for r,c in 0..2).

    Lean d-tracking Jacobi: state is (d01,d12,d02) eigenvalue differences and
    (p01,p12,p02) halved off-diagonals; 15 DVE ops per rotation."""
    # --- row normalize ---
    sq = em.tt(ALU.mult, m_ap, m_ap, 9)
    t = em.tt(ALU.add, sq[:, 0:9:3], sq[:, 1:9:3], 3)
    r2 = em.tt(ALU.add, t, sq[:, 2:9:3], 3)
    r2c = em.ts(ALU.max, r2, 1e-24, 3)
    rinv = em.rsqrt(r2c, 3)
    A = em.tt3(ALU.mult, m_ap.rearrange("p (r c) -> p r c", r=3, c=3),
               _bcast_r(rinv), 9)

    # --- S = A^T A (s_ij at col 3i+j) ---
    terms = []
    for r in range(3):
        arow = A[:, 3 * r:3 * r + 3]
        terms.append(em.tt3(ALU.mult, _bcast_r(arow), _bcast_l(arow), 9))
    s01 = em.tt(ALU.add, terms[0], terms[1], 9)
    S9 = em.tt(ALU.add, s01, terms[2], 9)

    # d/pt state (SSA-tracked APs)
    d01 = em.tt(ALU.subtract, S9[:, 0:1], S9[:, 4:5])
    d12 = em.tt(ALU.subtract, S9[:, 4:5], S9[:, 8:9])
    d02 = em.tt(ALU.add, d01, d12)
    p01 = em.ts(ALU.mult, S9[:, 1:2], 0.5)
    p12 = em.ts(ALU.mult, S9[:, 5:6], 0.5)
    p02 = em.ts(ALU.mult, S9[:, 2:3], 0.5)

    # V columns as [32,3] blocks, init = identity
    Vc = []
    for j in range(3):
        vj = em.new(3)
        nc.vector.memset(vj, 0.0)
        nc.vector.memset(vj[:, j:j + 1], 1.0)
        Vc.append(vj)

    st = {'d01': d01, 'd12': d12, 'd02': d02,
          'p01': p01, 'p12': p12, 'p02': p02}

    def angle(ch, sh):
        ssum = em.cdve(OP_SQSUM, ch, sh)
        w2 = em.new(1)
        nc.vector.reciprocal(w2, ssum)
        craw = em.cdve(OP_CSUBW, ch, sh, s0=w2)
        sraw = em.cdve(OP_XYW2, ch, sh, s0=w2)
        g = em.cdve(OP_GATE, ch, sh, s1=GAMMA)
        c = em.cdve(OP_SELPOS, craw, g, s1=0.0, imm2=CQ45)
        s = em.cdve(OP_SELPOS, sraw, g, s1=0.0, imm2=CQ45)
        c2 = em.cdve(OP_SQDIFF, c, s)
        s2 = em.cdve(OP_XY2, c, s)
        return c, s, c2, s2

    def vup(p, q, c, s):
        # V rotation runs on GPSIMD, off the DVE critical chain (the V
        # columns are only consumed after the rotation loop).
        cb = c.broadcast_to([32, 3])
        sb = s.broadcast_to([32, 3])
        t1 = em.gtt(ALU.mult, Vc[q], sb, 3)
        t2 = em.gtt(ALU.mult, Vc[p], cb, 3)
        nvp = em.gtt(ALU.add, t2, t1, 3)
        t3 = em.gtt(ALU.mult, Vc[p], sb, 3)
        t4 = em.gtt(ALU.mult, Vc[q], cb, 3)
        nvq = em.gtt(ALU.subtract, t4, t3, 3)
        Vc[p], Vc[q] = nvp, nvq

    for k in range(N_ROT):
        rt = k % 3
        if rt == 0:     # (p,q,r) = (0,1,2)
            c, s, c2, s2 = angle(st['d01'], st['p01'])
            dd = em.cdve(OP_AXPB4Y, st['d01'], st['p01'], s0=c2, s1=s2)
            pn = em.cdve(OP_AXMBYC, st['p01'], st['d01'], s0=c2, s1=s2,
                         imm2=0.25)
            t_ = em.tt(ALU.add, st['d02'], st['d12'])
            nd02 = em.cdve(OP_AXPBY, t_, dd, s0=0.5, s1=0.5)
            nd12 = em.cdve(OP_AXMBY, t_, dd, s0=0.5, s1=0.5)
            np02 = em.cdve(OP_AXPBY, st['p02'], st['p12'], s0=c, s1=s)
            np12 = em.cdve(OP_AXMBY, st['p12'], st['p02'], s0=c, s1=s)
            st.update(d01=dd, p01=pn, d02=nd02, d12=nd12, p02=np02, p12=np12)
            vup(0, 1, c, s)
        elif rt == 1:   # (1,2,0)
            c, s, c2, s2 = angle(st['d12'], st['p12'])
            dd = em.cdve(OP_AXPB4Y, st['d12'], st['p12'], s0=c2, s1=s2)
            pn = em.cdve(OP_AXMBYC, st['p12'], st['d12'], s0=c2, s1=s2,
                         imm2=0.25)
            t_ = em.tt(ALU.add, st['d01'], st['d02'])
            nd01 = em.cdve(OP_AXMBY, t_, dd, s0=0.5, s1=0.5)
            nd02 = em.cdve(OP_AXPBY, t_, dd, s0=0.5, s1=0.5)
            np01 = em.cdve(OP_AXPBY, st['p01'], st['p02'], s0=c, s1=s)
            np02 = em.cdve(OP_AXMBY, st['p02'], st['p01'], s0=c, s1=s)
            st.update(d12=dd, p12=pn, d01=nd01, d02=nd02, p01=np01, p02=np02)
            vup(1, 2, c, s)
        else:           # (0,2,1)
            c, s, c2, s2 = angle(st['d02'], st['p02'])
            dd = em.cdve(OP_AXPB4Y, st['d02'], st['p02'], s0=c2, s1=s2)
            pn = em.cdve(OP_AXMBYC, st['p02'], st['d02'], s0=c2, s1=s2,
                         imm2=0.25)
            t_ = em.tt(ALU.subtract, st['d01'], st['d12'])
            nd01 = em.cdve(OP_AXPBY, t_, dd, s0=0.5, s1=0.5)
            nd12 = em.cdve(OP_AXMBY, dd, t_, s0=0.5, s1=0.5)
            np01 = em.cdve(OP_AXPBY, st['p01'], st['p12'], s0=c, s1=s)
            np12 = em.cdve(OP_AXMBY, st['p12'], st['p01'], s0=c, s1=s)
            st.update(d02=dd, p02=pn, d01=nd01, d12=nd12, p01=np01, p12=np12)
            vup(0, 2, c, s)

    # --- sort eigenpairs descending (det(V) stays +1 via column negation) ---
    def cond_swap(i, j):
        key_ij = f'd{i}{j}'
        k = 3 - i - j
        key_ik = f'd{min(i,k)}{max(i,k)}'
        key_jk = f'd{min(j,k)}{max(j,k)}'
        mask = em.ts(ALU.is_lt, st[key_ij], 0.0)
        nij = em.cdve(OP_WHERENEG, st[key_ij], st[key_ij], s0=mask)
        nik = em.cdve(OP_WHERE, st[key_jk], st[key_ik], s0=mask)
        njk = em.cdve(OP_WHERE, st[key_ik], st[key_jk], s0=mask)
        st[key_ij], st[key_ik], st[key_jk] = nij, nik, njk
        # V swap on GPSIMD: vi' = vi + m*(vj-vi); vj' = vj - m*(vi+vj)
        m3 = mask.broadcast_to([32, 3])
        dv = em.gtt(ALU.subtract, Vc[j], Vc[i], 3)
        md = em.gtt(ALU.mult, dv, m3, 3)
        vi = em.gtt(ALU.add, Vc[i], md, 3)
        sv = em.gtt(ALU.add, Vc[i], Vc[j], 3)
        ms = em.gtt(ALU.mult, sv, m3, 3)
        vj = em.gtt(ALU.subtract, Vc[j], ms, 3)
        Vc[i], Vc[j] = vi, vj

    cond_swap(0, 1)
    cond_swap(1, 2)
    cond_swap(0, 1)

    # --- B columns (j=0,1): b_j[r] = sum_c A[r][c] * V[c][j] ---
    Astr = [A[:, c:c + 7:3] for c in range(3)]

    def bcol(j):
        t0 = em.cdve(OP_AXPBY, Astr[0], Astr[1],
                     s0=Vc[j][:, 0:1], s1=Vc[j][:, 1:2], n=3)
        return em.stt(Astr[2], Vc[j][:, 2:3], t0, ALU.mult, ALU.add, 3)

    def bcol_gps(j):
        t0 = em.gtt(ALU.mult, Astr[0],
                    Vc[j][:, 0:1].broadcast_to([32, 3]), 3)
        t1 = em.gtt(ALU.mult, Astr[1],
                    Vc[j][:, 1:2].broadcast_to([32, 3]), 3)
        t01 = em.gtt(ALU.add, t0, t1, 3)
        t2 = em.gtt(ALU.mult, Astr[2],
                    Vc[j][:, 2:3].broadcast_to([32, 3]), 3)
        return em.gtt(ALU.add, t01, t2, 3)

    b0 = bcol(0)
    b1 = bcol_gps(1)

    def normalize(v3):
        sqv = em.tt(ALU.mult, v3, v3, 3)
        n_ = em.tt(ALU.add, sqv[:, 0:1], sqv[:, 1:2])
        n_ = em.tt(ALU.add, n_, sqv[:, 2:3])
        nc_ = em.ts(ALU.max, n_, 1e-30)
        inv = em.rsqrt(nc_)
        return em.ts(ALU.mult, v3, inv, 3)

    u1 = normalize(b0)
    p_ = em.tt(ALU.mult, u1, b1, 3)
    d_ = em.tt(ALU.add, p_[:, 0:1], p_[:, 1:2])
    d_ = em.tt(ALU.add, d_, p_[:, 2:3])
    dneg = em.ts(ALU.mult, d_, -1.0)
    b2o = em.stt(u1, dneg, b1, ALU.mult, ALU.add, 3)
    u2 = normalize(b2o)
    u3 = em.new(3)
    for k, (i1, i2) in enumerate(((1, 2), (2, 0), (0, 1))):
        em.cdve(OP_AXMBY, u1[:, i1:i1 + 1], u1[:, i2:i2 + 1],
                s0=u2[:, i2:i2 + 1], s1=u2[:, i1:i1 + 1],
                out=u3[:, k:k + 1])

    # --- R = u1 v1^T + u2 v2^T + u3 v3^T ---
    t0 = em.tt3(ALU.mult, _bcast_r(u1), _bcast_l(Vc[0]), 9)
    t1 = em.tt3(ALU.mult, _bcast_r(u2), _bcast_l(Vc[1]), 9)
    t01 = em.tt(ALU.add, t0, t1, 9)
    t2 = em.tt3(ALU.mult, _bcast_r(u3), _bcast_l(Vc[2]), 9)
    pose_R = pose_tile[:].rearrange("p (r c) -> p r c", r=4, c=4)[:, 0:3, 0:3]
    nc.vector.tensor_tensor(
        pose_R, t01.rearrange("p (r c) -> p r c", r=3, c=3),
        t2.rearrange("p (r c) -> p r c", r=3, c=3), ALU.add)


# ---------------------------------------------------------------------------
# kernel build
# ---------------------------------------------------------------------------
def build_nc():
    nc = bacc.Bacc("TRN2", target_bir_lowering=False)

    xT8 = nc.dram_tensor("xT8", [D, T_CORE], F8, kind="ExternalInput")
    w8 = nc.dram_tensor("w8", [6, 128, 2048], F8, kind="ExternalInput")
    bs = nc.dram_tensor("bs", [6, D], F32, kind="ExternalInput")
    x0s = nc.dram_tensor("x0s", [128, 4 * S_CORE], F32, kind="ExternalInput")
    mwt = nc.dram_tensor("mwt", [2, D, D], BF16, kind="ExternalInput")
    mbs = nc.dram_tensor("mbs", [2, D], F32, kind="ExternalInput")
    hwT = nc.dram_tensor("hwT", [D, 12], BF16, kind="ExternalInput")
    hb = nc.dram_tensor("hb", [S_CORE, 12], F32, kind="ExternalInput")
    pose = nc.dram_tensor("pose", [S_CORE, 16], F32, kind="ExternalOutput")

    with tile.TileContext(nc) as tc:
        with (
            tc.tile_pool(name="wp", bufs=1) as wpool,
            tc.tile_pool(name="xp", bufs=3) as xpool,
            tc.tile_pool(name="hp", bufs=2) as hpool,
            tc.tile_pool(name="h3p", bufs=2) as h3pool,
            tc.tile_pool(name="pp", bufs=1) as ppool,
            tc.tile_pool(name="ps", bufs=4, space="PSUM") as pspool,
            tc.tile_pool(name="sm", bufs=1) as smpool,
        ):
            # warm the ACT function table while DMAs stream
            warm = smpool.tile([32, 1], F32, tag="warm", name="warm")
            nc.vector.memset(warm[:], 0.0)
            nc.scalar.activation(warm[:], warm[:], AF.Relu)

            # ---- first x supertile + layer-0 weights first, chunked across
            # three DMA queues so the PE can start right after the preamble
            xt0 = xpool.tile([128, 4 * T_SUP], F8, tag="xt", name="xt")
            w_sb = [wpool.tile([128, 2048], F8, tag=f"w{l}", name=f"w{l}")
                    for l in range(6)]
            b_sb = wpool.tile([128, 24], F32, tag="b", name="b_sb")
            QS = (nc.sync, nc.scalar, nc.gpsimd)
            for c in range(8):  # w0 in (o,kp)-chunk order across queues
                QS[c % 3].dma_start(w_sb[0][:, 256 * c:256 * (c + 1)],
                                    w8[0, :, 256 * c:256 * (c + 1)])
            for k, eng in enumerate((nc.gpsimd, nc.scalar, nc.sync,
                                     nc.gpsimd)):
                eng.dma_start(xt0[:, T_SUP * k:T_SUP * (k + 1)],
                              xT8[128 * k:128 * (k + 1), 0:T_SUP])
            # all 24 bias columns in one strided DMA
            nc.scalar.dma_start(
                b_sb[:].rearrange("p (l o) -> p l o", l=6, o=4),
                bs[:].rearrange("l (o p) -> p l o", p=128, o=4))
            for l in range(1, 6):
                QS[l % 3].dma_start(w_sb[l][:], w8[l])
            x0s_sb = wpool.tile([128, 4 * S_CORE], F32, tag="x0s",
                                name="x0s_sb")
            nc.sync.dma_start(x0s_sb[:], x0s[:])
            mw_sb = [wpool.tile([128, 2048], BF16, tag=f"mw{l}",
                                name=f"mw{l}") for l in range(2)]
            for l in range(2):
                for k in range(4):
                    nc.sync.dma_start(
                        mw_sb[l][:, D * k:D * (k + 1)],
                        mwt[l, 128 * k:128 * (k + 1), :])
            mb_sb = wpool.tile([128, 8], F32, tag="mb", name="mb_sb")
            for l in range(2):
                nc.sync.dma_start(mb_sb[:, 4 * l:4 * l + 4],
                                  mbs[l].rearrange("(o p) -> p o", p=128, o=4))
            hw_sb = wpool.tile([128, 48], BF16, tag="hw", name="hw_sb")
            for k in range(4):
                nc.sync.dma_start(hw_sb[:, 12 * k:12 * (k + 1)],
                                  hwT[128 * k:128 * (k + 1), :])
            hb_sb = wpool.tile([32, 12], F32, tag="hbt", name="hb_sb")
            nc.sync.dma_start(hb_sb[:], hb[:])

            # per-tile h3 sums for the two blocks: [128, 4k x 32 samples]
            pb1 = ppool.tile([128, 4 * S_CORE], F32, tag="pb1", name="pb1")
            pb2 = ppool.tile([128, 4 * S_CORE], F32, tag="pb2", name="pb2")

            def wap(l, o, kp):
                c0 = (o * 2 + kp) * 256
                return w_sb[l][:, c0:c0 + 256].rearrange(
                    "p (i m) -> p i m", i=2)

            def rhs(t, kp, th):
                # kp-pair chunks of a [128, 4*T_SUP] supertile, token half th
                return t[:, 2 * T_SUP * kp:2 * T_SUP * (kp + 1)].rearrange(
                    "p (i n) -> p i n", i=2)[:, :, 512 * th:512 * (th + 1)]

            def relu_drain(engine, h_slice, ps, bias_ap):
                if engine == 'act':
                    nc.scalar.activation(h_slice, ps[:], AF.Relu,
                                         bias=bias_ap, scale=1.0)
                else:
                    nc.vector.tensor_scalar(h_slice, ps[:], bias_ap, 0.0,
                                            ALU.add, ALU.max)

            # engine assignment per layer (ACT 16 / DVE 8 drains per sup;
            # DVE also owns the two pooling reduces)
            ENG = {
                0: ['act'] * 4,
                1: ['dve'] * 4,
                2: ['act'] * 4,
                3: ['act'] * 4,
                4: ['act'] * 4,
                5: ['dve'] * 4,
            }

            # ---- main loop over supertiles (1024 tokens each) ----
            pending_red = None
            for ti in range(N_SUP):
                if ti == 0:
                    xt = xt0
                else:
                    xt = xpool.tile([128, 4 * T_SUP], F8, tag="xt",
                                    name="xt")
                    for k in range(4):
                        nc.gpsimd.dma_start(
                            xt[:, T_SUP * k:T_SUP * (k + 1)],
                            xT8[128 * k:128 * (k + 1),
                                T_SUP * ti:T_SUP * (ti + 1)])

                def emit_reduce(h3t, pb, tj, o=None):
                    """Per-supertile h3 pooling sum; o=None reduces all 4
                    chunks in one 4D op, else a single o-chunk."""
                    if o is None:
                        nc.vector.tensor_reduce(
                            pb[:].rearrange("p (o s) -> p o s", o=4,
                                            s=S_CORE)[:, :, S_SUP * tj:
                                                      S_SUP * (tj + 1)],
                            h3t[:].rearrange("p (o g t) -> p o g t", o=4,
                                             g=S_SUP),
                            axis=AX.X, op=ALU.add)
                    else:
                        nc.vector.tensor_reduce(
                            pb[:, S_CORE * o + S_SUP * tj:
                               S_CORE * o + S_SUP * (tj + 1)],
                            h3t[:, T_SUP * o:T_SUP * (o + 1)].rearrange(
                                "p (g t) -> p g t", g=S_SUP),
                            axis=AX.X, op=ALU.add)

                def run_layer(l, src, out_dtype=F8, tag="h", extra_src=None,
                              red_to=None):
                    pool_ = h3pool if l in (2, 5) else hpool
                    h_out = pool_.tile([128, 4 * T_SUP], out_dtype,
                                       tag=tag, name=f"h{l}")
                    for o in range(4):
                        ps = pspool.tile([128, T_SUP], F32, tag="ps",
                                         name="ps")
                        srcs = [src] if extra_src is None else [src,
                                                                extra_src]
                        n_mm = 4 * len(srcs)
                        mi = 0
                        for s_ in srcs:
                            for kp in range(2):
                                for th in range(2):
                                    nc.tensor.matmul(
                                        ps[:, 512 * th:512 * (th + 1)],
                                        wap(l, o, kp), rhs(s_, kp, th),
                                        start=(mi < 2),
                                        stop=(mi >= n_mm - 2),
                                        perf_mode=DR)
                                    mi += 1
                        relu_drain(ENG[l][o],
                                   h_out[:, T_SUP * o:T_SUP * (o + 1)],
                                   ps, b_sb[:, 4 * l + o:4 * l + o + 1])
                        if red_to is not None:
                            emit_reduce(h_out, red_to, ti, o=o)
                    return h_out

                h1 = run_layer(0, xt)
                h2 = run_layer(1, h1)
                if pending_red is not None:
                    # deferred h3b reduce of the previous supertile: lands
                    # behind this supertile's l1 drains in the DVE queue so
                    # the PE never waits on it
                    emit_reduce(pending_red[0], pb2, pending_red[1])
                    pending_red = None
                h3a = run_layer(2, h2, tag="h3a")          # fp8: matmul input
                emit_reduce(h3a, pb1, ti)
                g1 = run_layer(3, xt, extra_src=h3a)        # fused residual
                g2 = run_layer(4, g1)
                last = ti == N_SUP - 1
                h3b = run_layer(5, g2, out_dtype=BF16, tag="h3b",
                                red_to=pb2 if last else None)
                if not last:
                    pending_red = (h3b, ti)

            # ---- pooled = x0s + pb1 + pb2, per k-chunk so the first tail
            # matmuls start before the whole combine finishes ----
            pool_f32 = smpool.tile([128, 4 * S_CORE], F32, tag="poolf",
                                   name="pool_f32")
            pool_bf = smpool.tile([128, 4 * S_CORE], BF16, tag="poolb",
                                  name="pool_bf")
            for k in range(4):
                sl = slice(S_CORE * k, S_CORE * (k + 1))
                nc.vector.tensor_add(pool_f32[:, sl], pb1[:, sl], pb2[:, sl])
                nc.vector.tensor_add(pool_bf[:, sl], pool_f32[:, sl],
                                     x0s_sb[:, sl])

            # ---- tail MLPs (bf16), psum reused from the main pool ----
            f_prev = pool_bf
            scales = [1.0 / TOK, 1.0]
            for l in range(2):
                f_out = smpool.tile([128, 4 * S_CORE], BF16, tag=f"f{l}",
                                    name=f"f{l}")
                for o in range(4):
                    ps_w = pspool.tile([128, T_SUP], F32, tag="ps",
                                       name="pst")
                    ps = ps_w[:, 0:S_CORE]
                    for k in range(4):
                        nc.tensor.matmul(
                            ps,
                            mw_sb[l][:, D * k + 128 * o:D * k + 128 * (o + 1)],
                            f_prev[:, S_CORE * k:S_CORE * (k + 1)],
                            start=(k == 0), stop=(k == 3))
                    nc.scalar.activation(
                        f_out[:, S_CORE * o:S_CORE * (o + 1)], ps, AF.Relu,
                        bias=mb_sb[:, 4 * l + o:4 * l + o + 1],
                        scale=scales[l])
                f_prev = f_out

            # ---- heads: [32 samples, 12] = t(3) ++ rot(9) ----
            psh_w = pspool.tile([128, T_SUP], F32, tag="ps", name="psh")
            psh = psh_w[0:32, 0:12]
            for k in range(4):
                nc.tensor.matmul(psh,
                                 f_prev[:, S_CORE * k:S_CORE * (k + 1)],
                                 hw_sb[:, 12 * k:12 * (k + 1)],
                                 start=(k == 0), stop=(k == 3))
            mm = smpool.tile([32, 12], F32, tag="mm", name="mm")
            nc.vector.tensor_add(mm[:], psh, hb_sb[:])

            # ---- pose assembly + SVD ----
            pose_t = smpool.tile([32, 16], F32, tag="pose", name="pose_t")
            nc.vector.memset(pose_t[:], 0.0)
            nc.vector.memset(pose_t[:, 15:16], 1.0)
            nc.vector.tensor_copy(
                pose_t[:].rearrange("p (r c) -> p r c", r=4, c=4)[:, 0:3, 3],
                mm[:, 0:3])

            em = Emit(nc, smpool)
            emit_svd_so3(nc, em, mm[:, 3:12], pose_t)

            nc.sync.dma_start(pose[:], pose_t[:])

    nc.compile()
    return nc


_NC_CACHE = None


def _get_nc():
    global _NC_CACHE
    if _NC_CACHE is None:
        _NC_CACHE = build_nc()
    return _NC_CACHE


F8NP = ml_dtypes.float8_e4m3fn
BF16NP = ml_dtypes.bfloat16


def kernel(**inputs):
    feat = np.asarray(inputs["feat"], dtype=np.float32)
    b_, v_, n_, d_ = feat.shape
    xs = feat.reshape(b_ * v_, n_, d_)
    x0sum = xs.sum(axis=1, dtype=np.float32)          # (256, 512)

    # DoubleRow weight prepack: [p, o, kp, i, m] <- wT[128*(2kp+i)+p, 128o+m]
    w8_list = []
    for blk in (1, 2):
        for li in (1, 2, 3):
            wT = np.asarray(inputs[f"r{blk}_w{li}"], np.float32).T
            arr = wT.astype(F8NP).reshape(2, 2, 128, 4, 128)
            arr = np.ascontiguousarray(arr.transpose(2, 3, 0, 1, 4))
            w8_list.append(arr.reshape(128, 2048))
    w8 = np.stack(w8_list)
    bs = np.stack([np.asarray(inputs[f"r{blk}_b{li}"], np.float32)
                   for blk in (1, 2) for li in (1, 2, 3)])
    mwt = np.stack([np.ascontiguousarray(
        np.asarray(inputs[f"m_w{li}"], np.float32).T).astype(BF16NP)
        for li in (1, 2)])
    mbs = np.stack([np.asarray(inputs[f"m_b{li}"], np.float32)
                    for li in (1, 2)])
    hwT = np.ascontiguousarray(np.concatenate(
        [np.asarray(inputs["t_w"], np.float32).T,
         np.asarray(inputs["rot_w"], np.float32).T], axis=1)).astype(BF16NP)
    hb = np.broadcast_to(np.concatenate(
        [np.asarray(inputs["t_b"], np.float32),
         np.asarray(inputs["rot_b"], np.float32)])[None, :],
        (S_CORE, 12)).copy()

    in_maps = []
    for c in range(N_CORES):
        xT8 = np.ascontiguousarray(
            xs[c * S_CORE:(c + 1) * S_CORE].reshape(T_CORE, D).T).astype(F8NP)
        xs_c = x0sum[c * S_CORE:(c + 1) * S_CORE]     # (32, 512)
        x0s = np.ascontiguousarray(
            xs_c.T.reshape(4, 128, S_CORE).transpose(1, 0, 2).reshape(
                128, 4 * S_CORE))
        in_maps.append({
            "xT8": xT8, "w8": w8, "bs": bs, "x0s": x0s, "mwt": mwt,
            "mbs": mbs, "hwT": hwT, "hb": hb,
        })

    nc = _get_nc()
    import os
    kwargs = {}
    if os.environ.get("KERNEL_TRACE") == "1":
        kwargs["trace"] = True
    res = run_bass_kernel_spmd(nc, in_maps, core_ids=list(range(N_CORES)),
                               **kwargs)
    if kwargs.get("trace"):
        kernel.last_results = res
    poses = np.concatenate([r["pose"] for r in res.results], axis=0)
    return poses.reshape(b_, v_, 4, 4)


# revision 28
# speedup vs baseline: 1.6024x; 1.0836x over previous
"""CameraHead Trainium2 kernel — data-parallel over b*v across 8 NeuronCores.

fp8(e4m3) DoubleRow edition. Per-core layout: activations feature-major in
SBUF (X^T: 4x128-part chunks x tokens, fp8), all six 512x512 Linears run as
DoubleRow fp8 matmuls (K=256 per pass, 512-token moving operand, 2 passes for
K=512) at 2x the bf16/fp32r PE rate. Relu+bias drains PSUM on three engines
(ACT / DVE / GPSIMD) in parallel. The block-2 first Linear consumes x0 and
h3(block1) as two accumulated matmul streams, so the residual add never
materializes. Pooling = host-exact sum(x0) + on-device tensor_reduce over the
two h3 streams. Tail: bf16 MLP/head matmuls + a lean d-tracking Jacobi
(9 rotations) for the SVD->SO(3) projection, then pose assembly.
"""
import sys
import numpy as np

sys.path.insert(0, '/opt/trn_rl_repo')

import ml_dtypes  # noqa: E402

import concourse.bacc as bacc  # noqa: E402
import concourse.mybir as mybir  # noqa: E402
from concourse import tile  # noqa: E402
from concourse import dve_ops as _dvo  # noqa: E402
from concourse.bass_utils import run_bass_kernel_spmd  # noqa: E402
from concourse.dve_spec import (  # noqa: E402
    C0, C1, C2, One, Spec, Src0, Src1, select as dve_select, sq as dve_sq,
)


def _reg_op(name, body, ref):
    """Register a custom DVE op (per-NEFF uop table; no firmware change).

    The uops sha pin is bootstrapped by parsing compile()'s drift error."""
    for op in _dvo.OPS:
        if op.name == name:
            return op
    import re as _re

    from concourse.dve_table_gen import dve_ver_for

    row = _dvo._CUSTOM_DVE_ROW_BASE + len(_dvo.OPS)
    assert row < 0x20, "custom DVE opcode rows exhausted"
    spec = Spec(body=body, reference=ref)
    op = _dvo.DveOp(name, spec, subdim=False, uops_sha={})
    _dvo.OPS.append(op)
    _dvo._SUB_OPCODE_FOR_NAME[name] = row
    _dvo.CUSTOM_DVE_SPECS[name] = spec
    ver = dve_ver_for("TRN2")
    try:
        op.compile(ver)
    except ValueError as e:
        m = _re.search(r'uops_sha\["' + ver + r'"\]="([0-9a-f]+)"', str(e))
        if not m:
            raise
        op.uops_sha[ver] = m.group(1)
        op.compile(ver)
    return op


_f32 = np.float32
OP_AXPBY = _reg_op(
    "ANT_AXPBY", Src0 * C0 + Src1 * C1,
    lambda in0, in1, s0, s1, imm2: (in0 * s0 + in1 * s1).astype(_f32))
OP_AXMBY = _reg_op(
    "ANT_AXMBY", Src0 * C0 - Src1 * C1,
    lambda in0, in1, s0, s1, imm2: (in0 * s0 - in1 * s1).astype(_f32))
OP_AXPBY2 = _reg_op(
    "ANT_AXPBY2", (Src0 * C0 + Src1 * C1) * C2,
    lambda in0, in1, s0, s1, imm2: ((in0 * s0 + in1 * s1) * imm2).astype(_f32))
OP_AXMBY2 = _reg_op(
    "ANT_AXMBY2", (Src0 * C0 - Src1 * C1) * C2,
    lambda in0, in1, s0, s1, imm2: ((in0 * s0 - in1 * s1) * imm2).astype(_f32))
OP_SQDIFF = _reg_op(
    "ANT_SQDIFF", dve_sq(Src0) - dve_sq(Src1),
    lambda in0, in1, s0, s1, imm2: (in0 * in0 - in1 * in1).astype(_f32))


def _xy2_body():
    t = Src0 * Src1
    return t + t


OP_XY2 = _reg_op(
    "ANT_XY2", _xy2_body(),
    lambda in0, in1, s0, s1, imm2: (2.0 * in0 * in1).astype(_f32))
OP_WHERE = _reg_op(
    "ANT_WHERE", dve_select(C0, Src0, Src1),
    lambda in0, in1, s0, s1, imm2: np.where(
        s0 != 0, in0, in1).astype(_f32))
OP_WHERENEG = _reg_op(
    "ANT_WHERENEG", dve_select(C0, -Src0, Src1),
    lambda in0, in1, s0, s1, imm2: np.where(
        s0 != 0, -in0, in1).astype(_f32))
OP_SQSUM = _reg_op(
    "ANT_SQSUM", dve_sq(Src0) + dve_sq(Src1),
    lambda in0, in1, s0, s1, imm2: (in0 * in0 + in1 * in1).astype(_f32))
# (ch^2 - sh^2) * w2  — raw cosine
OP_CSUBW = _reg_op(
    "ANT_CSUBW", (dve_sq(Src0) - dve_sq(Src1)) * C0,
    lambda in0, in1, s0, s1, imm2: (
        (in0 * in0 - in1 * in1) * s0).astype(_f32))


def _xyw_body():
    t = Src0 * Src1
    return (t + t) * C0


# 2 * ch * sh * w2 — raw sine
OP_XYW2 = _reg_op(
    "ANT_XYW2", _xyw_body(),
    lambda in0, in1, s0, s1, imm2: (2.0 * in0 * in1 * s0).astype(_f32))
# gate value: ch^2 - gamma*sh^2 (>0 -> use raw angle, else pi/4 fallback)
OP_GATE = _reg_op(
    "ANT_GATE", dve_sq(Src0) - dve_sq(Src1) * C1,
    lambda in0, in1, s0, s1, imm2: (
        in0 * in0 - in1 * in1 * s1).astype(_f32))
# select(g < 0, imm2, raw)
OP_SELPOS = _reg_op(
    "ANT_SELPOS", dve_select(Src1 < C1, C2, Src0),
    lambda in0, in1, s0, s1, imm2: np.where(
        in1 < s1, imm2, in0).astype(_f32))


def _axpb4y_body():
    t = Src1 * C1
    u = t + t
    return Src0 * C0 + (u + u)


# d' = c2*d + 4*s2*pt
OP_AXPB4Y = _reg_op(
    "ANT_AXPB4Y", _axpb4y_body(),
    lambda in0, in1, s0, s1, imm2: (in0 * s0 + 4.0 * in1 * s1).astype(_f32))
# pt' = c2*pt - 0.25*s2*d   (imm2 carries the 0.25)
OP_AXMBYC = _reg_op(
    "ANT_AXMBYC", Src0 * C0 - (Src1 * C1) * C2,
    lambda in0, in1, s0, s1, imm2: (in0 * s0 - in1 * s1 * imm2).astype(_f32))

F32 = mybir.dt.float32
BF16 = mybir.dt.bfloat16
F8 = mybir.dt.float8e4
AF = mybir.ActivationFunctionType
ALU = mybir.AluOpType
AX = mybir.AxisListType
DR = mybir.MatmulPerfMode.DoubleRow

N_CORES = 8
D = 512
SAMPLES = 256          # b*v
TOK = 256              # tokens per sample
S_CORE = SAMPLES // N_CORES       # 32 samples per core
T_CORE = S_CORE * TOK             # 8192 token rows per core
T_TILE = 512
N_TILES = T_CORE // T_TILE        # 16
S_TILE = T_TILE // TOK            # 2 samples per token tile

T_SUP = 1024           # supertile: 2 token-tiles drained with wide ops
N_SUP = T_CORE // T_SUP           # 8
S_SUP = T_SUP // TOK              # 4 samples per supertile

N_ROT = 9              # lean Jacobi rotations (sim: rel_err 3.3e-3)

GAMMA = float(3.0 + 2.0 * np.sqrt(2.0))
CQ45 = float(np.cos(np.pi / 4))


# ---------------------------------------------------------------------------
# small-op emitter for the SVD tail: SSA-style column allocation on a scratch
# tile; every value is an AP (or list of APs).
# ---------------------------------------------------------------------------
class Emit:
    def __init__(self, nc, pool):
        self.nc = nc
        self.scr = pool.tile([32, 2048], F32, tag="svd_scratch",
                             name="svd_scratch")
        self.ptr = 0

    def new(self, n=1):
        c = self.ptr
        self.ptr += n
        assert self.ptr <= 2048, "svd scratch overflow"
        return self.scr[:, c:c + n]

    def tt(self, op, a, b, n=1):
        o = self.new(n)
        self.nc.vector.tensor_tensor(o, a, b, op)
        return o

    def tt3(self, op, a, b, n=9):
        o = self.new(n)
        self.nc.vector.tensor_tensor(
            o.rearrange("p (i j) -> p i j", i=3, j=n // 3), a, b, op)
        return o

    def ts(self, op, a, s, n=1):
        o = self.new(n)
        self.nc.vector.tensor_scalar(o, a, s, None, op)
        return o

    def stt(self, a, scal, b, op0, op1, n=1):
        o = self.new(n)
        self.nc.vector.scalar_tensor_tensor(o, a, scal, b, op0=op0, op1=op1)
        return o

    # --- gpsimd variant (tensor_tensor only; Pool supports no Ptr ops) ---
    def gtt(self, op, a, b, n=1):
        o = self.new(n)
        self.nc.gpsimd.tensor_tensor(o, a, b, op)
        return o

    def rsqrt(self, a, n=1):
        t = self.new(n)
        self.nc.scalar.activation(t, a, AF.Sqrt)
        o = self.new(n)
        self.nc.vector.reciprocal(o, t)
        return o

    def cdve(self, op, in0, in1, s0=0.0, s1=0.0, imm2=0.0, n=1, out=None):
        if out is None:
            out = self.new(n)
        self.nc.vector._custom_dve(op, out=out, in0=in0, in1=in1,
                                   s0=s0, s1=s1, imm2=imm2)
        return out

    def const(self, val, n=1):
        o = self.new(n)
        self.nc.vector.memset(o, val)
        return o


def _bcast_r(ap3):
    return ap3.unsqueeze(2).broadcast_to([32, 3, 3])


def _bcast_l(ap3):
    return ap3.unsqueeze(1).broadcast_to([32, 3, 3])


def emit_svd_so3(nc, em, m_ap, pose_tile):
    """m_ap: [32,9] raw 3x3 per sample (row-major). Writes the SO(3)
    projection into pose_tile columns (4r+c for r,c in 0..2).

    Lean d-tracking Jacobi: state is (d01,d12,d02) eigenvalue differences and
    (p01,p12,p02) halved off-diagonals; 15 DVE ops per rotation."""
    # --- row normalize ---
    sq = em.tt(ALU.mult, m_ap, m_ap, 9)
    t = em.tt(ALU.add, sq[:, 0:9:3], sq[:, 1:9:3], 3)
    r2 = em.tt(ALU.add, t, sq[:, 2:9:3], 3)
    r2c = em.ts(ALU.max, r2, 1e-24, 3)
    rinv = em.rsqrt(r2c, 3)
    A = em.tt3(ALU.mult, m_ap.rearrange("p (r c) -> p r c", r=3, c=3),
               _bcast_r(rinv), 9)

    # --- S = A^T A (s_ij at col 3i+j) ---
    terms = []
    for r in range(3):
        arow = A[:, 3 * r:3 * r + 3]
        terms.append(em.tt3(ALU.mult, _bcast_r(arow), _bcast_l(arow), 9))
    s01 = em.tt(ALU.add, terms[0], terms[1], 9)
    S9 = em.tt(ALU.add, s01, terms[2], 9)

    # d/pt state (SSA-tracked APs)
    d01 = em.tt(ALU.subtract, S9[:, 0:1], S9[:, 4:5])
    d12 = em.tt(ALU.subtract, S9[:, 4:5], S9[:, 8:9])
    d02 = em.tt(ALU.add, d01, d12)
    p01 = em.ts(ALU.mult, S9[:, 1:2], 0.5)
    p12 = em.ts(ALU.mult, S9[:, 5:6], 0.5)
    p02 = em.ts(ALU.mult, S9[:, 2:3], 0.5)

    # V columns as [32,3] blocks, init = identity
    Vc = []
    for j in range(3):
        vj = em.new(3)
        nc.vector.memset(vj, 0.0)
        nc.vector.memset(vj[:, j:j + 1], 1.0)
        Vc.append(vj)

    st = {'d01': d01, 'd12': d12, 'd02': d02,
          'p01': p01, 'p12': p12, 'p02': p02}

    def angle(ch, sh):
        ssum = em.cdve(OP_SQSUM, ch, sh)
        w2 = em.new(1)
        nc.vector.reciprocal(w2, ssum)
        craw = em.cdve(OP_CSUBW, ch, sh, s0=w2)
        sraw = em.cdve(OP_XYW2, ch, sh, s0=w2)
        g = em.cdve(OP_GATE, ch, sh, s1=GAMMA)
        c = em.cdve(OP_SELPOS, craw, g, s1=0.0, imm2=CQ45)
        s = em.cdve(OP_SELPOS, sraw, g, s1=0.0, imm2=CQ45)
        c2 = em.cdve(OP_SQDIFF, c, s)
        s2 = em.cdve(OP_XY2, c, s)
        return c, s, c2, s2

    def vup(p, q, c, s):
        # V rotation runs on GPSIMD, off the DVE critical chain (the V
        # columns are only consumed after the rotation loop).
        cb = c.broadcast_to([32, 3])
        sb = s.broadcast_to([32, 3])
        t1 = em.gtt(ALU.mult, Vc[q], sb, 3)
        t2 = em.gtt(ALU.mult, Vc[p], cb, 3)
        nvp = em.gtt(ALU.add, t2, t1, 3)
        t3 = em.gtt(ALU.mult, Vc[p], sb, 3)
        t4 = em.gtt(ALU.mult, Vc[q], cb, 3)
        nvq = em.gtt(ALU.subtract, t4, t3, 3)
        Vc[p], Vc[q] = nvp, nvq

    for k in range(N_ROT):
        rt = k % 3
        if rt == 0:     # (p,q,r) = (0,1,2)
            c, s, c2, s2 = angle(st['d01'], st['p01'])
            dd = em.cdve(OP_AXPB4Y, st['d01'], st['p01'], s0=c2, s1=s2)
            pn = em.cdve(OP_AXMBYC, st['p01'], st['d01'], s0=c2, s1=s2,
                         imm2=0.25)
            t_ = em.tt(ALU.add, st['d02'], st['d12'])
            nd02 = em.cdve(OP_AXPBY, t_, dd, s0=0.5, s1=0.5)
            nd12 = em.cdve(OP_AXMBY, t_, dd, s0=0.5, s1=0.5)
            np02 = em.cdve(OP_AXPBY, st['p02'], st['p12'], s0=c, s1=s)
            np12 = em.cdve(OP_AXMBY, st['p12'], st['p02'], s0=c, s1=s)
            st.update(d01=dd, p01=pn, d02=nd02, d12=nd12, p02=np02, p12=np12)
            vup(0, 1, c, s)
        elif rt == 1:   # (1,2,0)
            c, s, c2, s2 = angle(st['d12'], st['p12'])
            dd = em.cdve(OP_AXPB4Y, st['d12'], st['p12'], s0=c2, s1=s2)
            pn = em.cdve(OP_AXMBYC, st['p12'], st['d12'], s0=c2, s1=s2,
                         imm2=0.25)
            t_ = em.tt(ALU.add, st['d01'], st['d02'])
            nd01 = em.cdve(OP_AXMBY, t_, dd, s0=0.5, s1=0.5)
            nd02 = em.cdve(OP_AXPBY, t_, dd, s0=0.5, s1=0.5)
            np01 = em.cdve(OP_AXPBY, st['p01'], st['p02'], s0=c, s1=s)
            np02 = em.cdve(OP_AXMBY, st['p02'], st['p01'], s0=c, s1=s)
            st.update(d12=dd, p12=pn, d01=nd01, d02=nd02, p01=np01, p02=np02)
            vup(1, 2, c, s)
        else:           # (0,2,1)
            c, s, c2, s2 = angle(st['d02'], st['p02'])
            dd = em.cdve(OP_AXPB4Y, st['d02'], st['p02'], s0=c2, s1=s2)
            pn = em.cdve(OP_AXMBYC, st['p02'], st['d02'], s0=c2, s1=s2,
                         imm2=0.25)
            t_ = em.tt(ALU.subtract, st['d01'], st['d12'])
            nd01 = em.cdve(OP_AXPBY, t_, dd, s0=0.5, s1=0.5)
            nd12 = em.cdve(OP_AXMBY, dd, t_, s0=0.5, s1=0.5)
            np01 = em.cdve(OP_AXPBY, st['p01'], st['p12'], s0=c, s1=s)
            np12 = em.cdve(OP_AXMBY, st['p12'], st['p01'], s0=c, s1=s)
            st.update(d02=dd, p02=pn, d01=nd01, d12=nd12, p01=np01, p12=np12)
            vup(0, 2, c, s)

    # --- sort eigenpairs descending (det(V) stays +1 via column negation) ---
    def cond_swap(i, j):
        key_ij = f'd{i}{j}'
        k = 3 - i - j
        key_ik = f'd{min(i,k)}{max(i,k)}'
        key_jk = f'd{min(j,k)}{max(j,k)}'
        mask = em.ts(ALU.is_lt, st[key_ij], 0.0)
        nij = em.cdve(OP_WHERENEG, st[key_ij], st[key_ij], s0=mask)
        nik = em.cdve(OP_WHERE, st[key_jk], st[key_ik], s0=mask)
        njk = em.cdve(OP_WHERE, st[key_ik], st[key_jk], s0=mask)
        st[key_ij], st[key_ik], st[key_jk] = nij, nik, njk
        # V swap on GPSIMD: vi' = vi + m*(vj-vi); vj' = vj - m*(vi+vj)
        m3 = mask.broadcast_to([32, 3])
        dv = em.gtt(ALU.subtract, Vc[j], Vc[i], 3)
        md = em.gtt(ALU.mult, dv, m3, 3)
        vi = em.gtt(ALU.add, Vc[i], md, 3)
        sv = em.gtt(ALU.add, Vc[i], Vc[j], 3)
        ms = em.gtt(ALU.mult, sv, m3, 3)
        vj = em.gtt(ALU.subtract, Vc[j], ms, 3)
        Vc[i], Vc[j] = vi, vj

    cond_swap(0, 1)
    cond_swap(1, 2)
    cond_swap(0, 1)

    # --- B columns (j=0,1): b_j[r] = sum_c A[r][c] * V[c][j] ---
    Astr = [A[:, c:c + 7:3] for c in range(3)]

    def bcol(j):
        t0 = em.cdve(OP_AXPBY, Astr[0], Astr[1],
                     s0=Vc[j][:, 0:1], s1=Vc[j][:, 1:2], n=3)
        return em.stt(Astr[2], Vc[j][:, 2:3], t0, ALU.mult, ALU.add, 3)

    def bcol_gps(j):
        t0 = em.gtt(ALU.mult, Astr[0],
                    Vc[j][:, 0:1].broadcast_to([32, 3]), 3)
        t1 = em.gtt(ALU.mult, Astr[1],
                    Vc[j][:, 1:2].broadcast_to([32, 3]), 3)
        t01 = em.gtt(ALU.add, t0, t1, 3)
        t2 = em.gtt(ALU.mult, Astr[2],
                    Vc[j][:, 2:3].broadcast_to([32, 3]), 3)
        return em.gtt(ALU.add, t01, t2, 3)

    b0 = bcol(0)
    b1 = bcol_gps(1)

    def normalize(v3):
        sqv = em.tt(ALU.mult, v3, v3, 3)
        n_ = em.tt(ALU.add, sqv[:, 0:1], sqv[:, 1:2])
        n_ = em.tt(ALU.add, n_, sqv[:, 2:3])
        nc_ = em.ts(ALU.max, n_, 1e-30)
        inv = em.rsqrt(nc_)
        return em.ts(ALU.mult, v3, inv, 3)

    u1 = normalize(b0)
    p_ = em.tt(ALU.mult, u1, b1, 3)
    d_ = em.tt(ALU.add, p_[:, 0:1], p_[:, 1:2])
    d_ = em.tt(ALU.add, d_, p_[:, 2:3])
    dneg = em.ts(ALU.mult, d_, -1.0)
    b2o = em.stt(u1, dneg, b1, ALU.mult, ALU.add, 3)
    u2 = normalize(b2o)
    u3 = em.new(3)
    for k, (i1, i2) in enumerate(((1, 2), (2, 0), (0, 1))):
        em.cdve(OP_AXMBY, u1[:, i1:i1 + 1], u1[:, i2:i2 + 1],
                s0=u2[:, i2:i2 + 1], s1=u2[:, i1:i1 + 1],
                out=u3[:, k:k + 1])

    # --- R = u1 v1^T + u2 v2^T + u3 v3^T ---
    t0 = em.tt3(ALU.mult, _bcast_r(u1), _bcast_l(Vc[0]), 9)
    t1 = em.tt3(ALU.mult, _bcast_r(u2), _bcast_l(Vc[1]), 9)
    t01 = em.tt(ALU.add, t0, t1, 9)
    t2 = em.tt3(ALU.mult, _bcast_r(u3), _bcast_l(Vc[2]), 9)
    pose_R = pose_tile[:].rearrange("p (r c) -> p r c", r=4, c=4)[:, 0:3, 0:3]
    nc.vector.tensor_tensor(
        pose_R, t01.rearrange("p (r c) -> p r c", r=3, c=3),
        t2.rearrange("p (r c) -> p r c", r=3, c=3), ALU.add)


# ---------------------------------------------------------------------------
# kernel build
# ---------------------------------------------------------------------------
def build_nc():
    nc = bacc.Bacc("TRN2", target_bir_lowering=False)

    xT8 = nc.dram_tensor("xT8", [D, T_CORE], F8, kind="ExternalInput")
    w8 = nc.dram_tensor("w8", [6, 128, 2048], F8, kind="ExternalInput")
    bs = nc.dram_tensor("bs", [6, D], F32, kind="ExternalInput")
    x0s = nc.dram_tensor("x0s", [128, 4 * S_CORE], F32, kind="ExternalInput")
    mwt = nc.dram_tensor("mwt", [2, D, D], BF16, kind="ExternalInput")
    mbs = nc.dram_tensor("mbs", [2, D], F32, kind="ExternalInput")
    hwT = nc.dram_tensor("hwT", [D, 12], BF16, kind="ExternalInput")
    hb = nc.dram_tensor("hb", [S_CORE, 12], F32, kind="ExternalInput")
    pose = nc.dram_tensor("pose", [S_CORE, 16], F32, kind="ExternalOutput")

    with tile.TileContext(nc) as tc:
        with (
            tc.tile_pool(name="wp", bufs=1) as wpool,
            tc.tile_pool(name="xp", bufs=3) as xpool,
            tc.tile_pool(name="hp", bufs=2) as hpool,
            tc.tile_pool(name="h3p", bufs=2) as h3pool,
            tc.tile_pool(name="pp", bufs=1) as ppool,
            tc.tile_pool(name="ps", bufs=4, space="PSUM") as pspool,
            tc.tile_pool(name="sm", bufs=1) as smpool,
        ):
            # warm the ACT function table while DMAs stream
            warm = smpool.tile([32, 1], F32, tag="warm", name="warm")
            nc.vector.memset(warm[:], 0.0)
            nc.scalar.activation(warm[:], warm[:], AF.Relu)

            # ---- first x supertile + layer-0 weights first, chunked across
            # three DMA queues so the PE can start right after the preamble
            xt0 = xpool.tile([128, 4 * T_SUP], F8, tag="xt", name="xt")
            w_sb = [wpool.tile([128, 2048], F8, tag=f"w{l}", name=f"w{l}")
                    for l in range(6)]
            b_sb = wpool.tile([128, 24], F32, tag="b", name="b_sb")
            # x chunks first on gpsimd/scalar, w0 chunk-granular on sync so
            # the first (o0,kp0) matmul can fire ~2us after the preamble
            for k, eng in enumerate((nc.gpsimd, nc.scalar, nc.gpsimd,
                                     nc.scalar)):
                eng.dma_start(xt0[:, T_SUP * k:T_SUP * (k + 1)],
                              xT8[128 * k:128 * (k + 1), 0:T_SUP])
            for c in range(8):
                nc.sync.dma_start(w_sb[0][:, 256 * c:256 * (c + 1)],
                                  w8[0, :, 256 * c:256 * (c + 1)])
            nc.scalar.dma_start(w_sb[1][:], w8[1])
            nc.gpsimd.dma_start(w_sb[2][:], w8[2])
            for l in range(6):
                nc.sync.dma_start(b_sb[:, 4 * l:4 * l + 4],
                                  bs[l].rearrange("(o p) -> p o", p=128, o=4))
            nc.scalar.dma_start(w_sb[3][:], w8[3])
            nc.gpsimd.dma_start(w_sb[4][:], w8[4])
            nc.sync.dma_start(w_sb[5][:], w8[5])
            x0s_sb = wpool.tile([128, 4 * S_CORE], F32, tag="x0s",
                                name="x0s_sb")
            nc.sync.dma_start(x0s_sb[:], x0s[:])
            mw_sb = [wpool.tile([128, 2048], BF16, tag=f"mw{l}",
                                name=f"mw{l}") for l in range(2)]
            for l in range(2):
                for k in range(4):
                    nc.sync.dma_start(
                        mw_sb[l][:, D * k:D * (k + 1)],
                        mwt[l, 128 * k:128 * (k + 1), :])
            mb_sb = wpool.tile([128, 8], F32, tag="mb", name="mb_sb")
            for l in range(2):
                nc.sync.dma_start(mb_sb[:, 4 * l:4 * l + 4],
                                  mbs[l].rearrange("(o p) -> p o", p=128, o=4))
            hw_sb = wpool.tile([128, 48], BF16, tag="hw", name="hw_sb")
            for k in range(4):
                nc.sync.dma_start(hw_sb[:, 12 * k:12 * (k + 1)],
                                  hwT[128 * k:128 * (k + 1), :])
            hb_sb = wpool.tile([32, 12], F32, tag="hbt", name="hb_sb")
            nc.sync.dma_start(hb_sb[:], hb[:])

            # per-tile h3 sums for the two blocks: [128, 4k x 32 samples]
            pb1 = ppool.tile([128, 4 * S_CORE], F32, tag="pb1", name="pb1")
            pb2 = ppool.tile([128, 4 * S_CORE], F32, tag="pb2", name="pb2")

            def wap(l, o, kp):
                c0 = (o * 2 + kp) * 256
                return w_sb[l][:, c0:c0 + 256].rearrange(
                    "p (i m) -> p i m", i=2)

            def rhs(t, kp, th):
                # kp-pair chunks of a [128, 4*T_SUP] supertile, token half th
                return t[:, 2 * T_SUP * kp:2 * T_SUP * (kp + 1)].rearrange(
                    "p (i n) -> p i n", i=2)[:, :, 512 * th:512 * (th + 1)]

            def relu_drain(engine, h_slice, ps, bias_ap):
                if engine == 'act':
                    nc.scalar.activation(h_slice, ps[:], AF.Relu,
                                         bias=bias_ap, scale=1.0)
                else:
                    nc.vector.tensor_scalar(h_slice, ps[:], bias_ap, 0.0,
                                            ALU.add, ALU.max)

            # engine assignment per (layer, o): chunks 0 and 1 of every layer
            # drain on different engines so the next layer's kp0 matmuls
            # (which need both) never wait on one serial drain queue.
            # Totals: ACT 14, DVE 10 (+2 pooling reduces on DVE).
            ENG = {
                0: ['act', 'dve', 'act', 'act'],
                1: ['dve', 'act', 'dve', 'dve'],
                2: ['act', 'dve', 'act', 'act'],
                3: ['act', 'dve', 'act', 'act'],
                4: ['act', 'dve', 'act', 'act'],
                5: ['dve', 'act', 'dve', 'dve'],
            }

            # ---- main loop over supertiles (1024 tokens each) ----
            pending_red = None
            for ti in range(N_SUP):
                if ti == 0:
                    xt = xt0
                else:
                    xt = xpool.tile([128, 4 * T_SUP], F8, tag="xt",
                                    name="xt")
                    for k in range(4):
                        nc.gpsimd.dma_start(
                            xt[:, T_SUP * k:T_SUP * (k + 1)],
                            xT8[128 * k:128 * (k + 1),
                                T_SUP * ti:T_SUP * (ti + 1)])

                def emit_reduce(h3t, pb, tj, o=None):
                    """Per-supertile h3 pooling sum; o=None reduces all 4
                    chunks in one 4D op, else a single o-chunk."""
                    if o is None:
                        nc.vector.tensor_reduce(
                            pb[:].rearrange("p (o s) -> p o s", o=4,
                                            s=S_CORE)[:, :, S_SUP * tj:
                                                      S_SUP * (tj + 1)],
                            h3t[:].rearrange("p (o g t) -> p o g t", o=4,
                                             g=S_SUP),
                            axis=AX.X, op=ALU.add)
                    else:
                        nc.vector.tensor_reduce(
                            pb[:, S_CORE * o + S_SUP * tj:
                               S_CORE * o + S_SUP * (tj + 1)],
                            h3t[:, T_SUP * o:T_SUP * (o + 1)].rearrange(
                                "p (g t) -> p g t", g=S_SUP),
                            axis=AX.X, op=ALU.add)

                def run_layer(l, src, out_dtype=F8, tag="h", extra_src=None,
                              red_to=None):
                    pool_ = h3pool if l in (2, 5) else hpool
                    h_out = pool_.tile([128, 4 * T_SUP], out_dtype,
                                       tag=tag, name=f"h{l}")
                    for o in range(4):
                        ps = pspool.tile([128, T_SUP], F32, tag="ps",
                                         name="ps")
                        srcs = [src] if extra_src is None else [src,
                                                                extra_src]
                        n_mm = 4 * len(srcs)
                        mi = 0
                        for s_ in srcs:
                            for kp in range(2):
                                for th in range(2):
                                    nc.tensor.matmul(
                                        ps[:, 512 * th:512 * (th + 1)],
                                        wap(l, o, kp), rhs(s_, kp, th),
                                        start=(mi < 2),
                                        stop=(mi >= n_mm - 2),
                                        perf_mode=DR)
                                    mi += 1
                        relu_drain(ENG[l][o],
                                   h_out[:, T_SUP * o:T_SUP * (o + 1)],
                                   ps, b_sb[:, 4 * l + o:4 * l + o + 1])
                        if red_to is not None:
                            emit_reduce(h_out, red_to, ti, o=o)
                    return h_out

                h1 = run_layer(0, xt)
                h2 = run_layer(1, h1)
                if pending_red is not None:
                    # deferred h3b reduce of the previous supertile: lands
                    # behind this supertile's l1 drains in the DVE queue so
                    # the PE never waits on it
                    emit_reduce(pending_red[0], pb2, pending_red[1])
                    pending_red = None
                h3a = run_layer(2, h2, tag="h3a")          # fp8: matmul input
                emit_reduce(h3a, pb1, ti)
                g1 = run_layer(3, xt, extra_src=h3a)        # fused residual
                g2 = run_layer(4, g1)
                last = ti == N_SUP - 1
                h3b = run_layer(5, g2, out_dtype=BF16, tag="h3b",
                                red_to=pb2 if last else None)
                if not last:
                    pending_red = (h3b, ti)

            # ---- pooled = x0s + pb1 + pb2, per k-chunk so the first tail
            # matmuls start before the whole combine finishes ----
            pool_f32 = smpool.tile([128, 4 * S_CORE], F32, tag="poolf",
                                   name="pool_f32")
            pool_bf = smpool.tile([128, 4 * S_CORE], BF16, tag="poolb",
                                  name="pool_bf")
            for k in range(4):
                sl = slice(S_CORE * k, S_CORE * (k + 1))
                nc.vector.tensor_add(pool_f32[:, sl], pb1[:, sl], pb2[:, sl])
                nc.vector.tensor_add(pool_bf[:, sl], pool_f32[:, sl],
                                     x0s_sb[:, sl])

            # ---- tail MLPs (bf16), psum reused from the main pool ----
            f_prev = pool_bf
            scales = [1.0 / TOK, 1.0]
            for l in range(2):
                f_out = smpool.tile([128, 4 * S_CORE], BF16, tag=f"f{l}",
                                    name=f"f{l}")
                for o in range(4):
                    ps_w = pspool.tile([128, T_SUP], F32, tag="ps",
                                       name="pst")
                    ps = ps_w[:, 0:S_CORE]
                    for k in range(4):
                        nc.tensor.matmul(
                            ps,
                            mw_sb[l][:, D * k + 128 * o:D * k + 128 * (o + 1)],
                            f_prev[:, S_CORE * k:S_CORE * (k + 1)],
                            start=(k == 0), stop=(k == 3))
                    nc.scalar.activation(
                        f_out[:, S_CORE * o:S_CORE * (o + 1)], ps, AF.Relu,
                        bias=mb_sb[:, 4 * l + o:4 * l + o + 1],
                        scale=scales[l])
                f_prev = f_out

            # ---- heads: [32 samples, 12] = t(3) ++ rot(9) ----
            psh_w = pspool.tile([128, T_SUP], F32, tag="ps", name="psh")
            psh = psh_w[0:32, 0:12]
            for k in range(4):
                nc.tensor.matmul(psh,
                                 f_prev[:, S_CORE * k:S_CORE * (k + 1)],
                                 hw_sb[:, 12 * k:12 * (k + 1)],
                                 start=(k == 0), stop=(k == 3))
            mm = smpool.tile([32, 12], F32, tag="mm", name="mm")
            nc.vector.tensor_add(mm[:], psh, hb_sb[:])

            # ---- pose assembly + SVD ----
            pose_t = smpool.tile([32, 16], F32, tag="pose", name="pose_t")
            nc.vector.memset(pose_t[:], 0.0)
            nc.vector.memset(pose_t[:, 15:16], 1.0)
            nc.vector.tensor_copy(
                pose_t[:].rearrange("p (r c) -> p r c", r=4, c=4)[:, 0:3, 3],
                mm[:, 0:3])

            em = Emit(nc, smpool)
            emit_svd_so3(nc, em, mm[:, 3:12], pose_t)

            nc.sync.dma_start(pose[:], pose_t[:])

    nc.compile()
    return nc


_NC_CACHE = None


def _get_nc():
    global _NC_CACHE
    if _NC_CACHE is None:
        _NC_CACHE = build_nc()
    return _NC_CACHE


F8NP = ml_dtypes.float8_e4m3fn
BF16NP = ml_dtypes.bfloat16


def kernel(**inputs):
    feat = np.asarray(inputs["feat"], dtype=np.float32)
    b_, v_, n_, d_ = feat.shape
    xs = feat.reshape(b_ * v_, n_, d_)
    x0sum = xs.sum(axis=1, dtype=np.float32)          # (256, 512)

    # DoubleRow weight prepack: [p, o, kp, i, m] <- wT[128*(2kp+i)+p, 128o+m]
    w8_list = []
    for blk in (1, 2):
        for li in (1, 2, 3):
            wT = np.asarray(inputs[f"r{blk}_w{li}"], np.float32).T
            arr = wT.astype(F8NP).reshape(2, 2, 128, 4, 128)
            arr = np.ascontiguousarray(arr.transpose(2, 3, 0, 1, 4))
            w8_list.append(arr.reshape(128, 2048))
    w8 = np.stack(w8_list)
    bs = np.stack([np.asarray(inputs[f"r{blk}_b{li}"], np.float32)
                   for blk in (1, 2) for li in (1, 2, 3)])
    mwt = np.stack([np.ascontiguousarray(
        np.asarray(inputs[f"m_w{li}"], np.float32).T).astype(BF16NP)
        for li in (1, 2)])
    mbs = np.stack([np.asarray(inputs[f"m_b{li}"], np.float32)
                    for li in (1, 2)])
    hwT = np.ascontiguousarray(np.concatenate(
        [np.asarray(inputs["t_w"], np.float32).T,
         np.asarray(inputs["rot_w"], np.float32).T], axis=1)).astype(BF16NP)
    hb = np.broadcast_to(np.concatenate(
        [np.asarray(inputs["t_b"], np.float32),
         np.asarray(inputs["rot_b"], np.float32)])[None, :],
        (S_CORE, 12)).copy()

    in_maps = []
    for c in range(N_CORES):
        xT8 = np.ascontiguousarray(
            xs[c * S_CORE:(c + 1) * S_CORE].reshape(T_CORE, D).T).astype(F8NP)
        xs_c = x0sum[c * S_CORE:(c + 1) * S_CORE]     # (32, 512)
        x0s = np.ascontiguousarray(
            xs_c.T.reshape(4, 128, S_CORE).transpose(1, 0, 2).reshape(
                128, 4 * S_CORE))
        in_maps.append({
            "xT8": xT8, "w8": w8, "bs": bs, "x0s": x0s, "mwt": mwt,
            "mbs": mbs, "hwT": hwT, "hb": hb,
        })

    nc = _get_nc()
    import os
    kwargs = {}
    if os.environ.get("KERNEL_TRACE") == "1":
        kwargs["trace"] = True
    res = run_bass_kernel_spmd(nc, in_maps, core_ids=list(range(N_CORES)),
                               **kwargs)
    if kwargs.get("trace"):
        kernel.last_results = res
    poses = np.concatenate([r["pose"] for r in res.results], axis=0)
    return poses.reshape(b_, v_, 4, 4)


# revision 38
# speedup vs baseline: 1.6083x; 1.0037x over previous
"""CameraHead Trainium2 kernel — data-parallel over b*v across 8 NeuronCores.

fp8(e4m3) DoubleRow edition. Per-core layout: activations feature-major in
SBUF (X^T: 4x128-part chunks x tokens, fp8), all six 512x512 Linears run as
DoubleRow fp8 matmuls (K=256 per pass, 512-token moving operand, 2 passes for
K=512) at 2x the bf16/fp32r PE rate. Relu+bias drains PSUM on three engines
(ACT / DVE / GPSIMD) in parallel. The block-2 first Linear consumes x0 and
h3(block1) as two accumulated matmul streams, so the residual add never
materializes. Pooling = host-exact sum(x0) + on-device tensor_reduce over the
two h3 streams. Tail: bf16 MLP/head matmuls + a lean d-tracking Jacobi
(9 rotations) for the SVD->SO(3) projection, then pose assembly.
"""
import sys
import numpy as np

sys.path.insert(0, '/opt/trn_rl_repo')

import ml_dtypes  # noqa: E402

import concourse.bacc as bacc  # noqa: E402
import concourse.mybir as mybir  # noqa: E402
from concourse import tile  # noqa: E402
from concourse import dve_ops as _dvo  # noqa: E402
from concourse.bass_utils import run_bass_kernel_spmd  # noqa: E402
from concourse.dve_spec import (  # noqa: E402
    C0, C1, C2, One, Spec, Src0, Src1, select as dve_select, sq as dve_sq,
)


def _reg_op(name, body, ref):
    """Register a custom DVE op (per-NEFF uop table; no firmware change).

    The uops sha pin is bootstrapped by parsing compile()'s drift error."""
    for op in _dvo.OPS:
        if op.name == name:
            return op
    import re as _re

    from concourse.dve_table_gen import dve_ver_for

    row = _dvo._CUSTOM_DVE_ROW_BASE + len(_dvo.OPS)
    assert row < 0x20, "custom DVE opcode rows exhausted"
    spec = Spec(body=body, reference=ref)
    op = _dvo.DveOp(name, spec, subdim=False, uops_sha={})
    _dvo.OPS.append(op)
    _dvo._SUB_OPCODE_FOR_NAME[name] = row
    _dvo.CUSTOM_DVE_SPECS[name] = spec
    ver = dve_ver_for("TRN2")
    try:
        op.compile(ver)
    except ValueError as e:
        m = _re.search(r'uops_sha\["' + ver + r'"\]="([0-9a-f]+)"', str(e))
        if not m:
            raise
        op.uops_sha[ver] = m.group(1)
        op.compile(ver)
    return op


_f32 = np.float32
OP_AXPBY = _reg_op(
    "ANT_AXPBY", Src0 * C0 + Src1 * C1,
    lambda in0, in1, s0, s1, imm2: (in0 * s0 + in1 * s1).astype(_f32))
OP_AXMBY = _reg_op(
    "ANT_AXMBY", Src0 * C0 - Src1 * C1,
    lambda in0, in1, s0, s1, imm2: (in0 * s0 - in1 * s1).astype(_f32))
OP_AXPBY2 = _reg_op(
    "ANT_AXPBY2", (Src0 * C0 + Src1 * C1) * C2,
    lambda in0, in1, s0, s1, imm2: ((in0 * s0 + in1 * s1) * imm2).astype(_f32))
OP_AXMBY2 = _reg_op(
    "ANT_AXMBY2", (Src0 * C0 - Src1 * C1) * C2,
    lambda in0, in1, s0, s1, imm2: ((in0 * s0 - in1 * s1) * imm2).astype(_f32))
OP_SQDIFF = _reg_op(
    "ANT_SQDIFF", dve_sq(Src0) - dve_sq(Src1),
    lambda in0, in1, s0, s1, imm2: (in0 * in0 - in1 * in1).astype(_f32))


def _xy2_body():
    t = Src0 * Src1
    return t + t


OP_XY2 = _reg_op(
    "ANT_XY2", _xy2_body(),
    lambda in0, in1, s0, s1, imm2: (2.0 * in0 * in1).astype(_f32))
OP_WHERE = _reg_op(
    "ANT_WHERE", dve_select(C0, Src0, Src1),
    lambda in0, in1, s0, s1, imm2: np.where(
        s0 != 0, in0, in1).astype(_f32))
OP_WHERENEG = _reg_op(
    "ANT_WHERENEG", dve_select(C0, -Src0, Src1),
    lambda in0, in1, s0, s1, imm2: np.where(
        s0 != 0, -in0, in1).astype(_f32))
OP_SQSUM = _reg_op(
    "ANT_SQSUM", dve_sq(Src0) + dve_sq(Src1),
    lambda in0, in1, s0, s1, imm2: (in0 * in0 + in1 * in1).astype(_f32))
# (ch^2 - sh^2) * w2  — raw cosine
OP_CSUBW = _reg_op(
    "ANT_CSUBW", (dve_sq(Src0) - dve_sq(Src1)) * C0,
    lambda in0, in1, s0, s1, imm2: (
        (in0 * in0 - in1 * in1) * s0).astype(_f32))


def _xyw_body():
    t = Src0 * Src1
    return (t + t) * C0


# 2 * ch * sh * w2 — raw sine
OP_XYW2 = _reg_op(
    "ANT_XYW2", _xyw_body(),
    lambda in0, in1, s0, s1, imm2: (2.0 * in0 * in1 * s0).astype(_f32))
# gate value: ch^2 - gamma*sh^2 (>0 -> use raw angle, else pi/4 fallback)
OP_GATE = _reg_op(
    "ANT_GATE", dve_sq(Src0) - dve_sq(Src1) * C1,
    lambda in0, in1, s0, s1, imm2: (
        in0 * in0 - in1 * in1 * s1).astype(_f32))
# select(g < 0, imm2, raw)
OP_SELPOS = _reg_op(
    "ANT_SELPOS", dve_select(Src1 < C1, C2, Src0),
    lambda in0, in1, s0, s1, imm2: np.where(
        in1 < s1, imm2, in0).astype(_f32))


def _axpb4y_body():
    t = Src1 * C1
    u = t + t
    return Src0 * C0 + (u + u)


# d' = c2*d + 4*s2*pt
OP_AXPB4Y = _reg_op(
    "ANT_AXPB4Y", _axpb4y_body(),
    lambda in0, in1, s0, s1, imm2: (in0 * s0 + 4.0 * in1 * s1).astype(_f32))
# pt' = c2*pt - 0.25*s2*d   (imm2 carries the 0.25)
OP_AXMBYC = _reg_op(
    "ANT_AXMBYC", Src0 * C0 - (Src1 * C1) * C2,
    lambda in0, in1, s0, s1, imm2: (in0 * s0 - in1 * s1 * imm2).astype(_f32))

F32 = mybir.dt.float32
BF16 = mybir.dt.bfloat16
F16 = mybir.dt.float16
F8 = mybir.dt.float8e4
AF = mybir.ActivationFunctionType
ALU = mybir.AluOpType
AX = mybir.AxisListType
DR = mybir.MatmulPerfMode.DoubleRow

N_CORES = 8
D = 512
SAMPLES = 256          # b*v
TOK = 256              # tokens per sample
S_CORE = SAMPLES // N_CORES       # 32 samples per core
T_CORE = S_CORE * TOK             # 8192 token rows per core
T_TILE = 512
N_TILES = T_CORE // T_TILE        # 16
S_TILE = T_TILE // TOK            # 2 samples per token tile

T_SUP = 1024           # supertile: 2 token-tiles drained with wide ops
N_SUP = T_CORE // T_SUP           # 8
S_SUP = T_SUP // TOK              # 4 samples per supertile

N_ROT = 8              # lean Jacobi rotations (sim: rel_err ~6e-3)

GAMMA = float(3.0 + 2.0 * np.sqrt(2.0))
CQ45 = float(np.cos(np.pi / 4))


# ---------------------------------------------------------------------------
# small-op emitter for the SVD tail: SSA-style column allocation on a scratch
# tile; every value is an AP (or list of APs).
# ---------------------------------------------------------------------------
class Emit:
    def __init__(self, nc, pool):
        self.nc = nc
        self.scr = pool.tile([32, 2048], F32, tag="svd_scratch",
                             name="svd_scratch")
        self.ptr = 0

    def new(self, n=1):
        c = self.ptr
        self.ptr += n
        assert self.ptr <= 2048, "svd scratch overflow"
        return self.scr[:, c:c + n]

    def tt(self, op, a, b, n=1):
        o = self.new(n)
        self.nc.vector.tensor_tensor(o, a, b, op)
        return o

    def tt3(self, op, a, b, n=9):
        o = self.new(n)
        self.nc.vector.tensor_tensor(
            o.rearrange("p (i j) -> p i j", i=3, j=n // 3), a, b, op)
        return o

    def ts(self, op, a, s, n=1):
        o = self.new(n)
        self.nc.vector.tensor_scalar(o, a, s, None, op)
        return o

    def stt(self, a, scal, b, op0, op1, n=1):
        o = self.new(n)
        self.nc.vector.scalar_tensor_tensor(o, a, scal, b, op0=op0, op1=op1)
        return o

    # --- gpsimd variant (tensor_tensor only; Pool supports no Ptr ops) ---
    def gtt(self, op, a, b, n=1):
        o = self.new(n)
        self.nc.gpsimd.tensor_tensor(o, a, b, op)
        return o

    def rsqrt(self, a, n=1):
        t = self.new(n)
        self.nc.scalar.activation(t, a, AF.Sqrt)
        o = self.new(n)
        self.nc.vector.reciprocal(o, t)
        return o

    def cdve(self, op, in0, in1, s0=0.0, s1=0.0, imm2=0.0, n=1, out=None):
        if out is None:
            out = self.new(n)
        self.nc.vector._custom_dve(op, out=out, in0=in0, in1=in1,
                                   s0=s0, s1=s1, imm2=imm2)
        return out

    def const(self, val, n=1):
        o = self.new(n)
        self.nc.vector.memset(o, val)
        return o


def _bcast_r(ap3):
    return ap3.unsqueeze(2).broadcast_to([32, 3, 3])


def _bcast_l(ap3):
    return ap3.unsqueeze(1).broadcast_to([32, 3, 3])


def emit_svd_so3(nc, em, m_ap, pose_tile):
    """m_ap: [32,9] raw 3x3 per sample (row-major). Writes the SO(3)
    projection into pose_tile columns (4r+c for r,c in 0..2).

    Lean d-tracking Jacobi: state is (d01,d12,d02) eigenvalue differences and
    (p01,p12,p02) halved off-diagonals; 15 DVE ops per rotation."""
    # --- row normalize ---
    sq = em.tt(ALU.mult, m_ap, m_ap, 9)
    t = em.tt(ALU.add, sq[:, 0:9:3], sq[:, 1:9:3], 3)
    r2 = em.tt(ALU.add, t, sq[:, 2:9:3], 3)
    r2c = em.ts(ALU.max, r2, 1e-24, 3)
    rinv = em.rsqrt(r2c, 3)
    A = em.tt3(ALU.mult, m_ap.rearrange("p (r c) -> p r c", r=3, c=3),
               _bcast_r(rinv), 9)

    # --- S = A^T A (s_ij at col 3i+j) ---
    terms = []
    for r in range(3):
        arow = A[:, 3 * r:3 * r + 3]
        terms.append(em.tt3(ALU.mult, _bcast_r(arow), _bcast_l(arow), 9))
    s01 = em.tt(ALU.add, terms[0], terms[1], 9)
    S9 = em.tt(ALU.add, s01, terms[2], 9)

    # d/pt state (SSA-tracked APs)
    d01 = em.tt(ALU.subtract, S9[:, 0:1], S9[:, 4:5])
    d12 = em.tt(ALU.subtract, S9[:, 4:5], S9[:, 8:9])
    d02 = em.tt(ALU.add, d01, d12)
    p01 = em.ts(ALU.mult, S9[:, 1:2], 0.5)
    p12 = em.ts(ALU.mult, S9[:, 5:6], 0.5)
    p02 = em.ts(ALU.mult, S9[:, 2:3], 0.5)

    # V columns as [32,3] blocks, init = identity
    Vc = []
    for j in range(3):
        vj = em.new(3)
        nc.vector.memset(vj, 0.0)
        nc.vector.memset(vj[:, j:j + 1], 1.0)
        Vc.append(vj)

    st = {'d01': d01, 'd12': d12, 'd02': d02,
          'p01': p01, 'p12': p12, 'p02': p02}

    def angle(ch, sh):
        # gate condition gamma*sh^2 >= ch^2 is exactly craw <= cos(pi/4),
        # so the pi/4 fallback is a clamp on craw and a craw-keyed select.
        ssum = em.cdve(OP_SQSUM, ch, sh)
        w2 = em.new(1)
        nc.vector.reciprocal(w2, ssum)
        craw = em.cdve(OP_CSUBW, ch, sh, s0=w2)
        sraw = em.cdve(OP_XYW2, ch, sh, s0=w2)
        c = em.ts(ALU.max, craw, CQ45)
        s = em.cdve(OP_SELPOS, sraw, craw, s1=CQ45, imm2=CQ45)
        c2 = em.cdve(OP_SQDIFF, c, s)
        s2 = em.cdve(OP_XY2, c, s)
        return c, s, c2, s2

    def np_pair(pa, pb_, c, s):
        """Rotate the r-row p~ pair on GPSIMD; it runs concurrently with the
        DVE's own post-angle update ops and hands back to the next angle."""
        t1 = em.gtt(ALU.mult, pa, c, 1)
        t2 = em.gtt(ALU.mult, pb_, s, 1)
        na = em.gtt(ALU.add, t1, t2, 1)
        t3 = em.gtt(ALU.mult, pb_, c, 1)
        t4 = em.gtt(ALU.mult, pa, s, 1)
        nb = em.gtt(ALU.subtract, t3, t4, 1)
        return na, nb

    def vup(p, q, c, s):
        # V rotation runs on GPSIMD, off the DVE critical chain (the V
        # columns are only consumed after the rotation loop).
        cb = c.broadcast_to([32, 3])
        sb = s.broadcast_to([32, 3])
        t1 = em.gtt(ALU.mult, Vc[q], sb, 3)
        t2 = em.gtt(ALU.mult, Vc[p], cb, 3)
        nvp = em.gtt(ALU.add, t2, t1, 3)
        t3 = em.gtt(ALU.mult, Vc[p], sb, 3)
        t4 = em.gtt(ALU.mult, Vc[q], cb, 3)
        nvq = em.gtt(ALU.subtract, t4, t3, 3)
        Vc[p], Vc[q] = nvp, nvq

    for k in range(N_ROT):
        rt = k % 3
        if rt == 0:     # (p,q,r) = (0,1,2)
            c, s, c2, s2 = angle(st['d01'], st['p01'])
            dd = em.cdve(OP_AXPB4Y, st['d01'], st['p01'], s0=c2, s1=s2)
            pn = em.cdve(OP_AXMBYC, st['p01'], st['d01'], s0=c2, s1=s2,
                         imm2=0.25)
            t_ = em.tt(ALU.add, st['d02'], st['d12'])
            nd02 = em.cdve(OP_AXPBY, t_, dd, s0=0.5, s1=0.5)
            nd12 = em.cdve(OP_AXMBY, t_, dd, s0=0.5, s1=0.5)
            np02, np12 = np_pair(st['p02'], st['p12'], c, s)
            st.update(d01=dd, p01=pn, d02=nd02, d12=nd12, p02=np02, p12=np12)
            vup(0, 1, c, s)
        elif rt == 1:   # (1,2,0)
            c, s, c2, s2 = angle(st['d12'], st['p12'])
            dd = em.cdve(OP_AXPB4Y, st['d12'], st['p12'], s0=c2, s1=s2)
            pn = em.cdve(OP_AXMBYC, st['p12'], st['d12'], s0=c2, s1=s2,
                         imm2=0.25)
            t_ = em.tt(ALU.add, st['d01'], st['d02'])
            nd01 = em.cdve(OP_AXMBY, t_, dd, s0=0.5, s1=0.5)
            nd02 = em.cdve(OP_AXPBY, t_, dd, s0=0.5, s1=0.5)
            np01, np02 = np_pair(st['p01'], st['p02'], c, s)
            st.update(d12=dd, p12=pn, d01=nd01, d02=nd02, p01=np01, p02=np02)
            vup(1, 2, c, s)
        else:           # (0,2,1)
            c, s, c2, s2 = angle(st['d02'], st['p02'])
            dd = em.cdve(OP_AXPB4Y, st['d02'], st['p02'], s0=c2, s1=s2)
            pn = em.cdve(OP_AXMBYC, st['p02'], st['d02'], s0=c2, s1=s2,
                         imm2=0.25)
            t_ = em.tt(ALU.subtract, st['d01'], st['d12'])
            nd01 = em.cdve(OP_AXPBY, t_, dd, s0=0.5, s1=0.5)
            nd12 = em.cdve(OP_AXMBY, dd, t_, s0=0.5, s1=0.5)
            np01, np12 = np_pair(st['p01'], st['p12'], c, s)
            st.update(d02=dd, p02=pn, d01=nd01, d12=nd12, p01=np01, p12=np12)
            vup(0, 2, c, s)

    # --- sort eigenpairs descending (det(V) stays +1 via column negation) ---
    def cond_swap(i, j):
        key_ij = f'd{i}{j}'
        k = 3 - i - j
        key_ik = f'd{min(i,k)}{max(i,k)}'
        key_jk = f'd{min(j,k)}{max(j,k)}'
        mask = em.ts(ALU.is_lt, st[key_ij], 0.0)
        nij = em.cdve(OP_WHERENEG, st[key_ij], st[key_ij], s0=mask)
        nik = em.cdve(OP_WHERE, st[key_jk], st[key_ik], s0=mask)
        njk = em.cdve(OP_WHERE, st[key_ik], st[key_jk], s0=mask)
        st[key_ij], st[key_ik], st[key_jk] = nij, nik, njk
        # V swap on GPSIMD: vi' = vi + m*(vj-vi); vj' = vj - m*(vi+vj)
        m3 = mask.broadcast_to([32, 3])
        dv = em.gtt(ALU.subtract, Vc[j], Vc[i], 3)
        md = em.gtt(ALU.mult, dv, m3, 3)
        vi = em.gtt(ALU.add, Vc[i], md, 3)
        sv = em.gtt(ALU.add, Vc[i], Vc[j], 3)
        ms = em.gtt(ALU.mult, sv, m3, 3)
        vj = em.gtt(ALU.subtract, Vc[j], ms, 3)
        Vc[i], Vc[j] = vi, vj

    cond_swap(0, 1)
    cond_swap(1, 2)
    cond_swap(0, 1)

    # --- B columns (j=0,1): b_j[r] = sum_c A[r][c] * V[c][j] ---
    Astr = [A[:, c:c + 7:3] for c in range(3)]

    def bcol(j):
        t0 = em.cdve(OP_AXPBY, Astr[0], Astr[1],
                     s0=Vc[j][:, 0:1], s1=Vc[j][:, 1:2], n=3)
        return em.stt(Astr[2], Vc[j][:, 2:3], t0, ALU.mult, ALU.add, 3)

    def bcol_gps(j):
        t0 = em.gtt(ALU.mult, Astr[0],
                    Vc[j][:, 0:1].broadcast_to([32, 3]), 3)
        t1 = em.gtt(ALU.mult, Astr[1],
                    Vc[j][:, 1:2].broadcast_to([32, 3]), 3)
        t01 = em.gtt(ALU.add, t0, t1, 3)
        t2 = em.gtt(ALU.mult, Astr[2],
                    Vc[j][:, 2:3].broadcast_to([32, 3]), 3)
        return em.gtt(ALU.add, t01, t2, 3)

    b0 = bcol(0)
    b1 = bcol_gps(1)

    def normalize(v3):
        sqv = em.tt(ALU.mult, v3, v3, 3)
        n_ = em.tt(ALU.add, sqv[:, 0:1], sqv[:, 1:2])
        n_ = em.tt(ALU.add, n_, sqv[:, 2:3])
        nc_ = em.ts(ALU.max, n_, 1e-30)
        inv = em.rsqrt(nc_)
        return em.ts(ALU.mult, v3, inv, 3)

    u1 = normalize(b0)
    p_ = em.tt(ALU.mult, u1, b1, 3)
    d_ = em.tt(ALU.add, p_[:, 0:1], p_[:, 1:2])
    d_ = em.tt(ALU.add, d_, p_[:, 2:3])
    dneg = em.ts(ALU.mult, d_, -1.0)
    b2o = em.stt(u1, dneg, b1, ALU.mult, ALU.add, 3)
    u2 = normalize(b2o)
    u3 = em.new(3)
    for k, (i1, i2) in enumerate(((1, 2), (2, 0), (0, 1))):
        em.cdve(OP_AXMBY, u1[:, i1:i1 + 1], u1[:, i2:i2 + 1],
                s0=u2[:, i2:i2 + 1], s1=u2[:, i1:i1 + 1],
                out=u3[:, k:k + 1])

    # --- R = u1 v1^T + u2 v2^T + u3 v3^T ---
    t0 = em.tt3(ALU.mult, _bcast_r(u1), _bcast_l(Vc[0]), 9)
    t1 = em.tt3(ALU.mult, _bcast_r(u2), _bcast_l(Vc[1]), 9)
    t01 = em.tt(ALU.add, t0, t1, 9)
    t2 = em.tt3(ALU.mult, _bcast_r(u3), _bcast_l(Vc[2]), 9)
    pose_R = pose_tile[:].rearrange("p (r c) -> p r c", r=4, c=4)[:, 0:3, 0:3]
    nc.vector.tensor_tensor(
        pose_R, t01.rearrange("p (r c) -> p r c", r=3, c=3),
        t2.rearrange("p (r c) -> p r c", r=3, c=3), ALU.add)


# ---------------------------------------------------------------------------
# kernel build
# ---------------------------------------------------------------------------
def build_nc():
    nc = bacc.Bacc("TRN2", target_bir_lowering=False)

    xT8 = nc.dram_tensor("xT8", [D, T_CORE], F8, kind="ExternalInput")
    w8 = nc.dram_tensor("w8", [6, 128, 2048], F8, kind="ExternalInput")
    bs = nc.dram_tensor("bs", [6, D], F32, kind="ExternalInput")
    x0s = nc.dram_tensor("x0s", [128, 4 * S_CORE], F32, kind="ExternalInput")
    mwt = nc.dram_tensor("mwt", [2, D, D], BF16, kind="ExternalInput")
    mbs = nc.dram_tensor("mbs", [2, D], F32, kind="ExternalInput")
    hwT = nc.dram_tensor("hwT", [D, 12], BF16, kind="ExternalInput")
    hb = nc.dram_tensor("hb", [S_CORE, 12], F32, kind="ExternalInput")
    pose = nc.dram_tensor("pose", [S_CORE, 16], F32, kind="ExternalOutput")

    with tile.TileContext(nc) as tc:
        with (
            tc.tile_pool(name="wp", bufs=1) as wpool,
            tc.tile_pool(name="xp", bufs=3) as xpool,
            tc.tile_pool(name="hp", bufs=2) as hpool,
            tc.tile_pool(name="h3p", bufs=2) as h3pool,
            tc.tile_pool(name="pp", bufs=1) as ppool,
            tc.tile_pool(name="ps", bufs=4, space="PSUM") as pspool,
            tc.tile_pool(name="sm", bufs=1) as smpool,
        ):
            # warm the ACT function table while DMAs stream
            warm = smpool.tile([32, 1], F32, tag="warm", name="warm")
            nc.vector.memset(warm[:], 0.0)
            nc.scalar.activation(warm[:], warm[:], AF.Relu)

            # ---- first x supertile + layer-0 weights first, chunked across
            # three DMA queues so the PE can start right after the preamble
            xt0 = xpool.tile([128, 4 * T_SUP], F8, tag="xt", name="xt")
            w_sb = [wpool.tile([128, 2048], F8, tag=f"w{l}", name=f"w{l}")
                    for l in range(6)]
            b_sb = wpool.tile([128, 24], F32, tag="b", name="b_sb")
            # x chunks first on gpsimd/scalar, w0 chunk-granular on sync so
            # the first (o0,kp0) matmul can fire ~2us after the preamble
            for k, eng in enumerate((nc.gpsimd, nc.scalar, nc.gpsimd,
                                     nc.scalar)):
                for h in range(2):  # halves: all 4 chunks land ~2 transfers in
                    eng.dma_start(
                        xt0[:, T_SUP * k + 512 * h:T_SUP * k + 512 * (h + 1)],
                        xT8[128 * k:128 * (k + 1), 512 * h:512 * (h + 1)])
            for c in range(8):
                nc.sync.dma_start(w_sb[0][:, 256 * c:256 * (c + 1)],
                                  w8[0, :, 256 * c:256 * (c + 1)])
            nc.scalar.dma_start(w_sb[1][:], w8[1])
            nc.gpsimd.dma_start(w_sb[2][:], w8[2])
            for l in range(6):
                nc.sync.dma_start(b_sb[:, 4 * l:4 * l + 4],
                                  bs[l].rearrange("(o p) -> p o", p=128, o=4))
            nc.scalar.dma_start(w_sb[3][:], w8[3])
            nc.gpsimd.dma_start(w_sb[4][:], w8[4])
            nc.sync.dma_start(w_sb[5][:], w8[5])
            x0s_sb = wpool.tile([128, 4 * S_CORE], F32, tag="x0s",
                                name="x0s_sb")
            nc.sync.dma_start(x0s_sb[:], x0s[:])
            mw_sb = [wpool.tile([128, 2048], BF16, tag=f"mw{l}",
                                name=f"mw{l}") for l in range(2)]
            for l in range(2):
                for k in range(4):
                    nc.sync.dma_start(
                        mw_sb[l][:, D * k:D * (k + 1)],
                        mwt[l, 128 * k:128 * (k + 1), :])
            mb_sb = wpool.tile([128, 8], F32, tag="mb", name="mb_sb")
            for l in range(2):
                nc.sync.dma_start(mb_sb[:, 4 * l:4 * l + 4],
                                  mbs[l].rearrange("(o p) -> p o", p=128, o=4))
            hw_sb = wpool.tile([128, 48], BF16, tag="hw", name="hw_sb")
            for k in range(4):
                nc.sync.dma_start(hw_sb[:, 12 * k:12 * (k + 1)],
                                  hwT[128 * k:128 * (k + 1), :])
            hb_sb = wpool.tile([32, 12], F32, tag="hbt", name="hb_sb")
            nc.sync.dma_start(hb_sb[:], hb[:])

            # per-tile h3 sums for the two blocks: [128, 4k x 32 samples].
            # pb2 partials accumulate in fp16: 2-byte operands give the DVE
            # reduce its 2x mode; the ~0.03 ulp on ~50-magnitude sums is
            # ~2e-4 relative after the /256 pooling divide.
            pb1 = ppool.tile([128, 4 * S_CORE], F32, tag="pb1", name="pb1")
            pb2 = ppool.tile([128, 4 * S_CORE], F16, tag="pb2", name="pb2")

            def wap(l, o, kp):
                c0 = (o * 2 + kp) * 256
                return w_sb[l][:, c0:c0 + 256].rearrange(
                    "p (i m) -> p i m", i=2)

            def rhs(t, kp, th):
                # kp-pair chunks of a [128, 4*T_SUP] supertile, token half th
                return t[:, 2 * T_SUP * kp:2 * T_SUP * (kp + 1)].rearrange(
                    "p (i n) -> p i n", i=2)[:, :, 512 * th:512 * (th + 1)]

            def relu_drain(engine, h_slice, ps, bias_ap):
                if engine == 'act':
                    nc.scalar.activation(h_slice, ps[:], AF.Relu,
                                         bias=bias_ap, scale=1.0)
                else:
                    nc.vector.tensor_scalar(h_slice, ps[:], bias_ap, 0.0,
                                            ALU.add, ALU.max)

            # engine assignment per (layer, o): chunks 0 and 1 of every layer
            # drain on different engines so the next layer's kp0 matmuls
            # (which need both) never wait on one serial drain queue.
            # Totals: ACT 14, DVE 10 (+2 pooling reduces on DVE).
            ENG = {
                0: ['act', 'dve', 'act', 'act'],
                1: ['dve', 'act', 'dve', 'dve'],
                2: ['act', 'dve', 'act', 'act'],
                3: ['act', 'dve', 'act', 'act'],
                4: ['act', 'dve', 'act', 'act'],
                5: ['dve', 'act', 'dve', 'dve'],
            }

            # ---- main loop over supertiles (1024 tokens each) ----
            pending_red = None
            for ti in range(N_SUP):
                if ti == 0:
                    xt = xt0
                else:
                    xt = xpool.tile([128, 4 * T_SUP], F8, tag="xt",
                                    name="xt")
                    for k in range(4):
                        nc.gpsimd.dma_start(
                            xt[:, T_SUP * k:T_SUP * (k + 1)],
                            xT8[128 * k:128 * (k + 1),
                                T_SUP * ti:T_SUP * (ti + 1)])

                def emit_reduce(h3t, pb, tj, o=None):
                    """Per-supertile h3 pooling sum; o=None reduces all 4
                    chunks in one 4D op, else a single o-chunk."""
                    with nc.allow_low_precision("fp16 pooling partials"):
                        if o is None:
                            nc.vector.tensor_reduce(
                                pb[:].rearrange("p (o s) -> p o s", o=4,
                                                s=S_CORE)[:, :, S_SUP * tj:
                                                          S_SUP * (tj + 1)],
                                h3t[:].rearrange("p (o g t) -> p o g t", o=4,
                                                 g=S_SUP),
                                axis=AX.X, op=ALU.add)
                        else:
                            nc.vector.tensor_reduce(
                                pb[:, S_CORE * o + S_SUP * tj:
                                   S_CORE * o + S_SUP * (tj + 1)],
                                h3t[:, T_SUP * o:T_SUP * (o + 1)].rearrange(
                                    "p (g t) -> p g t", g=S_SUP),
                                axis=AX.X, op=ALU.add)

                def run_layer(l, src, out_dtype=F8, tag="h", extra_src=None,
                              red_to=None):
                    pool_ = h3pool if l in (2, 5) else hpool
                    h_out = pool_.tile([128, 4 * T_SUP], out_dtype,
                                       tag=tag, name=f"h{l}")
                    for o in range(4):
                        ps = pspool.tile([128, T_SUP], F32, tag="ps",
                                         name="ps")
                        srcs = [src] if extra_src is None else [src,
                                                                extra_src]
                        n_mm = 4 * len(srcs)
                        mi = 0
                        for s_ in srcs:
                            for kp in range(2):
                                for th in range(2):
                                    nc.tensor.matmul(
                                        ps[:, 512 * th:512 * (th + 1)],
                                        wap(l, o, kp), rhs(s_, kp, th),
                                        start=(mi < 2),
                                        stop=(mi >= n_mm - 2),
                                        perf_mode=DR)
                                    mi += 1
                        relu_drain(ENG[l][o],
                                   h_out[:, T_SUP * o:T_SUP * (o + 1)],
                                   ps, b_sb[:, 4 * l + o:4 * l + o + 1])
                        if red_to is not None:
                            emit_reduce(h_out, red_to, ti, o=o)
                    return h_out

                h1 = run_layer(0, xt)
                h2 = run_layer(1, h1)
                if pending_red is not None:
                    # deferred h3b reduce of the previous supertile: lands
                    # behind this supertile's l1 drains in the DVE queue so
                    # the PE never waits on it
                    emit_reduce(pending_red[0], pb2, pending_red[1])
                    pending_red = None
                h3a = run_layer(2, h2, tag="h3a")          # fp8: matmul input
                emit_reduce(h3a, pb1, ti)
                g1 = run_layer(3, xt, extra_src=h3a)        # fused residual
                g2 = run_layer(4, g1)
                last = ti == N_SUP - 1
                h3b = run_layer(5, g2, out_dtype=BF16, tag="h3b",
                                red_to=pb2 if last else None)
                if not last:
                    pending_red = (h3b, ti)

            # ---- pooled = x0s + pb1 + pb2, per k-chunk so the first tail
            # matmuls start before the whole combine finishes ----
            pool_f32 = smpool.tile([128, 4 * S_CORE], F32, tag="poolf",
                                   name="pool_f32")
            pool_bf = smpool.tile([128, 4 * S_CORE], BF16, tag="poolb",
                                  name="pool_bf")
            for k in range(4):
                sl = slice(S_CORE * k, S_CORE * (k + 1))
                nc.gpsimd.tensor_tensor(pool_f32[:, sl], pb1[:, sl],
                                        pb2[:, sl], ALU.add)
                nc.gpsimd.tensor_tensor(pool_bf[:, sl], pool_f32[:, sl],
                                        x0s_sb[:, sl], ALU.add)

            # ---- tail MLPs (bf16), psum reused from the main pool ----
            f_prev = pool_bf
            scales = [1.0 / TOK, 1.0]
            for l in range(2):
                f_out = smpool.tile([128, 4 * S_CORE], BF16, tag=f"f{l}",
                                    name=f"f{l}")
                for o in range(4):
                    ps_w = pspool.tile([128, T_SUP], F32, tag="ps",
                                       name="pst")
                    ps = ps_w[:, 0:S_CORE]
                    for k in range(4):
                        nc.tensor.matmul(
                            ps,
                            mw_sb[l][:, D * k + 128 * o:D * k + 128 * (o + 1)],
                            f_prev[:, S_CORE * k:S_CORE * (k + 1)],
                            start=(k == 0), stop=(k == 3))
                    nc.scalar.activation(
                        f_out[:, S_CORE * o:S_CORE * (o + 1)], ps, AF.Relu,
                        bias=mb_sb[:, 4 * l + o:4 * l + o + 1],
                        scale=scales[l])
                f_prev = f_out

            # ---- heads: [32 samples, 12] = t(3) ++ rot(9) ----
            psh_w = pspool.tile([128, T_SUP], F32, tag="ps", name="psh")
            psh = psh_w[0:32, 0:12]
            for k in range(4):
                nc.tensor.matmul(psh,
                                 f_prev[:, S_CORE * k:S_CORE * (k + 1)],
                                 hw_sb[:, 12 * k:12 * (k + 1)],
                                 start=(k == 0), stop=(k == 3))
            mm = smpool.tile([32, 12], F32, tag="mm", name="mm")
            nc.vector.tensor_add(mm[:], psh, hb_sb[:])

            # ---- pose assembly + SVD ----
            pose_t = smpool.tile([32, 16], F32, tag="pose", name="pose_t")
            nc.vector.memset(pose_t[:], 0.0)
            nc.vector.memset(pose_t[:, 15:16], 1.0)
            nc.vector.tensor_copy(
                pose_t[:].rearrange("p (r c) -> p r c", r=4, c=4)[:, 0:3, 3],
                mm[:, 0:3])

            em = Emit(nc, smpool)
            emit_svd_so3(nc, em, mm[:, 3:12], pose_t)

            nc.sync.dma_start(pose[:], pose_t[:])

    nc.compile()
    return nc


_NC_CACHE = None


def _get_nc():
    global _NC_CACHE
    if _NC_CACHE is None:
        _NC_CACHE = build_nc()
    return _NC_CACHE


F8NP = ml_dtypes.float8_e4m3fn
BF16NP = ml_dtypes.bfloat16


def kernel(**inputs):
    feat = np.asarray(inputs["feat"], dtype=np.float32)
    b_, v_, n_, d_ = feat.shape
    xs = feat.reshape(b_ * v_, n_, d_)
    x0sum = xs.sum(axis=1, dtype=np.float32)          # (256, 512)

    # DoubleRow weight prepack: [p, o, kp, i, m] <- wT[128*(2kp+i)+p, 128o+m]
    w8_list = []
    for blk in (1, 2):
        for li in (1, 2, 3):
            wT = np.asarray(inputs[f"r{blk}_w{li}"], np.float32).T
            arr = wT.astype(F8NP).reshape(2, 2, 128, 4, 128)
            arr = np.ascontiguousarray(arr.transpose(2, 3, 0, 1, 4))
            w8_list.append(arr.reshape(128, 2048))
    w8 = np.stack(w8_list)
    bs = np.stack([np.asarray(inputs[f"r{blk}_b{li}"], np.float32)
                   for blk in (1, 2) for li in (1, 2, 3)])
    mwt = np.stack([np.ascontiguousarray(
        np.asarray(inputs[f"m_w{li}"], np.float32).T).astype(BF16NP)
        for li in (1, 2)])
    mbs = np.stack([np.asarray(inputs[f"m_b{li}"], np.float32)
                    for li in (1, 2)])
    hwT = np.ascontiguousarray(np.concatenate(
        [np.asarray(inputs["t_w"], np.float32).T,
         np.asarray(inputs["rot_w"], np.float32).T], axis=1)).astype(BF16NP)
    hb = np.broadcast_to(np.concatenate(
        [np.asarray(inputs["t_b"], np.float32),
         np.asarray(inputs["rot_b"], np.float32)])[None, :],
        (S_CORE, 12)).copy()

    in_maps = []
    for c in range(N_CORES):
        xT8 = np.ascontiguousarray(
            xs[c * S_CORE:(c + 1) * S_CORE].reshape(T_CORE, D).T).astype(F8NP)
        xs_c = x0sum[c * S_CORE:(c + 1) * S_CORE]     # (32, 512)
        x0s = np.ascontiguousarray(
            xs_c.T.reshape(4, 128, S_CORE).transpose(1, 0, 2).reshape(
                128, 4 * S_CORE))
        in_maps.append({
            "xT8": xT8, "w8": w8, "bs": bs, "x0s": x0s, "mwt": mwt,
            "mbs": mbs, "hwT": hwT, "hb": hb,
        })

    nc = _get_nc()
    import os
    kwargs = {}
    if os.environ.get("KERNEL_TRACE") == "1":
        kwargs["trace"] = True
    res = run_bass_kernel_spmd(nc, in_maps, core_ids=list(range(N_CORES)),
                               **kwargs)
    if kwargs.get("trace"):
        kernel.last_results = res
    poses = np.concatenate([r["pose"] for r in res.results], axis=0)
    return poses.reshape(b_, v_, 4, 4)


# revision 40
# speedup vs baseline: 1.6283x; 1.0125x over previous
"""CameraHead Trainium2 kernel — data-parallel over b*v across 8 NeuronCores.

fp8(e4m3) DoubleRow edition. Per-core layout: activations feature-major in
SBUF (X^T: 4x128-part chunks x tokens, fp8), all six 512x512 Linears run as
DoubleRow fp8 matmuls (K=256 per pass, 512-token moving operand, 2 passes for
K=512) at 2x the bf16/fp32r PE rate. Relu+bias drains PSUM on three engines
(ACT / DVE / GPSIMD) in parallel. The block-2 first Linear consumes x0 and
h3(block1) as two accumulated matmul streams, so the residual add never
materializes. Pooling = host-exact sum(x0) + on-device tensor_reduce over the
two h3 streams. Tail: bf16 MLP/head matmuls + a lean d-tracking Jacobi
(9 rotations) for the SVD->SO(3) projection, then pose assembly.
"""
import sys
import numpy as np

sys.path.insert(0, '/opt/trn_rl_repo')

import ml_dtypes  # noqa: E402

import concourse.bacc as bacc  # noqa: E402
import concourse.mybir as mybir  # noqa: E402
from concourse import tile  # noqa: E402
from concourse import dve_ops as _dvo  # noqa: E402
from concourse.bass_utils import run_bass_kernel_spmd  # noqa: E402
from concourse.dve_spec import (  # noqa: E402
    C0, C1, C2, One, Spec, Src0, Src1, select as dve_select, sq as dve_sq,
)


def _reg_op(name, body, ref):
    """Register a custom DVE op (per-NEFF uop table; no firmware change).

    The uops sha pin is bootstrapped by parsing compile()'s drift error."""
    for op in _dvo.OPS:
        if op.name == name:
            return op
    import re as _re

    from concourse.dve_table_gen import dve_ver_for

    row = _dvo._CUSTOM_DVE_ROW_BASE + len(_dvo.OPS)
    assert row < 0x20, "custom DVE opcode rows exhausted"
    spec = Spec(body=body, reference=ref)
    op = _dvo.DveOp(name, spec, subdim=False, uops_sha={})
    _dvo.OPS.append(op)
    _dvo._SUB_OPCODE_FOR_NAME[name] = row
    _dvo.CUSTOM_DVE_SPECS[name] = spec
    ver = dve_ver_for("TRN2")
    try:
        op.compile(ver)
    except ValueError as e:
        m = _re.search(r'uops_sha\["' + ver + r'"\]="([0-9a-f]+)"', str(e))
        if not m:
            raise
        op.uops_sha[ver] = m.group(1)
        op.compile(ver)
    return op


_f32 = np.float32
OP_AXPBY = _reg_op(
    "ANT_AXPBY", Src0 * C0 + Src1 * C1,
    lambda in0, in1, s0, s1, imm2: (in0 * s0 + in1 * s1).astype(_f32))
OP_AXMBY = _reg_op(
    "ANT_AXMBY", Src0 * C0 - Src1 * C1,
    lambda in0, in1, s0, s1, imm2: (in0 * s0 - in1 * s1).astype(_f32))
OP_AXPBY2 = _reg_op(
    "ANT_AXPBY2", (Src0 * C0 + Src1 * C1) * C2,
    lambda in0, in1, s0, s1, imm2: ((in0 * s0 + in1 * s1) * imm2).astype(_f32))
OP_AXMBY2 = _reg_op(
    "ANT_AXMBY2", (Src0 * C0 - Src1 * C1) * C2,
    lambda in0, in1, s0, s1, imm2: ((in0 * s0 - in1 * s1) * imm2).astype(_f32))
OP_SQDIFF = _reg_op(
    "ANT_SQDIFF", dve_sq(Src0) - dve_sq(Src1),
    lambda in0, in1, s0, s1, imm2: (in0 * in0 - in1 * in1).astype(_f32))


def _xy2_body():
    t = Src0 * Src1
    return t + t


OP_XY2 = _reg_op(
    "ANT_XY2", _xy2_body(),
    lambda in0, in1, s0, s1, imm2: (2.0 * in0 * in1).astype(_f32))
OP_WHERE = _reg_op(
    "ANT_WHERE", dve_select(C0, Src0, Src1),
    lambda in0, in1, s0, s1, imm2: np.where(
        s0 != 0, in0, in1).astype(_f32))
OP_WHERENEG = _reg_op(
    "ANT_WHERENEG", dve_select(C0, -Src0, Src1),
    lambda in0, in1, s0, s1, imm2: np.where(
        s0 != 0, -in0, in1).astype(_f32))
OP_SQSUM = _reg_op(
    "ANT_SQSUM", dve_sq(Src0) + dve_sq(Src1),
    lambda in0, in1, s0, s1, imm2: (in0 * in0 + in1 * in1).astype(_f32))
# (ch^2 - sh^2) * w2  — raw cosine
OP_CSUBW = _reg_op(
    "ANT_CSUBW", (dve_sq(Src0) - dve_sq(Src1)) * C0,
    lambda in0, in1, s0, s1, imm2: (
        (in0 * in0 - in1 * in1) * s0).astype(_f32))


def _xyw_body():
    t = Src0 * Src1
    return (t + t) * C0


# 2 * ch * sh * w2 — raw sine
OP_XYW2 = _reg_op(
    "ANT_XYW2", _xyw_body(),
    lambda in0, in1, s0, s1, imm2: (2.0 * in0 * in1 * s0).astype(_f32))
# gate value: ch^2 - gamma*sh^2 (>0 -> use raw angle, else pi/4 fallback)
OP_GATE = _reg_op(
    "ANT_GATE", dve_sq(Src0) - dve_sq(Src1) * C1,
    lambda in0, in1, s0, s1, imm2: (
        in0 * in0 - in1 * in1 * s1).astype(_f32))
# select(g < 0, imm2, raw)
OP_SELPOS = _reg_op(
    "ANT_SELPOS", dve_select(Src1 < C1, C2, Src0),
    lambda in0, in1, s0, s1, imm2: np.where(
        in1 < s1, imm2, in0).astype(_f32))


def _axpb4y_body():
    t = Src1 * C1
    u = t + t
    return Src0 * C0 + (u + u)


# d' = c2*d + 4*s2*pt
OP_AXPB4Y = _reg_op(
    "ANT_AXPB4Y", _axpb4y_body(),
    lambda in0, in1, s0, s1, imm2: (in0 * s0 + 4.0 * in1 * s1).astype(_f32))
# pt' = c2*pt - 0.25*s2*d   (imm2 carries the 0.25)
OP_AXMBYC = _reg_op(
    "ANT_AXMBYC", Src0 * C0 - (Src1 * C1) * C2,
    lambda in0, in1, s0, s1, imm2: (in0 * s0 - in1 * s1 * imm2).astype(_f32))

F32 = mybir.dt.float32
BF16 = mybir.dt.bfloat16
F16 = mybir.dt.float16
F8 = mybir.dt.float8e4
AF = mybir.ActivationFunctionType
ALU = mybir.AluOpType
AX = mybir.AxisListType
DR = mybir.MatmulPerfMode.DoubleRow

N_CORES = 8
D = 512
SAMPLES = 256          # b*v
TOK = 256              # tokens per sample
S_CORE = SAMPLES // N_CORES       # 32 samples per core
T_CORE = S_CORE * TOK             # 8192 token rows per core
T_TILE = 512
N_TILES = T_CORE // T_TILE        # 16
S_TILE = T_TILE // TOK            # 2 samples per token tile

T_SUP = 1024           # supertile: 2 token-tiles drained with wide ops
N_SUP = T_CORE // T_SUP           # 8
S_SUP = T_SUP // TOK              # 4 samples per supertile

N_ROT = 8              # lean Jacobi rotations (sim: rel_err ~6e-3)

GAMMA = float(3.0 + 2.0 * np.sqrt(2.0))
CQ45 = float(np.cos(np.pi / 4))


# ---------------------------------------------------------------------------
# small-op emitter for the SVD tail: SSA-style column allocation on a scratch
# tile; every value is an AP (or list of APs).
# ---------------------------------------------------------------------------
class Emit:
    def __init__(self, nc, pool):
        self.nc = nc
        self.scr = pool.tile([32, 2048], F32, tag="svd_scratch",
                             name="svd_scratch")
        self.ptr = 0

    def new(self, n=1):
        c = self.ptr
        self.ptr += n
        assert self.ptr <= 2048, "svd scratch overflow"
        return self.scr[:, c:c + n]

    def tt(self, op, a, b, n=1):
        o = self.new(n)
        self.nc.vector.tensor_tensor(o, a, b, op)
        return o

    def tt3(self, op, a, b, n=9):
        o = self.new(n)
        self.nc.vector.tensor_tensor(
            o.rearrange("p (i j) -> p i j", i=3, j=n // 3), a, b, op)
        return o

    def ts(self, op, a, s, n=1):
        o = self.new(n)
        self.nc.vector.tensor_scalar(o, a, s, None, op)
        return o

    def stt(self, a, scal, b, op0, op1, n=1):
        o = self.new(n)
        self.nc.vector.scalar_tensor_tensor(o, a, scal, b, op0=op0, op1=op1)
        return o

    # --- gpsimd variant (tensor_tensor only; Pool supports no Ptr ops) ---
    def gtt(self, op, a, b, n=1):
        o = self.new(n)
        self.nc.gpsimd.tensor_tensor(o, a, b, op)
        return o

    def rsqrt(self, a, n=1):
        t = self.new(n)
        self.nc.scalar.activation(t, a, AF.Sqrt)
        o = self.new(n)
        self.nc.vector.reciprocal(o, t)
        return o

    def cdve(self, op, in0, in1, s0=0.0, s1=0.0, imm2=0.0, n=1, out=None):
        if out is None:
            out = self.new(n)
        self.nc.vector._custom_dve(op, out=out, in0=in0, in1=in1,
                                   s0=s0, s1=s1, imm2=imm2)
        return out

    def const(self, val, n=1):
        o = self.new(n)
        self.nc.vector.memset(o, val)
        return o


def _bcast_r(ap3):
    return ap3.unsqueeze(2).broadcast_to([32, 3, 3])


def _bcast_l(ap3):
    return ap3.unsqueeze(1).broadcast_to([32, 3, 3])


def emit_svd_so3(nc, em, m_ap, pose_tile):
    """m_ap: [32,9] raw 3x3 per sample (row-major). Writes the SO(3)
    projection into pose_tile columns (4r+c for r,c in 0..2).

    Lean d-tracking Jacobi: state is (d01,d12,d02) eigenvalue differences and
    (p01,p12,p02) halved off-diagonals; 15 DVE ops per rotation."""
    # --- row normalize ---
    sq = em.tt(ALU.mult, m_ap, m_ap, 9)
    t = em.tt(ALU.add, sq[:, 0:9:3], sq[:, 1:9:3], 3)
    r2 = em.tt(ALU.add, t, sq[:, 2:9:3], 3)
    r2c = em.ts(ALU.max, r2, 1e-24, 3)
    rinv = em.rsqrt(r2c, 3)
    A = em.tt3(ALU.mult, m_ap.rearrange("p (r c) -> p r c", r=3, c=3),
               _bcast_r(rinv), 9)

    # --- S = A^T A (s_ij at col 3i+j) ---
    terms = []
    for r in range(3):
        arow = A[:, 3 * r:3 * r + 3]
        terms.append(em.tt3(ALU.mult, _bcast_r(arow), _bcast_l(arow), 9))
    s01 = em.tt(ALU.add, terms[0], terms[1], 9)
    S9 = em.tt(ALU.add, s01, terms[2], 9)

    # d/pt state (SSA-tracked APs)
    d01 = em.tt(ALU.subtract, S9[:, 0:1], S9[:, 4:5])
    d12 = em.tt(ALU.subtract, S9[:, 4:5], S9[:, 8:9])
    d02 = em.tt(ALU.add, d01, d12)
    p01 = em.ts(ALU.mult, S9[:, 1:2], 0.5)
    p12 = em.ts(ALU.mult, S9[:, 5:6], 0.5)
    p02 = em.ts(ALU.mult, S9[:, 2:3], 0.5)

    # V columns as [32,3] blocks, init = identity
    Vc = []
    for j in range(3):
        vj = em.new(3)
        nc.vector.memset(vj, 0.0)
        nc.vector.memset(vj[:, j:j + 1], 1.0)
        Vc.append(vj)

    st = {'d01': d01, 'd12': d12, 'd02': d02,
          'p01': p01, 'p12': p12, 'p02': p02}

    def angle(ch, sh):
        # gate condition gamma*sh^2 >= ch^2 is exactly craw <= cos(pi/4),
        # so the pi/4 fallback is a clamp on craw and a craw-keyed select.
        ssum = em.cdve(OP_SQSUM, ch, sh)
        w2 = em.new(1)
        nc.vector.reciprocal(w2, ssum)
        craw = em.cdve(OP_CSUBW, ch, sh, s0=w2)
        sraw = em.cdve(OP_XYW2, ch, sh, s0=w2)
        c = em.ts(ALU.max, craw, CQ45)
        s = em.cdve(OP_SELPOS, sraw, craw, s1=CQ45, imm2=CQ45)
        c2 = em.cdve(OP_SQDIFF, c, s)
        s2 = em.cdve(OP_XY2, c, s)
        return c, s, c2, s2

    def np_pair(pa, pb_, c, s):
        """Rotate the r-row p~ pair (feeds the next rotation's angle; fused
        custom ops on DVE beat a 6-op GPSIMD chain plus handoff)."""
        na = em.cdve(OP_AXPBY, pa, pb_, s0=c, s1=s)
        nb = em.cdve(OP_AXMBY, pb_, pa, s0=c, s1=s)
        return na, nb

    def vup(p, q, c, s):
        # V rotation runs on GPSIMD, off the DVE critical chain (the V
        # columns are only consumed after the rotation loop).
        cb = c.broadcast_to([32, 3])
        sb = s.broadcast_to([32, 3])
        t1 = em.gtt(ALU.mult, Vc[q], sb, 3)
        t2 = em.gtt(ALU.mult, Vc[p], cb, 3)
        nvp = em.gtt(ALU.add, t2, t1, 3)
        t3 = em.gtt(ALU.mult, Vc[p], sb, 3)
        t4 = em.gtt(ALU.mult, Vc[q], cb, 3)
        nvq = em.gtt(ALU.subtract, t4, t3, 3)
        Vc[p], Vc[q] = nvp, nvq

    for k in range(N_ROT):
        rt = k % 3
        if rt == 0:     # (p,q,r) = (0,1,2)
            c, s, c2, s2 = angle(st['d01'], st['p01'])
            dd = em.cdve(OP_AXPB4Y, st['d01'], st['p01'], s0=c2, s1=s2)
            pn = em.cdve(OP_AXMBYC, st['p01'], st['d01'], s0=c2, s1=s2,
                         imm2=0.25)
            t_ = em.tt(ALU.add, st['d02'], st['d12'])
            nd02 = em.cdve(OP_AXPBY, t_, dd, s0=0.5, s1=0.5)
            nd12 = em.cdve(OP_AXMBY, t_, dd, s0=0.5, s1=0.5)
            np02, np12 = np_pair(st['p02'], st['p12'], c, s)
            st.update(d01=dd, p01=pn, d02=nd02, d12=nd12, p02=np02, p12=np12)
            vup(0, 1, c, s)
        elif rt == 1:   # (1,2,0)
            c, s, c2, s2 = angle(st['d12'], st['p12'])
            dd = em.cdve(OP_AXPB4Y, st['d12'], st['p12'], s0=c2, s1=s2)
            pn = em.cdve(OP_AXMBYC, st['p12'], st['d12'], s0=c2, s1=s2,
                         imm2=0.25)
            t_ = em.tt(ALU.add, st['d01'], st['d02'])
            nd01 = em.cdve(OP_AXMBY, t_, dd, s0=0.5, s1=0.5)
            nd02 = em.cdve(OP_AXPBY, t_, dd, s0=0.5, s1=0.5)
            np01, np02 = np_pair(st['p01'], st['p02'], c, s)
            st.update(d12=dd, p12=pn, d01=nd01, d02=nd02, p01=np01, p02=np02)
            vup(1, 2, c, s)
        else:           # (0,2,1)
            c, s, c2, s2 = angle(st['d02'], st['p02'])
            dd = em.cdve(OP_AXPB4Y, st['d02'], st['p02'], s0=c2, s1=s2)
            pn = em.cdve(OP_AXMBYC, st['p02'], st['d02'], s0=c2, s1=s2,
                         imm2=0.25)
            t_ = em.tt(ALU.subtract, st['d01'], st['d12'])
            nd01 = em.cdve(OP_AXPBY, t_, dd, s0=0.5, s1=0.5)
            nd12 = em.cdve(OP_AXMBY, dd, t_, s0=0.5, s1=0.5)
            np01, np12 = np_pair(st['p01'], st['p12'], c, s)
            st.update(d02=dd, p02=pn, d01=nd01, d12=nd12, p01=np01, p12=np12)
            vup(0, 2, c, s)

    # --- sort eigenpairs descending (det(V) stays +1 via column negation) ---
    def cond_swap(i, j):
        key_ij = f'd{i}{j}'
        k = 3 - i - j
        key_ik = f'd{min(i,k)}{max(i,k)}'
        key_jk = f'd{min(j,k)}{max(j,k)}'
        mask = em.ts(ALU.is_lt, st[key_ij], 0.0)
        nij = em.cdve(OP_WHERENEG, st[key_ij], st[key_ij], s0=mask)
        nik = em.cdve(OP_WHERE, st[key_jk], st[key_ik], s0=mask)
        njk = em.cdve(OP_WHERE, st[key_ik], st[key_jk], s0=mask)
        st[key_ij], st[key_ik], st[key_jk] = nij, nik, njk
        # V swap on GPSIMD: vi' = vi + m*(vj-vi); vj' = vj - m*(vi+vj)
        m3 = mask.broadcast_to([32, 3])
        dv = em.gtt(ALU.subtract, Vc[j], Vc[i], 3)
        md = em.gtt(ALU.mult, dv, m3, 3)
        vi = em.gtt(ALU.add, Vc[i], md, 3)
        sv = em.gtt(ALU.add, Vc[i], Vc[j], 3)
        ms = em.gtt(ALU.mult, sv, m3, 3)
        vj = em.gtt(ALU.subtract, Vc[j], ms, 3)
        Vc[i], Vc[j] = vi, vj

    cond_swap(0, 1)
    cond_swap(1, 2)
    cond_swap(0, 1)

    # --- B columns (j=0,1): b_j[r] = sum_c A[r][c] * V[c][j] ---
    Astr = [A[:, c:c + 7:3] for c in range(3)]

    def bcol(j):
        t0 = em.cdve(OP_AXPBY, Astr[0], Astr[1],
                     s0=Vc[j][:, 0:1], s1=Vc[j][:, 1:2], n=3)
        return em.stt(Astr[2], Vc[j][:, 2:3], t0, ALU.mult, ALU.add, 3)

    def bcol_gps(j):
        t0 = em.gtt(ALU.mult, Astr[0],
                    Vc[j][:, 0:1].broadcast_to([32, 3]), 3)
        t1 = em.gtt(ALU.mult, Astr[1],
                    Vc[j][:, 1:2].broadcast_to([32, 3]), 3)
        t01 = em.gtt(ALU.add, t0, t1, 3)
        t2 = em.gtt(ALU.mult, Astr[2],
                    Vc[j][:, 2:3].broadcast_to([32, 3]), 3)
        return em.gtt(ALU.add, t01, t2, 3)

    b0 = bcol(0)
    b1 = bcol_gps(1)

    def normalize(v3):
        sqv = em.tt(ALU.mult, v3, v3, 3)
        n_ = em.tt(ALU.add, sqv[:, 0:1], sqv[:, 1:2])
        n_ = em.tt(ALU.add, n_, sqv[:, 2:3])
        nc_ = em.ts(ALU.max, n_, 1e-30)
        inv = em.rsqrt(nc_)
        return em.ts(ALU.mult, v3, inv, 3)

    u1 = normalize(b0)
    p_ = em.tt(ALU.mult, u1, b1, 3)
    d_ = em.tt(ALU.add, p_[:, 0:1], p_[:, 1:2])
    d_ = em.tt(ALU.add, d_, p_[:, 2:3])
    dneg = em.ts(ALU.mult, d_, -1.0)
    b2o = em.stt(u1, dneg, b1, ALU.mult, ALU.add, 3)
    u2 = normalize(b2o)
    u3 = em.new(3)
    for k, (i1, i2) in enumerate(((1, 2), (2, 0), (0, 1))):
        em.cdve(OP_AXMBY, u1[:, i1:i1 + 1], u1[:, i2:i2 + 1],
                s0=u2[:, i2:i2 + 1], s1=u2[:, i1:i1 + 1],
                out=u3[:, k:k + 1])

    # --- R = u1 v1^T + u2 v2^T + u3 v3^T ---
    t0 = em.tt3(ALU.mult, _bcast_r(u1), _bcast_l(Vc[0]), 9)
    t1 = em.tt3(ALU.mult, _bcast_r(u2), _bcast_l(Vc[1]), 9)
    t01 = em.tt(ALU.add, t0, t1, 9)
    t2 = em.tt3(ALU.mult, _bcast_r(u3), _bcast_l(Vc[2]), 9)
    pose_R = pose_tile[:].rearrange("p (r c) -> p r c", r=4, c=4)[:, 0:3, 0:3]
    nc.vector.tensor_tensor(
        pose_R, t01.rearrange("p (r c) -> p r c", r=3, c=3),
        t2.rearrange("p (r c) -> p r c", r=3, c=3), ALU.add)


# ---------------------------------------------------------------------------
# kernel build
# ---------------------------------------------------------------------------
def build_nc():
    nc = bacc.Bacc("TRN2", target_bir_lowering=False)

    xT8 = nc.dram_tensor("xT8", [D, T_CORE], F8, kind="ExternalInput")
    w8 = nc.dram_tensor("w8", [6, 128, 2048], F8, kind="ExternalInput")
    bs = nc.dram_tensor("bs", [6, D], F32, kind="ExternalInput")
    x0s = nc.dram_tensor("x0s", [128, 4 * S_CORE], F32, kind="ExternalInput")
    mwt = nc.dram_tensor("mwt", [2, D, D], BF16, kind="ExternalInput")
    mbs = nc.dram_tensor("mbs", [2, D], F32, kind="ExternalInput")
    hwT = nc.dram_tensor("hwT", [D, 12], BF16, kind="ExternalInput")
    hb = nc.dram_tensor("hb", [S_CORE, 12], F32, kind="ExternalInput")
    pose = nc.dram_tensor("pose", [S_CORE, 16], F32, kind="ExternalOutput")

    with tile.TileContext(nc) as tc:
        with (
            tc.tile_pool(name="wp", bufs=1) as wpool,
            tc.tile_pool(name="xp", bufs=3) as xpool,
            tc.tile_pool(name="hp", bufs=2) as hpool,
            tc.tile_pool(name="h3p", bufs=2) as h3pool,
            tc.tile_pool(name="pp", bufs=1) as ppool,
            tc.tile_pool(name="ps", bufs=4, space="PSUM") as pspool,
            tc.tile_pool(name="sm", bufs=1) as smpool,
        ):
            # warm the ACT function table while DMAs stream
            warm = smpool.tile([32, 1], F32, tag="warm", name="warm")
            nc.vector.memset(warm[:], 0.0)
            nc.scalar.activation(warm[:], warm[:], AF.Relu)

            # ---- first x supertile + layer-0 weights first, chunked across
            # three DMA queues so the PE can start right after the preamble
            xt0 = xpool.tile([128, 4 * T_SUP], F8, tag="xt", name="xt")
            w_sb = [wpool.tile([128, 2048], F8, tag=f"w{l}", name=f"w{l}")
                    for l in range(6)]
            b_sb = wpool.tile([128, 24], F32, tag="b", name="b_sb")
            # x chunks first on gpsimd/scalar, w0 chunk-granular on sync so
            # the first (o0,kp0) matmul can fire ~2us after the preamble
            for k, eng in enumerate((nc.gpsimd, nc.scalar, nc.gpsimd,
                                     nc.scalar)):
                for h in range(2):  # halves: all 4 chunks land ~2 transfers in
                    eng.dma_start(
                        xt0[:, T_SUP * k + 512 * h:T_SUP * k + 512 * (h + 1)],
                        xT8[128 * k:128 * (k + 1), 512 * h:512 * (h + 1)])
            for c in range(8):
                nc.sync.dma_start(w_sb[0][:, 256 * c:256 * (c + 1)],
                                  w8[0, :, 256 * c:256 * (c + 1)])
            nc.scalar.dma_start(w_sb[1][:], w8[1])
            nc.gpsimd.dma_start(w_sb[2][:], w8[2])
            for l in range(6):
                nc.sync.dma_start(b_sb[:, 4 * l:4 * l + 4],
                                  bs[l].rearrange("(o p) -> p o", p=128, o=4))
            nc.scalar.dma_start(w_sb[3][:], w8[3])
            nc.gpsimd.dma_start(w_sb[4][:], w8[4])
            nc.sync.dma_start(w_sb[5][:], w8[5])
            x0s_sb = wpool.tile([128, 4 * S_CORE], F32, tag="x0s",
                                name="x0s_sb")
            nc.sync.dma_start(x0s_sb[:], x0s[:])
            mw_sb = [wpool.tile([128, 2048], BF16, tag=f"mw{l}",
                                name=f"mw{l}") for l in range(2)]
            for l in range(2):
                for k in range(4):
                    nc.sync.dma_start(
                        mw_sb[l][:, D * k:D * (k + 1)],
                        mwt[l, 128 * k:128 * (k + 1), :])
            mb_sb = wpool.tile([128, 8], F32, tag="mb", name="mb_sb")
            for l in range(2):
                nc.sync.dma_start(mb_sb[:, 4 * l:4 * l + 4],
                                  mbs[l].rearrange("(o p) -> p o", p=128, o=4))
            hw_sb = wpool.tile([128, 48], BF16, tag="hw", name="hw_sb")
            for k in range(4):
                nc.sync.dma_start(hw_sb[:, 12 * k:12 * (k + 1)],
                                  hwT[128 * k:128 * (k + 1), :])
            hb_sb = wpool.tile([32, 12], F32, tag="hbt", name="hb_sb")
            nc.sync.dma_start(hb_sb[:], hb[:])

            # per-tile h3 sums for the two blocks: [128, 4k x 32 samples].
            # pb2 partials accumulate in fp16: 2-byte operands give the DVE
            # reduce its 2x mode; the ~0.03 ulp on ~50-magnitude sums is
            # ~2e-4 relative after the /256 pooling divide.
            pb1 = ppool.tile([128, 4 * S_CORE], F32, tag="pb1", name="pb1")
            pb2 = ppool.tile([128, 4 * S_CORE], F16, tag="pb2", name="pb2")

            def wap(l, o, kp):
                c0 = (o * 2 + kp) * 256
                return w_sb[l][:, c0:c0 + 256].rearrange(
                    "p (i m) -> p i m", i=2)

            def rhs(t, kp, th):
                # kp-pair chunks of a [128, 4*T_SUP] supertile, token half th
                return t[:, 2 * T_SUP * kp:2 * T_SUP * (kp + 1)].rearrange(
                    "p (i n) -> p i n", i=2)[:, :, 512 * th:512 * (th + 1)]

            def relu_drain(engine, h_slice, ps, bias_ap):
                if engine == 'act':
                    nc.scalar.activation(h_slice, ps[:], AF.Relu,
                                         bias=bias_ap, scale=1.0)
                else:
                    nc.vector.tensor_scalar(h_slice, ps[:], bias_ap, 0.0,
                                            ALU.add, ALU.max)

            # engine assignment per (layer, o): chunks 0 and 1 of every layer
            # drain on different engines so the next layer's kp0 matmuls
            # (which need both) never wait on one serial drain queue.
            # Totals: ACT 14, DVE 10 (+2 pooling reduces on DVE).
            ENG = {
                0: ['act', 'dve', 'act', 'act'],
                1: ['dve', 'act', 'dve', 'dve'],
                2: ['act', 'dve', 'act', 'act'],
                3: ['act', 'dve', 'act', 'act'],
                4: ['act', 'dve', 'act', 'act'],
                5: ['dve', 'act', 'dve', 'dve'],
            }

            # ---- main loop over supertiles (1024 tokens each); x tiles are
            # prefetched one supertile ahead so l0 never waits on the DMA ----
            def fetch_xt(tj):
                xt_ = xpool.tile([128, 4 * T_SUP], F8, tag="xt", name="xt")
                for k in range(4):
                    nc.gpsimd.dma_start(
                        xt_[:, T_SUP * k:T_SUP * (k + 1)],
                        xT8[128 * k:128 * (k + 1),
                            T_SUP * tj:T_SUP * (tj + 1)])
                return xt_

            xts = {0: xt0, 1: fetch_xt(1)}
            pending_red = None
            for ti in range(N_SUP):
                xt = xts.pop(ti)
                if ti + 2 < N_SUP:
                    xts[ti + 2] = fetch_xt(ti + 2)

                def emit_reduce(h3t, pb, tj, o=None):
                    """Per-supertile h3 pooling sum; o=None reduces all 4
                    chunks in one 4D op, else a single o-chunk."""
                    with nc.allow_low_precision("fp16 pooling partials"):
                        if o is None:
                            nc.vector.tensor_reduce(
                                pb[:].rearrange("p (o s) -> p o s", o=4,
                                                s=S_CORE)[:, :, S_SUP * tj:
                                                          S_SUP * (tj + 1)],
                                h3t[:].rearrange("p (o g t) -> p o g t", o=4,
                                                 g=S_SUP),
                                axis=AX.X, op=ALU.add)
                        else:
                            nc.vector.tensor_reduce(
                                pb[:, S_CORE * o + S_SUP * tj:
                                   S_CORE * o + S_SUP * (tj + 1)],
                                h3t[:, T_SUP * o:T_SUP * (o + 1)].rearrange(
                                    "p (g t) -> p g t", g=S_SUP),
                                axis=AX.X, op=ALU.add)

                def run_layer(l, src, out_dtype=F8, tag="h", extra_src=None,
                              red_to=None):
                    pool_ = h3pool if l in (2, 5) else hpool
                    h_out = pool_.tile([128, 4 * T_SUP], out_dtype,
                                       tag=tag, name=f"h{l}")
                    for o in range(4):
                        ps = pspool.tile([128, T_SUP], F32, tag="ps",
                                         name="ps")
                        srcs = [src] if extra_src is None else [src,
                                                                extra_src]
                        n_mm = 4 * len(srcs)
                        mi = 0
                        for s_ in srcs:
                            for kp in range(2):
                                for th in range(2):
                                    nc.tensor.matmul(
                                        ps[:, 512 * th:512 * (th + 1)],
                                        wap(l, o, kp), rhs(s_, kp, th),
                                        start=(mi < 2),
                                        stop=(mi >= n_mm - 2),
                                        perf_mode=DR)
                                    mi += 1
                        relu_drain(ENG[l][o],
                                   h_out[:, T_SUP * o:T_SUP * (o + 1)],
                                   ps, b_sb[:, 4 * l + o:4 * l + o + 1])
                        if red_to is not None:
                            emit_reduce(h_out, red_to, ti, o=o)
                    return h_out

                h1 = run_layer(0, xt)
                h2 = run_layer(1, h1)
                if pending_red is not None:
                    # deferred h3b reduce of the previous supertile: lands
                    # behind this supertile's l1 drains in the DVE queue so
                    # the PE never waits on it
                    emit_reduce(pending_red[0], pb2, pending_red[1])
                    pending_red = None
                h3a = run_layer(2, h2, tag="h3a")          # fp8: matmul input
                emit_reduce(h3a, pb1, ti)
                g1 = run_layer(3, xt, extra_src=h3a)        # fused residual
                g2 = run_layer(4, g1)
                last = ti == N_SUP - 1
                h3b = run_layer(5, g2, out_dtype=BF16, tag="h3b",
                                red_to=pb2 if last else None)
                if not last:
                    pending_red = (h3b, ti)

            # ---- pooled = x0s + pb1 + pb2, per k-chunk so the first tail
            # matmuls start before the whole combine finishes ----
            pool_f32 = smpool.tile([128, 4 * S_CORE], F32, tag="poolf",
                                   name="pool_f32")
            pool_bf = smpool.tile([128, 4 * S_CORE], BF16, tag="poolb",
                                  name="pool_bf")
            for k in range(4):
                sl = slice(S_CORE * k, S_CORE * (k + 1))
                nc.gpsimd.tensor_tensor(pool_f32[:, sl], pb1[:, sl],
                                        pb2[:, sl], ALU.add)
                nc.gpsimd.tensor_tensor(pool_bf[:, sl], pool_f32[:, sl],
                                        x0s_sb[:, sl], ALU.add)

            # ---- tail MLPs (bf16), psum reused from the main pool ----
            f_prev = pool_bf
            scales = [1.0 / TOK, 1.0]
            for l in range(2):
                f_out = smpool.tile([128, 4 * S_CORE], BF16, tag=f"f{l}",
                                    name=f"f{l}")
                for o in range(4):
                    ps_w = pspool.tile([128, T_SUP], F32, tag="ps",
                                       name="pst")
                    ps = ps_w[:, 0:S_CORE]
                    for k in range(4):
                        nc.tensor.matmul(
                            ps,
                            mw_sb[l][:, D * k + 128 * o:D * k + 128 * (o + 1)],
                            f_prev[:, S_CORE * k:S_CORE * (k + 1)],
                            start=(k == 0), stop=(k == 3))
                    nc.scalar.activation(
                        f_out[:, S_CORE * o:S_CORE * (o + 1)], ps, AF.Relu,
                        bias=mb_sb[:, 4 * l + o:4 * l + o + 1],
                        scale=scales[l])
                f_prev = f_out

            # ---- heads: [32 samples, 12] = t(3) ++ rot(9) ----
            psh_w = pspool.tile([128, T_SUP], F32, tag="ps", name="psh")
            psh = psh_w[0:32, 0:12]
            for k in range(4):
                nc.tensor.matmul(psh,
                                 f_prev[:, S_CORE * k:S_CORE * (k + 1)],
                                 hw_sb[:, 12 * k:12 * (k + 1)],
                                 start=(k == 0), stop=(k == 3))
            mm = smpool.tile([32, 12], F32, tag="mm", name="mm")
            nc.vector.tensor_add(mm[:], psh, hb_sb[:])

            # ---- pose assembly + SVD ----
            pose_t = smpool.tile([32, 16], F32, tag="pose", name="pose_t")
            nc.vector.memset(pose_t[:], 0.0)
            nc.vector.memset(pose_t[:, 15:16], 1.0)
            nc.vector.tensor_copy(
                pose_t[:].rearrange("p (r c) -> p r c", r=4, c=4)[:, 0:3, 3],
                mm[:, 0:3])

            em = Emit(nc, smpool)
            emit_svd_so3(nc, em, mm[:, 3:12], pose_t)

            nc.sync.dma_start(pose[:], pose_t[:])

    nc.compile()
    return nc


_NC_CACHE = None


def _get_nc():
    global _NC_CACHE
    if _NC_CACHE is None:
        _NC_CACHE = build_nc()
    return _NC_CACHE


F8NP = ml_dtypes.float8_e4m3fn
BF16NP = ml_dtypes.bfloat16


def kernel(**inputs):
    feat = np.asarray(inputs["feat"], dtype=np.float32)
    b_, v_, n_, d_ = feat.shape
    xs = feat.reshape(b_ * v_, n_, d_)
    x0sum = xs.sum(axis=1, dtype=np.float32)          # (256, 512)

    # DoubleRow weight prepack: [p, o, kp, i, m] <- wT[128*(2kp+i)+p, 128o+m]
    w8_list = []
    for blk in (1, 2):
        for li in (1, 2, 3):
            wT = np.asarray(inputs[f"r{blk}_w{li}"], np.float32).T
            arr = wT.astype(F8NP).reshape(2, 2, 128, 4, 128)
            arr = np.ascontiguousarray(arr.transpose(2, 3, 0, 1, 4))
            w8_list.append(arr.reshape(128, 2048))
    w8 = np.stack(w8_list)
    bs = np.stack([np.asarray(inputs[f"r{blk}_b{li}"], np.float32)
                   for blk in (1, 2) for li in (1, 2, 3)])
    mwt = np.stack([np.ascontiguousarray(
        np.asarray(inputs[f"m_w{li}"], np.float32).T).astype(BF16NP)
        for li in (1, 2)])
    mbs = np.stack([np.asarray(inputs[f"m_b{li}"], np.float32)
                    for li in (1, 2)])
    hwT = np.ascontiguousarray(np.concatenate(
        [np.asarray(inputs["t_w"], np.float32).T,
         np.asarray(inputs["rot_w"], np.float32).T], axis=1)).astype(BF16NP)
    hb = np.broadcast_to(np.concatenate(
        [np.asarray(inputs["t_b"], np.float32),
         np.asarray(inputs["rot_b"], np.float32)])[None, :],
        (S_CORE, 12)).copy()

    in_maps = []
    for c in range(N_CORES):
        xT8 = np.ascontiguousarray(
            xs[c * S_CORE:(c + 1) * S_CORE].reshape(T_CORE, D).T).astype(F8NP)
        xs_c = x0sum[c * S_CORE:(c + 1) * S_CORE]     # (32, 512)
        x0s = np.ascontiguousarray(
            xs_c.T.reshape(4, 128, S_CORE).transpose(1, 0, 2).reshape(
                128, 4 * S_CORE))
        in_maps.append({
            "xT8": xT8, "w8": w8, "bs": bs, "x0s": x0s, "mwt": mwt,
            "mbs": mbs, "hwT": hwT, "hb": hb,
        })

    nc = _get_nc()
    import os
    kwargs = {}
    if os.environ.get("KERNEL_TRACE") == "1":
        kwargs["trace"] = True
    res = run_bass_kernel_spmd(nc, in_maps, core_ids=list(range(N_CORES)),
                               **kwargs)
    if kwargs.get("trace"):
        kernel.last_results = res
    poses = np.concatenate([r["pose"] for r in res.results], axis=0)
    return poses.reshape(b_, v_, 4, 4)
